# revision 1
# baseline (speedup 1.0000x reference)
"""3-layer GCN + img@pair_embed.T for Trainium2, distributed over 8 NeuronCores.

Strategy (destination-sharded graph parallelism):
  - Each core owns a contiguous slab of destination nodes (3567, padded to 3584).
  - Edges (plus self-loops) are bucketed per 256-destination tile, sorted by
    destination, padded to chunks of 128. Host builds, per edge chunk, a dense
    [128 edges x 256 dests] selection matrix S holding the GCN norm
    coefficients, so segment-sum aggregation becomes TensorE matmuls:
        aggT[f, d] += G[e, f].T @ S[e, d]      (G = gathered source rows)
  - GCN identity A@(X W) == (A@X) W lets layer 1 aggregate 512-wide inputs
    (not 2048-wide outputs).
  - The final  img @ pair_embed.T  folds into layer 3: with
    W3img = W3 @ img.T  [2048, 64], Q = h2 @ W3img, the layer-3 aggregation
    A @ Q directly produces output columns. Layer 3 aggregates 64-wide.
  - Five AllGathers move activations between layers: h1 (4x [3584,512]) and
    Q ([3584,64]).
  - Gathered/exchanged activations and S blocks travel as bf16 (FEAT_BF16
    toggle; halves HBM gather + collective bytes; measured rel err ~2.6e-3).
    GEMM weights are float32r (full PE rate at free-dim>=256, ~12-bit
    mantissa, host-pre-rounded); PSUM accumulation is always fp32.
"""

import numpy as np

from concourse import bacc, bass, mybir
from concourse import tile as tile_mod
from concourse.bass_utils import run_bass_kernel_spmd

# Problem shapes (hardcoded per spec nn_GraphModel_26268019982828)
N = 28535
E = 113000
D = 512
H = 2048
B = 64
N_SKIP = 115 + 245  # attrs + objs; pair nodes are N_SKIP..N-1

NCORES = 8
NODES_PER = -(-N // NCORES)  # 3567
P = 128
DT = 256  # destination tile width
NDT = 14  # dest tiles per core
SLAB = NDT * DT  # 3584 padded dests per core
NFI1 = D // P  # 4 input feature chunks (layer 1)
NFI2 = H // P  # 16 feature chunks (2048)
NG = 4  # h1 column groups (4 x 512)
NFO2 = H // P  # 16 output chunks for layer2 GEMM

f32 = mybir.dt.float32
f32r = mybir.dt.float32r
bf16 = mybir.dt.bfloat16
i32 = mybir.dt.int32
FEAT_BF16 = True  # False -> fp32r feature path (safer numerics, ~2x memory)
FEAT = bf16 if FEAT_BF16 else f32r  # gathered/exchanged activations + S blocks


def _round_fp32r(x: np.ndarray) -> np.ndarray:
    """Round-to-nearest-even fp32 -> fp32r (11-bit mantissa), numpy."""
    u = np.ascontiguousarray(x, dtype=np.float32).view(np.uint32)
    r = u + (0x7FF + ((u >> 12) & np.uint32(1)))
    r &= np.uint32(0xFFFFF000)
    return r.view(np.float32)


def _preprocess(edge_index: np.ndarray):
    """Sort/bucket edges by destination; build gather indices + S blocks.

    Returns (ECH, idxA, idxB, S) with
      idxA [NCORES, NDT, P, ECH] int32 — source node ids (original numbering)
      idxB [NCORES, NDT, P, ECH] int32 — source ids remapped to slab rows
      S    [NCORES, NDT, P, ECH*DT] float32 (fp32r-rounded) — norm matrix
    """
    src = np.concatenate([edge_index[0], np.arange(N, dtype=np.int64)])
    dst = np.concatenate([edge_index[1], np.arange(N, dtype=np.int64)])
    deg = np.bincount(dst, minlength=N).astype(np.float32)  # includes loops
    dinv = (1.0 / np.sqrt(deg)).astype(np.float32)
    norm = (dinv[src] * dinv[dst]).astype(np.float32)

    core = (dst // NODES_PER).astype(np.int64)
    local = (dst - core * NODES_PER).astype(np.int64)
    t_idx = local // DT
    d_local = local % DT
    bucket = core * NDT + t_idx  # global bucket id per edge

    order = np.argsort(bucket, kind="stable")
    src_s = src[order]
    bucket_s = bucket[order]
    dl_s = d_local[order]
    norm_s = norm[order]

    counts = np.bincount(bucket_s, minlength=NCORES * NDT)
    ECH = int(-(-counts.max() // P))

    idxA = np.zeros((NCORES, NDT, P, ECH), dtype=np.int32)
    idxB = np.zeros((NCORES, NDT, P, ECH), dtype=np.int32)
    S = np.zeros((NCORES, NDT, P, ECH * DT), dtype=np.float32)

    # position of each edge within its bucket
    starts = np.zeros(NCORES * NDT + 1, dtype=np.int64)
    np.cumsum(counts, out=starts[1:])
    pos = np.arange(len(bucket_s)) - starts[bucket_s]
    c_idx = pos // P  # edge chunk within bucket
    e_idx = pos % P  # partition row within chunk

    ci = bucket_s // NDT  # core
    ti = bucket_s % NDT  # dtile
    srcB = (src_s // NODES_PER) * SLAB + (src_s % NODES_PER)
    idxA[ci, ti, e_idx, c_idx] = src_s.astype(np.int32)
    idxB[ci, ti, e_idx, c_idx] = srcB.astype(np.int32)
    S[ci, ti, e_idx, c_idx * DT + dl_s] = norm_s
    S = _round_fp32r(S)
    return ECH, idxA, idxB, S


def _build(ECH: int, use_b1: bool, use_b2: bool, _phases: int = 3):
    nc = bacc.Bacc("TRN2", target_bir_lowering=False, num_devices=NCORES)

    nodes_t = nc.dram_tensor("nodes", [N, D], FEAT, kind="ExternalInput")
    w1_t = nc.dram_tensor("W1", [D, H], f32r, kind="ExternalInput")
    w2_t = nc.dram_tensor("W2", [H, H], f32r, kind="ExternalInput")
    w3i_t = nc.dram_tensor("W3img", [H, B], f32r, kind="ExternalInput")
    b1_t = nc.dram_tensor("b1", [1, H], f32r, kind="ExternalInput")
    b2_t = nc.dram_tensor("b2", [P, NFI2], f32r, kind="ExternalInput")
    idxA_t = nc.dram_tensor("idxA", [NDT, P, ECH], i32, kind="ExternalInput")
    idxB_t = nc.dram_tensor("idxB", [NDT, P, ECH], i32, kind="ExternalInput")
    s_tab = nc.dram_tensor("S", [NDT, P, ECH * DT], FEAT, kind="ExternalInput")
    out_t = nc.dram_tensor("out", [B, SLAB], f32, kind="ExternalOutput")

    h1p = [nc.dram_tensor(f"h1p{g}", [SLAB, D], FEAT) for g in range(NG)]
    h1pf = [
        nc.dram_tensor(f"h1pf{g}", [SLAB * NCORES, D], FEAT, addr_space="Shared")
        for g in range(NG)
    ]
    q_slab = nc.dram_tensor("q_slab", [SLAB, B], FEAT)
    q_full = nc.dram_tensor("q_full", [SLAB * NCORES, B], FEAT, addr_space="Shared")

    rg = [list(range(NCORES))]

    with tile_mod.TileContext(nc) as tc:
        with (
            tc.tile_pool(name="w", bufs=16) as wp,
            tc.tile_pool(name="gio", bufs=4) as gp,
            tc.tile_pool(name="stab", bufs=2) as sp,
            tc.tile_pool(name="agg", bufs=16) as ap,
            tc.tile_pool(name="small", bufs=3) as mp,
            tc.tile_pool(name="consts", bufs=1) as cp,
            tc.tile_pool(name="pagg", bufs=4, space="PSUM") as pag,
            tc.tile_pool(name="pz", bufs=2, space="PSUM") as pzp,
            tc.tile_pool(name="pq", bufs=2, space="PSUM") as pqp,
        ):
            # --- resident weights ---
            w1sb = []
            for fi in range(NFI1):
                w = wp.tile([P, H], f32r, tag="w", name="wsb")
                nc.sync.dma_start(out=w[:], in_=w1_t[fi * P : (fi + 1) * P, :])
                w1sb.append(w)
            w2sb = []
            for fi in range(NFI2):
                w = wp.tile([P, H], f32r, tag="w", name="wsb")
                nc.sync.dma_start(out=w[:], in_=w2_t[fi * P : (fi + 1) * P, :])
                w2sb.append(w)
            w3sb = []
            for fo in range(NFI2):
                w = wp.tile([P, B], f32r, tag="w3", name="w3sb")
                nc.sync.dma_start(out=w[:], in_=w3i_t[fo * P : (fo + 1) * P, :])
                w3sb.append(w)
            if use_b1:
                b1sb = cp.tile([1, H], f32r)
                nc.sync.dma_start(out=b1sb[:], in_=b1_t[:])
                ones1 = cp.tile([1, P], f32r)
                nc.gpsimd.memset(ones1[:], 1.0)
            if use_b2:
                b2sb = cp.tile([P, NFI2], f32r)
                nc.sync.dma_start(out=b2sb[:], in_=b2_t[:])

            relu = mybir.ActivationFunctionType.Relu

            # ---------------- Layer 1 ----------------
            for t in range(NDT if _phases >= 1 else 0):
                idx_t = mp.tile([P, ECH], i32, tag="idx")
                nc.sync.dma_start(out=idx_t[:], in_=idxA_t[t])
                s_t = sp.tile([P, ECH * DT], FEAT, tag="s")
                nc.sync.dma_start(out=s_t[:], in_=s_tab[t])

                pa = [pag.tile([P, DT], f32, tag="pagg", name="pa") for _ in range(NFI1)]
                for c in range(ECH):
                    g = gp.tile([P, D], FEAT, tag="g")
                    nc.gpsimd.indirect_dma_start(
                        out=g[:],
                        out_offset=None,
                        in_=nodes_t[:],
                        in_offset=bass.IndirectOffsetOnAxis(
                            ap=idx_t[:, c : c + 1], axis=0
                        ),
                    )
                    for fi in range(NFI1):
                        nc.tensor.matmul(
                            out=pa[fi][:],
                            lhsT=g[:, fi * P : (fi + 1) * P],
                            rhs=s_t[:, c * DT : (c + 1) * DT],
                            start=(c == 0),
                            stop=(c == ECH - 1),
                        )
                aggT = []
                for fi in range(NFI1):
                    a = ap.tile([P, DT], f32r, tag="aggT", name="aggTt")
                    nc.vector.tensor_copy(out=a[:], in_=pa[fi][:])
                    aggT.append(a)

                for dh in range(2):
                    for fo in range(NG):  # 4 output blocks of 512
                        pz = pzp.tile([P, D], f32, tag="pz")
                        if use_b1:
                            nc.tensor.matmul(
                                out=pz[:],
                                lhsT=ones1[:1, :],
                                rhs=b1sb[:1, fo * D : (fo + 1) * D],
                                start=True,
                                stop=False,
                            )
                        for fi in range(NFI1):
                            nc.tensor.matmul(
                                out=pz[:],
                                lhsT=aggT[fi][:, dh * P : (dh + 1) * P],
                                rhs=w1sb[fi][:, fo * D : (fo + 1) * D],
                                start=(fi == 0 and not use_b1),
                                stop=(fi == NFI1 - 1),
                            )
                        h_t = mp.tile([P, D], FEAT, tag="hout")
                        nc.scalar.activation(out=h_t[:], in_=pz[:], func=relu)
                        nc.sync.dma_start(
                            out=h1p[fo][t * DT + dh * P : t * DT + (dh + 1) * P, :],
                            in_=h_t[:],
                        )

            for g_i in range(NG if _phases >= 1.5 else 0):
                nc.gpsimd.collective_compute(
                    "AllGather",
                    mybir.AluOpType.bypass,
                    replica_groups=rg,
                    ins=[h1p[g_i][:]],
                    outs=[h1pf[g_i][:]],
                )

            # ---------------- Layer 2 + Q ----------------
            for t in range(NDT if _phases >= 2 else 0):
                idx_t = mp.tile([P, ECH], i32, tag="idx")
                nc.sync.dma_start(out=idx_t[:], in_=idxB_t[t])
                s_t = sp.tile([P, ECH * DT], FEAT, tag="s")
                nc.sync.dma_start(out=s_t[:], in_=s_tab[t])

                aggT = []
                for g_i in range(NG):
                    pa = [pag.tile([P, DT], f32, tag="pagg", name="pa") for _ in range(NFI1)]
                    for c in range(ECH):
                        g = gp.tile([P, D], FEAT, tag="g")
                        nc.gpsimd.indirect_dma_start(
                            out=g[:],
                            out_offset=None,
                            in_=h1pf[g_i][:],
                            in_offset=bass.IndirectOffsetOnAxis(
                                ap=idx_t[:, c : c + 1], axis=0
                            ),
                        )
                        for j in range(NFI1):
                            nc.tensor.matmul(
                                out=pa[j][:],
                                lhsT=g[:, j * P : (j + 1) * P],
                                rhs=s_t[:, c * DT : (c + 1) * DT],
                                start=(c == 0),
                                stop=(c == ECH - 1),
                            )
                    for j in range(NFI1):
                        a = ap.tile([P, DT], f32r, tag="aggT", name="aggTt")
                        nc.vector.tensor_copy(out=a[:], in_=pa[j][:])
                        aggT.append(a)

                pq = [pqp.tile([P, B], f32, tag="pq", name="pq") for _ in range(2)]
                for fo in range(NFO2):
                    pz = pzp.tile([P, DT], f32, tag="pz")
                    for fi in range(NFI2):
                        nc.tensor.matmul(
                            out=pz[:],
                            lhsT=w2sb[fi][:, fo * P : (fo + 1) * P],
                            rhs=aggT[fi][:],
                            start=(fi == 0),
                            stop=(fi == NFI2 - 1),
                        )
                    h2t = mp.tile([P, DT], f32r, tag="h2")
                    if use_b2:
                        nc.scalar.activation(
                            out=h2t[:], in_=pz[:], func=relu,
                            bias=b2sb[:, fo : fo + 1],
                        )
                    else:
                        nc.scalar.activation(out=h2t[:], in_=pz[:], func=relu)
                    for dh in range(2):
                        nc.tensor.matmul(
                            out=pq[dh][:],
                            lhsT=h2t[:, dh * P : (dh + 1) * P],
                            rhs=w3sb[fo][:],
                            start=(fo == 0),
                            stop=(fo == NFO2 - 1),
                        )
                for dh in range(2):
                    qn = mp.tile([P, B], FEAT, tag="qn")
                    nc.vector.tensor_copy(out=qn[:], in_=pq[dh][:])
                    nc.sync.dma_start(
                        out=q_slab[t * DT + dh * P : t * DT + (dh + 1) * P, :],
                        in_=qn[:],
                    )

            if _phases >= 2:
                nc.gpsimd.collective_compute(
                    "AllGather",
                    mybir.AluOpType.bypass,
                    replica_groups=rg,
                    ins=[q_slab[:]],
                    outs=[q_full[:]],
                )

            # ---------------- Layer 3 (= output) ----------------
            for t in range(NDT if _phases >= 3 else 0):
                idx_t = mp.tile([P, ECH], i32, tag="idx")
                nc.sync.dma_start(out=idx_t[:], in_=idxB_t[t])
                s_t = sp.tile([P, ECH * DT], FEAT, tag="s")
                nc.sync.dma_start(out=s_t[:], in_=s_tab[t])

                pa = pag.tile([B, DT], f32, tag="pagg", name="pa3")
                for c in range(ECH):
                    g = gp.tile([P, B], FEAT, tag="g")
                    nc.gpsimd.indirect_dma_start(
                        out=g[:],
                        out_offset=None,
                        in_=q_full[:],
                        in_offset=bass.IndirectOffsetOnAxis(
                            ap=idx_t[:, c : c + 1], axis=0
                        ),
                    )
                    nc.tensor.matmul(
                        out=pa[:],
                        lhsT=g[:],
                        rhs=s_t[:, c * DT : (c + 1) * DT],
                        start=(c == 0),
                        stop=(c == ECH - 1),
                    )
                o_t = mp.tile([B, DT], f32, tag="ot")
                nc.vector.tensor_copy(out=o_t[:], in_=pa[:])
                nc.sync.dma_start(out=out_t[:, t * DT : (t + 1) * DT], in_=o_t[:])

    nc.finalize()
    return nc


_CACHE: dict = {}


def kernel(**inputs: np.ndarray) -> np.ndarray:
    nodes = np.asarray(inputs["nodes"], dtype=np.float32)
    edge_index = np.asarray(inputs["edge_index"])
    img = np.asarray(inputs["img"], dtype=np.float32)
    W1 = np.asarray(inputs["W1"], dtype=np.float32)
    b1 = np.asarray(inputs["b1"], dtype=np.float32)
    W2 = np.asarray(inputs["W2"], dtype=np.float32)
    b2 = np.asarray(inputs["b2"], dtype=np.float32)
    W3 = np.asarray(inputs["W3"], dtype=np.float32)
    b3 = np.asarray(inputs["b3"], dtype=np.float32)

    ECH, idxA, idxB, S = _preprocess(edge_index)
    if FEAT_BF16:
        S = S.astype(__import__("ml_dtypes").bfloat16)
    use_b1 = bool(np.any(b1))
    use_b2 = bool(np.any(b2))

    key = (ECH, use_b1, use_b2)
    if key not in _CACHE:
        _CACHE[key] = _build(ECH, use_b1, use_b2)
    nc = _CACHE[key]

    w3img = _round_fp32r(W3.astype(np.float32) @ img.astype(np.float32).T)  # [H, B]
    outbias = img @ b3  # [B]

    import ml_dtypes
    feat_np = ml_dtypes.bfloat16 if FEAT_BF16 else np.float32
    nodes_r = nodes.astype(feat_np) if FEAT_BF16 else _round_fp32r(nodes)
    w1_r = _round_fp32r(W1)
    w2_r = _round_fp32r(W2)
    b1_r = _round_fp32r(b1.reshape(1, H))
    b2_r = _round_fp32r(np.ascontiguousarray(b2.reshape(NFI2, P).T))

    in_maps = []
    for k in range(NCORES):
        in_maps.append(
            {
                "nodes": nodes_r,
                "W1": w1_r,
                "W2": w2_r,
                "W3img": w3img,
                "b1": b1_r,
                "b2": b2_r,
                "idxA": np.ascontiguousarray(idxA[k]),
                "idxB": np.ascontiguousarray(idxB[k]),
                "S": np.ascontiguousarray(S[k]),
            }
        )

    res = run_bass_kernel_spmd(nc, in_maps, core_ids=list(range(NCORES)))

    full = np.concatenate([res.results[k]["out"] for k in range(NCORES)], axis=1)
    n_ids = np.arange(N_SKIP, N)
    cols = (n_ids // NODES_PER) * SLAB + (n_ids % NODES_PER)
    out = full[:, cols] + outbias[:, None]
    return out.astype(np.float32)


if __name__ == "__main__":
    # quick self-exercise with random data
    rng = np.random.default_rng(0)
    ins = {
        "nodes": rng.standard_normal((N, D)).astype(np.float32),
        "edge_index": rng.integers(0, N, size=(2, E)).astype(np.int64),
        "img": rng.standard_normal((B, D)).astype(np.float32),
        "W1": (rng.standard_normal((D, H)) * 0.02).astype(np.float32),
        "b1": np.zeros(H, np.float32),
        "W2": (rng.standard_normal((H, H)) * 0.02).astype(np.float32),
        "b2": np.zeros(H, np.float32),
        "W3": (rng.standard_normal((H, D)) * 0.02).astype(np.float32),
        "b3": np.zeros(D, np.float32),
    }
    out = kernel(**ins)
    print("out", out.shape, out.dtype, np.abs(out).mean())



# revision 6
# speedup vs baseline: 1.1132x; 1.1132x over previous
"""3-layer GCN + img@pair_embed.T for Trainium2, distributed over 8 NeuronCores.

Strategy (destination-sharded graph parallelism):
  - Each core owns a contiguous slab of destination nodes (3567, padded to 3584).
  - Edges (plus self-loops) are bucketed per 256-destination tile, sorted by
    destination, padded to chunks of 128. Host builds, per edge chunk, a dense
    [128 edges x 256 dests] selection matrix S holding the GCN norm
    coefficients, so segment-sum aggregation becomes TensorE matmuls:
        aggT[f, d] += G[e, f].T @ S[e, d]      (G = gathered source rows)
  - GCN identity A@(X W) == (A@X) W lets layer 1 aggregate 512-wide inputs.
    Layer-1 source rows are PRE-GATHERED ON HOST (static input X) so layer 1
    needs no on-device indirect DMA at all.
  - h1 is stored as ONE unified [SLAB, 2048] tensor; a single AllGather
    produces [8*SLAB, 2048] and layer-2 gathers fetch full 2048-wide rows in
    one indirect DMA per 128-edge chunk (SWDGE fixed cost amortized 4x).
  - The final  img @ pair_embed.T  folds into layer 3: with
    W3img = W3 @ img.T  [2048, 64], Q = h2 @ W3img, the layer-3 aggregation
    A @ Q directly produces output columns. Layer 3 aggregates 64-wide.
  - Activations/S travel as bf16; W2 is bf16-resident in SBUF (fits alongside
    2048-wide gather tiles); W1/W3img float32r; PSUM accumulation fp32.
"""

import numpy as np

from concourse import bacc, bass, mybir
from concourse import tile as tile_mod
from concourse.bass_utils import run_bass_kernel_spmd

# Problem shapes (hardcoded per spec nn_GraphModel_26268019982828)
N = 28535
E = 113000
D = 512
H = 2048
B = 64
N_SKIP = 115 + 245  # attrs + objs; pair nodes are N_SKIP..N-1

NCORES = 8
NODES_PER = -(-N // NCORES)  # 3567
P = 128
DT = 256  # destination tile width
NDT = 14  # dest tiles per core
SLAB = NDT * DT  # 3584 padded dests per core
NFI1 = D // P  # 4 input feature chunks (layer 1)
NFI2 = H // P  # 16 feature chunks (2048)

f32 = mybir.dt.float32
f32r = mybir.dt.float32r
bf16 = mybir.dt.bfloat16
i32 = mybir.dt.int32


def _round_fp32r(x: np.ndarray) -> np.ndarray:
    """Round-to-nearest-even fp32 -> fp32r (11-bit mantissa), numpy."""
    u = np.ascontiguousarray(x, dtype=np.float32).view(np.uint32)
    r = u + (0x7FF + ((u >> 12) & np.uint32(1)))
    r &= np.uint32(0xFFFFF000)
    return r.view(np.float32)


def _preprocess(edge_index: np.ndarray):
    """Sort/bucket edges by destination; build gather indices + S blocks.

    Returns (ECH, idxA, idxB, S) with
      idxA [NCORES, NDT, P, ECH] int32 — source node ids (original numbering)
      idxB [NCORES, NDT, P, ECH] int32 — source ids remapped to slab rows
      S    [NCORES, NDT, P, ECH*DT] float32 — norm matrix
    """
    src = np.concatenate([edge_index[0], np.arange(N, dtype=np.int64)])
    dst = np.concatenate([edge_index[1], np.arange(N, dtype=np.int64)])
    deg = np.bincount(dst, minlength=N).astype(np.float32)  # includes loops
    dinv = (1.0 / np.sqrt(deg)).astype(np.float32)
    norm = (dinv[src] * dinv[dst]).astype(np.float32)

    core = (dst // NODES_PER).astype(np.int64)
    local = (dst - core * NODES_PER).astype(np.int64)
    t_idx = local // DT
    d_local = local % DT
    bucket = core * NDT + t_idx  # global bucket id per edge

    order = np.argsort(bucket, kind="stable")
    src_s = src[order]
    bucket_s = bucket[order]
    dl_s = d_local[order]
    norm_s = norm[order]

    counts = np.bincount(bucket_s, minlength=NCORES * NDT)
    ECH = int(-(-counts.max() // P))

    idxA = np.zeros((NCORES, NDT, P, ECH), dtype=np.int32)
    idxB = np.zeros((NCORES, NDT, P, ECH), dtype=np.int32)
    S = np.zeros((NCORES, NDT, P, ECH * DT), dtype=np.float32)

    starts = np.zeros(NCORES * NDT + 1, dtype=np.int64)
    np.cumsum(counts, out=starts[1:])
    pos = np.arange(len(bucket_s)) - starts[bucket_s]
    c_idx = pos // P  # edge chunk within bucket
    e_idx = pos % P  # partition row within chunk

    ci = bucket_s // NDT  # core
    ti = bucket_s % NDT  # dtile
    srcB = (src_s // NODES_PER) * SLAB + (src_s % NODES_PER)
    idxA[ci, ti, e_idx, c_idx] = src_s.astype(np.int32)
    idxB[ci, ti, e_idx, c_idx] = srcB.astype(np.int32)
    S[ci, ti, e_idx, c_idx * DT + dl_s] = norm_s
    return ECH, idxA, idxB, S


def _build(ECH: int, use_b1: bool, use_b2: bool):
    nc = bacc.Bacc("TRN2", target_bir_lowering=False, num_devices=NCORES)

    g1_t = nc.dram_tensor("G1", [NDT, P, ECH * D], bf16, kind="ExternalInput")
    w1_t = nc.dram_tensor("W1", [D, H], f32r, kind="ExternalInput")
    w2_t = nc.dram_tensor("W2", [H, H], bf16, kind="ExternalInput")
    w3i_t = nc.dram_tensor("W3img", [H, B], f32r, kind="ExternalInput")
    b1_t = nc.dram_tensor("b1", [1, H], f32r, kind="ExternalInput")
    b2_t = nc.dram_tensor("b2", [P, NFI2], f32r, kind="ExternalInput")
    idxB_t = nc.dram_tensor("idxB", [NDT, P, 2 * ECH], i32, kind="ExternalInput")
    idxQ_t = nc.dram_tensor("idxQ", [NDT, P, ECH], i32, kind="ExternalInput")
    s_tab = nc.dram_tensor("S", [NDT, P, ECH * DT], bf16, kind="ExternalInput")
    out_t = nc.dram_tensor("out", [B, SLAB], f32, kind="ExternalOutput")

    h1p = nc.dram_tensor("h1p", [SLAB, H], bf16)
    h1pf = nc.dram_tensor("h1pf", [SLAB * NCORES, H], bf16, addr_space="Shared")
    q_slab = nc.dram_tensor("q_slab", [SLAB, B], bf16)
    q_full = nc.dram_tensor("q_full", [SLAB * NCORES, B], bf16, addr_space="Shared")

    rg = [list(range(NCORES))]

    with tile_mod.TileContext(nc) as tc:
        with (
            tc.tile_pool(name="w", bufs=1) as wp,
            tc.tile_pool(name="gio", bufs=3) as gp,
            tc.tile_pool(name="g1io", bufs=2) as g1p,
            tc.tile_pool(name="stab", bufs=2) as sp,
            tc.tile_pool(name="agg", bufs=1) as ap,
            tc.tile_pool(name="small", bufs=3) as mp,
            tc.tile_pool(name="hout", bufs=2) as hp,
            tc.tile_pool(name="consts", bufs=1) as cp,
            tc.tile_pool(name="ps", bufs=8, space="PSUM") as ps,
        ):
            # --- resident weights ---
            w1sb = []
            for fi in range(NFI1):
                w = wp.tile([P, H], f32r, tag="w1", name="wsb", bufs=NFI1)
                nc.sync.dma_start(out=w[:], in_=w1_t[fi * P : (fi + 1) * P, :])
                w1sb.append(w)
            w2sb = []
            for fi in range(NFI2):
                w = wp.tile([P, H], bf16, tag="w2", name="w2sb", bufs=NFI2)
                nc.sync.dma_start(out=w[:], in_=w2_t[fi * P : (fi + 1) * P, :])
                w2sb.append(w)
            w3sb = []
            for fo in range(NFI2):
                w = wp.tile([P, B], f32r, tag="w3", name="w3sb", bufs=NFI2)
                nc.sync.dma_start(out=w[:], in_=w3i_t[fo * P : (fo + 1) * P, :])
                w3sb.append(w)
            if use_b1:
                b1sb = cp.tile([1, H], f32r)
                nc.sync.dma_start(out=b1sb[:], in_=b1_t[:])
                ones1 = cp.tile([1, P], f32r)
                nc.gpsimd.memset(ones1[:], 1.0)
            if use_b2:
                b2sb = cp.tile([P, NFI2], f32r)
                nc.sync.dma_start(out=b2sb[:], in_=b2_t[:])

            relu = mybir.ActivationFunctionType.Relu

            # ---------------- Layer 1 (pre-gathered sources) ----------------
            for t in range(NDT):
                g1 = g1p.tile([P, ECH * D], bf16, tag="g1")
                nc.sync.dma_start(out=g1[:], in_=g1_t[t])
                s_t = sp.tile([P, ECH * DT], bf16, tag="s")
                nc.sync.dma_start(out=s_t[:], in_=s_tab[t])

                pa = [ps.tile([P, DT], f32, tag="ps", name="pa") for _ in range(NFI1)]
                for c in range(ECH):
                    for fi in range(NFI1):
                        nc.tensor.matmul(
                            out=pa[fi][:],
                            lhsT=g1[:, c * D + fi * P : c * D + (fi + 1) * P],
                            rhs=s_t[:, c * DT : (c + 1) * DT],
                            start=(c == 0),
                            stop=(c == ECH - 1),
                        )
                aggT = []
                for fi in range(NFI1):
                    a = ap.tile([P, DT], f32r, tag="aggT", name="aggTt", bufs=8)
                    nc.vector.tensor_copy(out=a[:], in_=pa[fi][:])
                    aggT.append(a)

                for dh in range(2):
                    h_t = hp.tile([P, H], bf16, tag="hout")
                    for fo in range(NFI1):  # 4 output blocks of 512
                        pz = ps.tile([P, D], f32, tag="ps", name="pz")
                        if use_b1:
                            nc.tensor.matmul(
                                out=pz[:],
                                lhsT=ones1[:1, :],
                                rhs=b1sb[:1, fo * D : (fo + 1) * D],
                                start=True,
                                stop=False,
                            )
                        for fi in range(NFI1):
                            nc.tensor.matmul(
                                out=pz[:],
                                lhsT=aggT[fi][:, dh * P : (dh + 1) * P],
                                rhs=w1sb[fi][:, fo * D : (fo + 1) * D],
                                start=(fi == 0 and not use_b1),
                                stop=(fi == NFI1 - 1),
                            )
                        nc.scalar.activation(
                            out=h_t[:, fo * D : (fo + 1) * D], in_=pz[:], func=relu
                        )
                    nc.sync.dma_start(
                        out=h1p[t * DT + dh * P : t * DT + (dh + 1) * P, :],
                        in_=h_t[:],
                    )

            nc.gpsimd.collective_compute(
                "AllGather",
                mybir.AluOpType.bypass,
                replica_groups=rg,
                ins=[h1p[:]],
                outs=[h1pf[:]],
            )

            # ---------------- Layer 2 + Q ----------------
            NH = NFI2 // 2  # 8 feature chunks per half-row pass
            h1pf_half = h1pf[:].rearrange("n (h d) -> (n h) d", h=2)
            for t in range(NDT):
                idx_t = mp.tile([P, 2 * ECH], i32, tag="idx")
                nc.sync.dma_start(out=idx_t[:], in_=idxB_t[t])
                s_t = sp.tile([P, ECH * DT], bf16, tag="s")
                nc.sync.dma_start(out=s_t[:], in_=s_tab[t])

                aggT = []
                for hf in range(2):
                    pa = [ps.tile([P, DT], f32, tag="ps", name="pa2") for _ in range(NH)]
                    for c in range(ECH):
                        g = gp.tile([P, H // 2], bf16, tag="g")
                        nc.gpsimd.indirect_dma_start(
                            out=g[:],
                            out_offset=None,
                            in_=h1pf_half,
                            in_offset=bass.IndirectOffsetOnAxis(
                                ap=idx_t[:, hf * ECH + c : hf * ECH + c + 1], axis=0
                            ),
                        )
                        for j in range(NH):
                            nc.tensor.matmul(
                                out=pa[j][:],
                                lhsT=g[:, j * P : (j + 1) * P],
                                rhs=s_t[:, c * DT : (c + 1) * DT],
                                start=(c == 0),
                                stop=(c == ECH - 1),
                            )
                    for j in range(NH):
                        a = ap.tile([P, DT], bf16, tag="aggT2", name="aggTt2", bufs=NFI2)
                        nc.vector.tensor_copy(out=a[:], in_=pa[j][:])
                        aggT.append(a)

                h2sb = []
                for fo in range(NFI2):
                    pz = ps.tile([P, DT], f32, tag="ps", name="pz2")
                    for fi in range(NFI2):
                        nc.tensor.matmul(
                            out=pz[:],
                            lhsT=w2sb[fi][:, fo * P : (fo + 1) * P],
                            rhs=aggT[fi][:],
                            start=(fi == 0),
                            stop=(fi == NFI2 - 1),
                        )
                    h2t = ap.tile([P, DT], f32r, tag="h2", name="h2t", bufs=NFI2)
                    if use_b2:
                        nc.scalar.activation(
                            out=h2t[:], in_=pz[:], func=relu,
                            bias=b2sb[:, fo : fo + 1],
                        )
                    else:
                        nc.scalar.activation(out=h2t[:], in_=pz[:], func=relu)
                    h2sb.append(h2t)

                pq = [ps.tile([P, B], f32, tag="ps", name="pq") for _ in range(2)]
                for fo in range(NFI2):
                    for dh in range(2):
                        nc.tensor.matmul(
                            out=pq[dh][:],
                            lhsT=h2sb[fo][:, dh * P : (dh + 1) * P],
                            rhs=w3sb[fo][:],
                            start=(fo == 0),
                            stop=(fo == NFI2 - 1),
                        )
                for dh in range(2):
                    qn = mp.tile([P, B], bf16, tag="qn")
                    nc.vector.tensor_copy(out=qn[:], in_=pq[dh][:])
                    nc.sync.dma_start(
                        out=q_slab[t * DT + dh * P : t * DT + (dh + 1) * P, :],
                        in_=qn[:],
                    )

            nc.gpsimd.collective_compute(
                "AllGather",
                mybir.AluOpType.bypass,
                replica_groups=rg,
                ins=[q_slab[:]],
                outs=[q_full[:]],
            )

            # ---------------- Layer 3 (= output) ----------------
            for t in range(NDT):
                idx_t = mp.tile([P, ECH], i32, tag="idxq")
                nc.sync.dma_start(out=idx_t[:], in_=idxQ_t[t])
                s_t = sp.tile([P, ECH * DT], bf16, tag="s")
                nc.sync.dma_start(out=s_t[:], in_=s_tab[t])

                pa = ps.tile([B, DT], f32, tag="ps", name="pa3")
                for c in range(ECH):
                    g = gp.tile([P, B], bf16, tag="g3")
                    nc.gpsimd.indirect_dma_start(
                        out=g[:],
                        out_offset=None,
                        in_=q_full[:],
                        in_offset=bass.IndirectOffsetOnAxis(
                            ap=idx_t[:, c : c + 1], axis=0
                        ),
                    )
                    nc.tensor.matmul(
                        out=pa[:],
                        lhsT=g[:],
                        rhs=s_t[:, c * DT : (c + 1) * DT],
                        start=(c == 0),
                        stop=(c == ECH - 1),
                    )
                o_t = mp.tile([B, DT], f32, tag="ot")
                nc.vector.tensor_copy(out=o_t[:], in_=pa[:])
                nc.sync.dma_start(out=out_t[:, t * DT : (t + 1) * DT], in_=o_t[:])

    nc.finalize()
    return nc


_CACHE: dict = {}


def kernel(**inputs: np.ndarray) -> np.ndarray:
    import ml_dtypes

    nodes = np.asarray(inputs["nodes"], dtype=np.float32)
    edge_index = np.asarray(inputs["edge_index"])
    img = np.asarray(inputs["img"], dtype=np.float32)
    W1 = np.asarray(inputs["W1"], dtype=np.float32)
    b1 = np.asarray(inputs["b1"], dtype=np.float32)
    W2 = np.asarray(inputs["W2"], dtype=np.float32)
    b2 = np.asarray(inputs["b2"], dtype=np.float32)
    W3 = np.asarray(inputs["W3"], dtype=np.float32)
    b3 = np.asarray(inputs["b3"], dtype=np.float32)

    ECH, idxA, idxB, S = _preprocess(edge_index)
    S = S.astype(ml_dtypes.bfloat16)
    # doubled indices for half-row (1024-wide) layer-2 gathers: [.., 2*ECH]
    idxB2 = np.concatenate([2 * idxB, 2 * idxB + 1], axis=3)
    use_b1 = bool(np.any(b1))
    use_b2 = bool(np.any(b2))

    key = (ECH, use_b1, use_b2)
    if key not in _CACHE:
        _CACHE[key] = _build(ECH, use_b1, use_b2)
    nc = _CACHE[key]

    w3img = _round_fp32r(W3.astype(np.float32) @ img.astype(np.float32).T)  # [H, B]
    outbias = img @ b3  # [B]

    nodes_r = nodes.astype(ml_dtypes.bfloat16)
    w1_r = _round_fp32r(W1)
    w2_r = W2.astype(ml_dtypes.bfloat16)
    b1_r = _round_fp32r(b1.reshape(1, H))
    b2_r = _round_fp32r(np.ascontiguousarray(b2.reshape(NFI2, P).T))

    in_maps = []
    for k in range(NCORES):
        # host pre-gather of layer-1 source rows: [NDT, P, ECH, D] -> [NDT, P, ECH*D]
        g1 = nodes_r[idxA[k]].reshape(NDT, P, ECH * D)
        in_maps.append(
            {
                "G1": np.ascontiguousarray(g1),
                "W1": w1_r,
                "W2": w2_r,
                "W3img": w3img,
                "b1": b1_r,
                "b2": b2_r,
                "idxB": np.ascontiguousarray(idxB2[k]),
                "idxQ": np.ascontiguousarray(idxB[k]),
                "S": np.ascontiguousarray(S[k]),
            }
        )

    res = run_bass_kernel_spmd(nc, in_maps, core_ids=list(range(NCORES)))

    full = np.concatenate([res.results[k]["out"] for k in range(NCORES)], axis=1)
    n_ids = np.arange(N_SKIP, N)
    cols = (n_ids // NODES_PER) * SLAB + (n_ids % NODES_PER)
    out = full[:, cols] + outbias[:, None]
    return out.astype(np.float32)


if __name__ == "__main__":
    rng = np.random.default_rng(0)
    ins = {
        "nodes": rng.standard_normal((N, D)).astype(np.float32),
        "edge_index": rng.integers(0, N, size=(2, E)).astype(np.int64),
        "img": rng.standard_normal((B, D)).astype(np.float32),
        "W1": (rng.standard_normal((D, H)) * 0.02).astype(np.float32),
        "b1": np.zeros(H, np.float32),
        "W2": (rng.standard_normal((H, H)) * 0.02).astype(np.float32),
        "b2": np.zeros(H, np.float32),
        "W3": (rng.standard_normal((H, D)) * 0.02).astype(np.float32),
        "b3": np.zeros(D, np.float32),
    }
    out = kernel(**ins)
    print("out", out.shape, out.dtype, np.abs(out).mean())


# revision 8
# speedup vs baseline: 1.1268x; 1.0122x over previous
"""3-layer GCN + img@pair_embed.T for Trainium2, distributed over 8 NeuronCores.

Strategy (destination-sharded graph parallelism):
  - Each core owns a contiguous slab of destination nodes (3567, padded to 3584).
  - Edges (plus self-loops) are bucketed per 256-destination tile, sorted by
    destination, padded to chunks of 128. Host builds, per edge chunk, a dense
    [128 edges x 256 dests] selection matrix S holding the GCN norm
    coefficients, so segment-sum aggregation becomes TensorE matmuls:
        aggT[f, d] += G[e, f].T @ S[e, d]      (G = gathered source rows)
  - GCN identity A@(X W) == (A@X) W lets layer 1 aggregate 512-wide inputs.
    Layer-1 source rows are PRE-GATHERED ON HOST (static input X) so layer 1
    needs no on-device indirect DMA at all.
  - h1 is stored as ONE unified [SLAB, 2048] tensor; a single AllGather
    produces [8*SLAB, 2048] and layer-2 gathers fetch full 2048-wide rows in
    one indirect DMA per 128-edge chunk (SWDGE fixed cost amortized 4x).
  - The final  img @ pair_embed.T  folds into layer 3: with
    W3img = W3 @ img.T  [2048, 64], Q = h2 @ W3img, the layer-3 aggregation
    A @ Q directly produces output columns. Layer 3 aggregates 64-wide.
  - Activations/S travel as bf16; W2 is bf16-resident in SBUF (fits alongside
    2048-wide gather tiles); W1/W3img float32r; PSUM accumulation fp32.
"""

import numpy as np

from concourse import bacc, bass, mybir
from concourse import tile as tile_mod
from concourse.bass_utils import run_bass_kernel_spmd

# Problem shapes (hardcoded per spec nn_GraphModel_26268019982828)
N = 28535
E = 113000
D = 512
H = 2048
B = 64
N_SKIP = 115 + 245  # attrs + objs; pair nodes are N_SKIP..N-1

NCORES = 8
NODES_PER = -(-N // NCORES)  # 3567
P = 128
DT = 256  # destination tile width
NDT = 14  # dest tiles per core
SLAB = NDT * DT  # 3584 padded dests per core
NFI1 = D // P  # 4 input feature chunks (layer 1)
NFI2 = H // P  # 16 feature chunks (2048)

f32 = mybir.dt.float32
f32r = mybir.dt.float32r
bf16 = mybir.dt.bfloat16
i32 = mybir.dt.int32


def _round_fp32r(x: np.ndarray) -> np.ndarray:
    """Round-to-nearest-even fp32 -> fp32r (11-bit mantissa), numpy."""
    u = np.ascontiguousarray(x, dtype=np.float32).view(np.uint32)
    r = u + (0x7FF + ((u >> 12) & np.uint32(1)))
    r &= np.uint32(0xFFFFF000)
    return r.view(np.float32)


def _preprocess(edge_index: np.ndarray):
    """Sort/bucket edges by destination; build gather indices + S blocks.

    Returns (ECH, idxA, idxB, S) with
      idxA [NCORES, NDT, P, ECH] int32 — source node ids (original numbering)
      idxB [NCORES, NDT, P, ECH] int32 — source ids remapped to slab rows
      S    [NCORES, NDT, P, ECH*DT] float32 — norm matrix
    """
    src = np.concatenate([edge_index[0], np.arange(N, dtype=np.int64)])
    dst = np.concatenate([edge_index[1], np.arange(N, dtype=np.int64)])
    deg = np.bincount(dst, minlength=N).astype(np.float32)  # includes loops
    dinv = (1.0 / np.sqrt(deg)).astype(np.float32)
    norm = (dinv[src] * dinv[dst]).astype(np.float32)

    core = (dst // NODES_PER).astype(np.int64)
    local = (dst - core * NODES_PER).astype(np.int64)
    t_idx = local // DT
    d_local = local % DT
    bucket = core * NDT + t_idx  # global bucket id per edge

    order = np.argsort(bucket, kind="stable")
    src_s = src[order]
    bucket_s = bucket[order]
    dl_s = d_local[order]
    norm_s = norm[order]

    counts = np.bincount(bucket_s, minlength=NCORES * NDT)
    ECH = int(-(-counts.max() // P))

    idxA = np.zeros((NCORES, NDT, P, ECH), dtype=np.int32)
    idxB = np.zeros((NCORES, NDT, P, ECH), dtype=np.int32)
    S = np.zeros((NCORES, NDT, P, ECH * DT), dtype=np.float32)

    starts = np.zeros(NCORES * NDT + 1, dtype=np.int64)
    np.cumsum(counts, out=starts[1:])
    pos = np.arange(len(bucket_s)) - starts[bucket_s]
    c_idx = pos // P  # edge chunk within bucket
    e_idx = pos % P  # partition row within chunk

    ci = bucket_s // NDT  # core
    ti = bucket_s % NDT  # dtile
    srcB = (src_s // NODES_PER) * SLAB + (src_s % NODES_PER)
    idxA[ci, ti, e_idx, c_idx] = src_s.astype(np.int32)
    idxB[ci, ti, e_idx, c_idx] = srcB.astype(np.int32)
    S[ci, ti, e_idx, c_idx * DT + dl_s] = norm_s
    return ECH, idxA, idxB, S


def _build(ECH: int, use_b1: bool, use_b2: bool):
    nc = bacc.Bacc("TRN2", target_bir_lowering=False, num_devices=NCORES)

    g1_t = nc.dram_tensor("G1", [NDT, P, ECH * D], bf16, kind="ExternalInput")
    w1_t = nc.dram_tensor("W1", [D, H], f32r, kind="ExternalInput")
    w2_t = nc.dram_tensor("W2", [H, H], bf16, kind="ExternalInput")
    w3i_t = nc.dram_tensor("W3img", [H, B], bf16, kind="ExternalInput")
    b1_t = nc.dram_tensor("b1", [1, H], f32r, kind="ExternalInput")
    b2_t = nc.dram_tensor("b2", [P, NFI2], f32r, kind="ExternalInput")
    idxB_t = nc.dram_tensor("idxB", [NDT, P, 2 * ECH], i32, kind="ExternalInput")
    idxQ_t = nc.dram_tensor("idxQ", [NDT, P, ECH], i32, kind="ExternalInput")
    s_tab = nc.dram_tensor("S", [NDT, P, ECH * DT], bf16, kind="ExternalInput")
    out_t = nc.dram_tensor("out", [B, SLAB], f32, kind="ExternalOutput")

    h1p = nc.dram_tensor("h1p", [SLAB, H], bf16)
    h1pf = nc.dram_tensor("h1pf", [SLAB * NCORES, H], bf16, addr_space="Shared")
    q_slab = nc.dram_tensor("q_slab", [SLAB, B], bf16)
    q_full = nc.dram_tensor("q_full", [SLAB * NCORES, B], bf16, addr_space="Shared")

    rg = [list(range(NCORES))]

    with tile_mod.TileContext(nc) as tc:
        with (
            tc.tile_pool(name="w", bufs=1) as wp,
            tc.tile_pool(name="gio", bufs=3) as gp,
            tc.tile_pool(name="g1io", bufs=2) as g1p,
            tc.tile_pool(name="stab", bufs=2) as sp,
            tc.tile_pool(name="agg", bufs=1) as ap,
            tc.tile_pool(name="small", bufs=3) as mp,
            tc.tile_pool(name="hout", bufs=2) as hp,
            tc.tile_pool(name="consts", bufs=1) as cp,
            tc.tile_pool(name="ps", bufs=8, space="PSUM") as ps,
        ):
            # --- resident weights ---
            w1sb = []
            for fi in range(NFI1):
                w = wp.tile([P, H], f32r, tag="w1", name="wsb", bufs=NFI1)
                nc.sync.dma_start(out=w[:], in_=w1_t[fi * P : (fi + 1) * P, :])
                w1sb.append(w)
            w2sb = []
            for fi in range(NFI2):
                w = wp.tile([P, H], bf16, tag="w2", name="w2sb", bufs=NFI2)
                nc.sync.dma_start(out=w[:], in_=w2_t[fi * P : (fi + 1) * P, :])
                w2sb.append(w)
            w3sb = []
            for fo in range(NFI2):
                w = wp.tile([P, B], bf16, tag="w3", name="w3sb", bufs=NFI2)
                nc.sync.dma_start(out=w[:], in_=w3i_t[fo * P : (fo + 1) * P, :])
                w3sb.append(w)
            if use_b1:
                b1sb = cp.tile([1, H], f32r)
                nc.sync.dma_start(out=b1sb[:], in_=b1_t[:])
                ones1 = cp.tile([1, P], f32r)
                nc.gpsimd.memset(ones1[:], 1.0)
            if use_b2:
                b2sb = cp.tile([P, NFI2], f32r)
                nc.sync.dma_start(out=b2sb[:], in_=b2_t[:])

            relu = mybir.ActivationFunctionType.Relu

            # ---------------- Layer 1 (pre-gathered sources) ----------------
            for t in range(NDT):
                g1 = g1p.tile([P, ECH * D], bf16, tag="g1")
                nc.sync.dma_start(out=g1[:], in_=g1_t[t])
                s_t = sp.tile([P, ECH * DT], bf16, tag="s")
                nc.sync.dma_start(out=s_t[:], in_=s_tab[t])

                pa = [ps.tile([P, DT], f32, tag="ps", name="pa") for _ in range(NFI1)]
                for c in range(ECH):
                    for fi in range(NFI1):
                        nc.tensor.matmul(
                            out=pa[fi][:],
                            lhsT=g1[:, c * D + fi * P : c * D + (fi + 1) * P],
                            rhs=s_t[:, c * DT : (c + 1) * DT],
                            start=(c == 0),
                            stop=(c == ECH - 1),
                        )
                aggT = []
                for fi in range(NFI1):
                    a = ap.tile([P, DT], f32r, tag="aggT", name="aggTt", bufs=8)
                    nc.vector.tensor_copy(out=a[:], in_=pa[fi][:])
                    aggT.append(a)

                for dh in range(2):
                    h_t = hp.tile([P, H], bf16, tag="hout")
                    for fo in range(NFI1):  # 4 output blocks of 512
                        pz = ps.tile([P, D], f32, tag="ps", name="pz")
                        if use_b1:
                            nc.tensor.matmul(
                                out=pz[:],
                                lhsT=ones1[:1, :],
                                rhs=b1sb[:1, fo * D : (fo + 1) * D],
                                start=True,
                                stop=False,
                            )
                        for fi in range(NFI1):
                            nc.tensor.matmul(
                                out=pz[:],
                                lhsT=aggT[fi][:, dh * P : (dh + 1) * P],
                                rhs=w1sb[fi][:, fo * D : (fo + 1) * D],
                                start=(fi == 0 and not use_b1),
                                stop=(fi == NFI1 - 1),
                            )
                        nc.scalar.activation(
                            out=h_t[:, fo * D : (fo + 1) * D], in_=pz[:], func=relu
                        )
                    nc.sync.dma_start(
                        out=h1p[t * DT + dh * P : t * DT + (dh + 1) * P, :],
                        in_=h_t[:],
                    )

            RC = SLAB // 4  # 896-row AllGather chunks
            for j in range(4):
                nc.gpsimd.collective_compute(
                    "AllGather",
                    mybir.AluOpType.bypass,
                    replica_groups=rg,
                    ins=[h1p[j * RC : (j + 1) * RC, :]],
                    outs=[h1pf[j * RC * NCORES : (j + 1) * RC * NCORES, :]],
                )

            # ---------------- Layer 2 + Q ----------------
            NH = NFI2 // 2  # 8 feature chunks per half-row pass
            h1pf_half = h1pf[:].rearrange("n (h d) -> (n h) d", h=2)
            for tp in range(NDT // 2):
                aggT = [
                    ap.tile([P, 2 * DT], bf16, tag="aggT2", name="aggTt2", bufs=NFI2)
                    for _ in range(NFI2)
                ]
                for t2 in range(2):
                    t = tp * 2 + t2
                    idx_t = mp.tile([P, 2 * ECH], i32, tag="idx")
                    nc.sync.dma_start(out=idx_t[:], in_=idxB_t[t])
                    s_t = sp.tile([P, ECH * DT], bf16, tag="s")
                    nc.sync.dma_start(out=s_t[:], in_=s_tab[t])

                    for hf in range(2):
                        pa = [ps.tile([P, DT], f32, tag="ps", name="pa2") for _ in range(NH)]
                        for c in range(ECH):
                            g = gp.tile([P, H // 2], bf16, tag="g")
                            nc.gpsimd.indirect_dma_start(
                                out=g[:],
                                out_offset=None,
                                in_=h1pf_half,
                                in_offset=bass.IndirectOffsetOnAxis(
                                    ap=idx_t[:, hf * ECH + c : hf * ECH + c + 1], axis=0
                                ),
                            )
                            for j in range(NH):
                                nc.tensor.matmul(
                                    out=pa[j][:],
                                    lhsT=g[:, j * P : (j + 1) * P],
                                    rhs=s_t[:, c * DT : (c + 1) * DT],
                                    start=(c == 0),
                                    stop=(c == ECH - 1),
                                )
                        for j in range(NH):
                            fi = hf * NH + j
                            nc.vector.tensor_copy(
                                out=aggT[fi][:, t2 * DT : (t2 + 1) * DT], in_=pa[j][:]
                            )

                h2sb = []
                pq = [ps.tile([P, B], f32, tag="ps", name="pq") for _ in range(4)]
                for fo in range(NFI2):
                    pz = ps.tile([P, 2 * DT], f32, tag="ps", name="pz2")
                    for fi in range(NFI2):
                        nc.tensor.matmul(
                            out=pz[:],
                            lhsT=w2sb[fi][:, fo * P : (fo + 1) * P],
                            rhs=aggT[fi][:],
                            start=(fi == 0),
                            stop=(fi == NFI2 - 1),
                        )
                    h2t = ap.tile([P, 2 * DT], bf16, tag="h2", name="h2t", bufs=NFI2)
                    if use_b2:
                        nc.scalar.activation(
                            out=h2t[:], in_=pz[:], func=relu,
                            bias=b2sb[:, fo : fo + 1],
                        )
                    else:
                        nc.scalar.activation(out=h2t[:], in_=pz[:], func=relu)
                    h2sb.append(h2t)

                    for dh in range(4):
                        nc.tensor.matmul(
                            out=pq[dh][:],
                            lhsT=h2t[:, dh * P : (dh + 1) * P],
                            rhs=w3sb[fo][:],
                            start=(fo == 0),
                            stop=(fo == NFI2 - 1),
                        )
                for dh in range(4):
                    qn = mp.tile([P, B], bf16, tag="qn")
                    nc.vector.tensor_copy(out=qn[:], in_=pq[dh][:])
                    nc.sync.dma_start(
                        out=q_slab[tp * 2 * DT + dh * P : tp * 2 * DT + (dh + 1) * P, :],
                        in_=qn[:],
                    )

            nc.gpsimd.collective_compute(
                "AllGather",
                mybir.AluOpType.bypass,
                replica_groups=rg,
                ins=[q_slab[:]],
                outs=[q_full[:]],
            )

            # ---------------- Layer 3 (= output) ----------------
            for t in range(NDT):
                idx_t = mp.tile([P, ECH], i32, tag="idxq")
                nc.sync.dma_start(out=idx_t[:], in_=idxQ_t[t])
                s_t = sp.tile([P, ECH * DT], bf16, tag="s")
                nc.sync.dma_start(out=s_t[:], in_=s_tab[t])

                pa = ps.tile([B, DT], f32, tag="ps", name="pa3")
                for c in range(ECH):
                    g = gp.tile([P, B], bf16, tag="g3")
                    nc.gpsimd.indirect_dma_start(
                        out=g[:],
                        out_offset=None,
                        in_=q_full[:],
                        in_offset=bass.IndirectOffsetOnAxis(
                            ap=idx_t[:, c : c + 1], axis=0
                        ),
                    )
                    nc.tensor.matmul(
                        out=pa[:],
                        lhsT=g[:],
                        rhs=s_t[:, c * DT : (c + 1) * DT],
                        start=(c == 0),
                        stop=(c == ECH - 1),
                    )
                o_t = mp.tile([B, DT], f32, tag="ot")
                nc.vector.tensor_copy(out=o_t[:], in_=pa[:])
                nc.sync.dma_start(out=out_t[:, t * DT : (t + 1) * DT], in_=o_t[:])

    nc.finalize()
    return nc


_CACHE: dict = {}


def kernel(**inputs: np.ndarray) -> np.ndarray:
    import ml_dtypes

    nodes = np.asarray(inputs["nodes"], dtype=np.float32)
    edge_index = np.asarray(inputs["edge_index"])
    img = np.asarray(inputs["img"], dtype=np.float32)
    W1 = np.asarray(inputs["W1"], dtype=np.float32)
    b1 = np.asarray(inputs["b1"], dtype=np.float32)
    W2 = np.asarray(inputs["W2"], dtype=np.float32)
    b2 = np.asarray(inputs["b2"], dtype=np.float32)
    W3 = np.asarray(inputs["W3"], dtype=np.float32)
    b3 = np.asarray(inputs["b3"], dtype=np.float32)

    ECH, idxA, idxB, S = _preprocess(edge_index)
    S = S.astype(ml_dtypes.bfloat16)
    # chunk-major remap of slab rows to match the 4 chunked AllGathers:
    # slab row r of core k -> (r//896)*896*8 + k*896 + (r%896)
    core_of = idxB // SLAB
    r_of = idxB % SLAB
    idxB_cm = (r_of // 896) * (896 * NCORES) + core_of * 896 + (r_of % 896)
    # doubled indices for half-row (1024-wide) layer-2 gathers: [.., 2*ECH]
    idxB2 = np.concatenate([2 * idxB_cm, 2 * idxB_cm + 1], axis=3)
    use_b1 = bool(np.any(b1))
    use_b2 = bool(np.any(b2))

    key = (ECH, use_b1, use_b2)
    if key not in _CACHE:
        _CACHE[key] = _build(ECH, use_b1, use_b2)
    nc = _CACHE[key]

    w3img = (W3.astype(np.float32) @ img.astype(np.float32).T).astype(ml_dtypes.bfloat16)  # [H, B]
    outbias = img @ b3  # [B]

    nodes_r = nodes.astype(ml_dtypes.bfloat16)
    w1_r = _round_fp32r(W1)
    w2_r = W2.astype(ml_dtypes.bfloat16)
    b1_r = _round_fp32r(b1.reshape(1, H))
    b2_r = _round_fp32r(np.ascontiguousarray(b2.reshape(NFI2, P).T))

    in_maps = []
    for k in range(NCORES):
        # host pre-gather of layer-1 source rows: [NDT, P, ECH, D] -> [NDT, P, ECH*D]
        g1 = nodes_r[idxA[k]].reshape(NDT, P, ECH * D)
        in_maps.append(
            {
                "G1": np.ascontiguousarray(g1),
                "W1": w1_r,
                "W2": w2_r,
                "W3img": w3img,
                "b1": b1_r,
                "b2": b2_r,
                "idxB": np.ascontiguousarray(idxB2[k]),
                "idxQ": np.ascontiguousarray(idxB[k]),
                "S": np.ascontiguousarray(S[k]),
            }
        )

    res = run_bass_kernel_spmd(nc, in_maps, core_ids=list(range(NCORES)))

    full = np.concatenate([res.results[k]["out"] for k in range(NCORES)], axis=1)
    n_ids = np.arange(N_SKIP, N)
    cols = (n_ids // NODES_PER) * SLAB + (n_ids % NODES_PER)
    out = full[:, cols] + outbias[:, None]
    return out.astype(np.float32)


if __name__ == "__main__":
    rng = np.random.default_rng(0)
    ins = {
        "nodes": rng.standard_normal((N, D)).astype(np.float32),
        "edge_index": rng.integers(0, N, size=(2, E)).astype(np.int64),
        "img": rng.standard_normal((B, D)).astype(np.float32),
        "W1": (rng.standard_normal((D, H)) * 0.02).astype(np.float32),
        "b1": np.zeros(H, np.float32),
        "W2": (rng.standard_normal((H, H)) * 0.02).astype(np.float32),
        "b2": np.zeros(H, np.float32),
        "W3": (rng.standard_normal((H, D)) * 0.02).astype(np.float32),
        "b3": np.zeros(D, np.float32),
    }
    out = kernel(**ins)
    print("out", out.shape, out.dtype, np.abs(out).mean())


# revision 12
# speedup vs baseline: 1.3625x; 1.2091x over previous
"""3-layer GCN + img@pair_embed.T for Trainium2, distributed over 8 NeuronCores.

Strategy (destination-sharded graph parallelism, agg1-exchange variant):
  - Each core owns a contiguous slab of destination nodes (3567, padded 3584).
  - Edges (plus self-loops) are bucketed per 256-destination tile and padded to
    128-edge chunks. Host builds per chunk a dense [128 edges x 256 dests]
    one-hot norm matrix S, so segment-sum aggregation becomes TensorE matmuls.
  - Layer-1 source rows are PRE-GATHERED ON HOST (X is a static input), and the
    layer-1 aggregation computes agg1 = A@X directly in node-row orientation
    (lhsT = S chunk), so agg1 [SLAB, 512] is written without any transpose.
  - KEY: the cross-core exchange moves agg1 (512 wide) instead of h1 (2048
    wide): ONE AllGather of [SLAB,512] -> [8*SLAB,512] (29MB out) instead of
    117MB. Each core then recomputes h1 = relu(agg1 @ W1) for only the unique
    source rows its layer-2/3 edges touch (~13k rows): gather agg1 rows,
    PE-transpose them into contraction layout, GEMM against resident W1.
  - Layer 2 gathers 1024-wide half-rows of the local recomputed h1_u in two
    passes (PSUM has only 8 accumulation banks), GEMMs in dtile pairs
    (free dim 512), and folds img into layer 3: W3img = W3@img.T, Q = h2@W3img.
  - Layer 3 aggregates 64-wide Q after a small Q AllGather.
  - Everything exchanged/gathered travels bf16; W1 float32r; W2/W3img bf16;
    PSUM accumulation fp32.
"""

import numpy as np

from concourse import bacc, bass, mybir
from concourse import tile as tile_mod
from concourse.bass_utils import run_bass_kernel_spmd

# Problem shapes (hardcoded per spec nn_GraphModel_26268019982828)
N = 28535
E = 113000
D = 512
H = 2048
B = 64
N_SKIP = 115 + 245  # attrs + objs; pair nodes are N_SKIP..N-1

NCORES = 8
NODES_PER = -(-N // NCORES)  # 3567
P = 128
DT = 256  # destination tile width
NDT = 14  # dest tiles per core
SLAB = NDT * DT  # 3584 padded dests per core
NFI1 = D // P  # 4 feature chunks of layer-1 width
NFI2 = H // P  # 16 feature chunks of hidden width

f32 = mybir.dt.float32
f32r = mybir.dt.float32r
bf16 = mybir.dt.bfloat16
i32 = mybir.dt.int32


def _round_fp32r(x: np.ndarray) -> np.ndarray:
    """Round-to-nearest-even fp32 -> fp32r (11-bit mantissa), numpy."""
    u = np.ascontiguousarray(x, dtype=np.float32).view(np.uint32)
    r = u + (0x7FF + ((u >> 12) & np.uint32(1)))
    r &= np.uint32(0xFFFFF000)
    return r.view(np.float32)


def _preprocess(edge_index: np.ndarray):
    """Sort/bucket edges by destination; build gather indices + S blocks."""
    src = np.concatenate([edge_index[0], np.arange(N, dtype=np.int64)])
    dst = np.concatenate([edge_index[1], np.arange(N, dtype=np.int64)])
    deg = np.bincount(dst, minlength=N).astype(np.float32)  # includes loops
    dinv = (1.0 / np.sqrt(deg)).astype(np.float32)
    norm = (dinv[src] * dinv[dst]).astype(np.float32)

    core = (dst // NODES_PER).astype(np.int64)
    local = (dst - core * NODES_PER).astype(np.int64)
    t_idx = local // DT
    d_local = local % DT
    bucket = core * NDT + t_idx

    order = np.argsort(bucket, kind="stable")
    src_s = src[order]
    bucket_s = bucket[order]
    dl_s = d_local[order]
    norm_s = norm[order]

    counts = np.bincount(bucket_s, minlength=NCORES * NDT)
    ECH = int(-(-counts.max() // P))

    idxA = np.zeros((NCORES, NDT, P, ECH), dtype=np.int32)
    idxB = np.zeros((NCORES, NDT, P, ECH), dtype=np.int32)
    S = np.zeros((NCORES, NDT, P, ECH * DT), dtype=np.float32)

    starts = np.zeros(NCORES * NDT + 1, dtype=np.int64)
    np.cumsum(counts, out=starts[1:])
    pos = np.arange(len(bucket_s)) - starts[bucket_s]
    c_idx = pos // P
    e_idx = pos % P

    ci = bucket_s // NDT
    ti = bucket_s % NDT
    srcB = (src_s // NODES_PER) * SLAB + (src_s % NODES_PER)
    idxA[ci, ti, e_idx, c_idx] = src_s.astype(np.int32)
    idxB[ci, ti, e_idx, c_idx] = srcB.astype(np.int32)
    S[ci, ti, e_idx, c_idx * DT + dl_s] = norm_s
    return ECH, idxA, idxB, S


def _build(ECH: int, NUCH: int, use_b1: bool, use_b2: bool):
    nc = bacc.Bacc("TRN2", target_bir_lowering=False, num_devices=NCORES)
    NU = NUCH * P  # padded unique-source rows per core

    g1_t = nc.dram_tensor("G1", [NDT, P, ECH * D], bf16, kind="ExternalInput")
    w1_t = nc.dram_tensor("W1", [D, H], bf16, kind="ExternalInput")
    w2_t = nc.dram_tensor("W2", [H, H], bf16, kind="ExternalInput")
    w3i_t = nc.dram_tensor("W3img", [H, B], bf16, kind="ExternalInput")
    b1_t = nc.dram_tensor("b1", [1, H], bf16, kind="ExternalInput")
    b2_t = nc.dram_tensor("b2", [P, NFI2], f32r, kind="ExternalInput")
    idxU_t = nc.dram_tensor("idxU", [NUCH, P], i32, kind="ExternalInput")
    idxL2_t = nc.dram_tensor("idxL2", [NDT, P, 2 * ECH], i32, kind="ExternalInput")
    idxQ_t = nc.dram_tensor("idxQ", [NDT, P, ECH], i32, kind="ExternalInput")
    s_tab = nc.dram_tensor("S", [NDT, P, ECH * DT], bf16, kind="ExternalInput")
    ident_t = nc.dram_tensor("IDENT", [P, P], bf16, kind="ExternalInput")
    out_t = nc.dram_tensor("out", [B, SLAB], f32, kind="ExternalOutput")

    agg1p = nc.dram_tensor("agg1p", [SLAB, D], bf16)
    agg1f = nc.dram_tensor("agg1f", [SLAB * NCORES, D], bf16, addr_space="Shared")
    h1u = nc.dram_tensor("h1u", [NU, H], bf16)
    q_slab = nc.dram_tensor("q_slab", [SLAB, B], bf16)
    q_full = nc.dram_tensor("q_full", [SLAB * NCORES, B], bf16, addr_space="Shared")

    rg = [list(range(NCORES))]

    with tile_mod.TileContext(nc) as tc:
        with (
            tc.tile_pool(name="w", bufs=1) as wp,
            tc.tile_pool(name="gio", bufs=4) as gp,
            tc.tile_pool(name="g1io", bufs=2) as g1p,
            tc.tile_pool(name="stab", bufs=2) as sp,
            tc.tile_pool(name="agg", bufs=1) as ap,
            tc.tile_pool(name="small", bufs=3) as mp,
            tc.tile_pool(name="hout", bufs=2) as hp,
            tc.tile_pool(name="consts", bufs=1) as cp,
            tc.tile_pool(name="ps", bufs=8, space="PSUM") as ps,
        ):
            # --- resident weights ---
            w1sb = []
            for fi in range(NFI1):
                w = wp.tile([P, H], bf16, tag="w1", name="wsb", bufs=NFI1)
                nc.sync.dma_start(out=w[:], in_=w1_t[fi * P : (fi + 1) * P, :])
                w1sb.append(w)
            w2sb = []
            for fi in range(NFI2):
                w = wp.tile([P, H], bf16, tag="w2", name="w2sb", bufs=NFI2)
                nc.sync.dma_start(out=w[:], in_=w2_t[fi * P : (fi + 1) * P, :])
                w2sb.append(w)
            w3sb = []
            for fo in range(NFI2):
                w = wp.tile([P, B], bf16, tag="w3", name="w3sb", bufs=NFI2)
                nc.sync.dma_start(out=w[:], in_=w3i_t[fo * P : (fo + 1) * P, :])
                w3sb.append(w)
            idt = cp.tile([P, P], bf16, tag="idt")
            nc.sync.dma_start(out=idt[:], in_=ident_t[:])
            if use_b1:
                b1sb = cp.tile([1, H], bf16, tag="b1")
                nc.sync.dma_start(out=b1sb[:], in_=b1_t[:])
                ones1 = cp.tile([1, P], bf16, tag="ones")
                nc.gpsimd.memset(ones1[:], 1.0)
            if use_b2:
                b2sb = cp.tile([P, NFI2], f32r, tag="b2")
                nc.sync.dma_start(out=b2sb[:], in_=b2_t[:])

            relu = mybir.ActivationFunctionType.Relu

            # ---------------- Layer 1: agg1 = A@X  (node-row orientation) ---
            for t in range(NDT):
                g1 = g1p.tile([P, ECH * D], bf16, tag="g1")
                nc.sync.dma_start(out=g1[:], in_=g1_t[t])
                s_t = sp.tile([P, ECH * DT], bf16, tag="s")
                nc.sync.dma_start(out=s_t[:], in_=s_tab[t])

                pd = [ps.tile([P, D], f32, tag="ps", name="pd") for _ in range(2)]
                for c in range(ECH):
                    for dh in range(2):
                        nc.tensor.matmul(
                            out=pd[dh][:],
                            lhsT=s_t[:, c * DT + dh * P : c * DT + (dh + 1) * P],
                            rhs=g1[:, c * D : (c + 1) * D],
                            start=(c == 0),
                            stop=(c == ECH - 1),
                        )
                for dh in range(2):
                    a1 = mp.tile([P, D], bf16, tag="a1")
                    nc.vector.tensor_copy(out=a1[:], in_=pd[dh][:])
                    nc.sync.dma_start(
                        out=agg1p[t * DT + dh * P : t * DT + (dh + 1) * P, :],
                        in_=a1[:],
                    )

            nc.gpsimd.collective_compute(
                "AllGather",
                mybir.AluOpType.bypass,
                replica_groups=rg,
                ins=[agg1p[:]],
                outs=[agg1f[:]],
            )

            # ------- Recompute h1 rows for unique needed sources -----------
            for u in range(NUCH):
                idxu = mp.tile([P, 1], i32, tag="idxu")
                nc.sync.dma_start(out=idxu[:], in_=idxU_t[u : u + 1].rearrange("o p -> p o"))
                gu = gp.tile([P, D], bf16, tag="gu")
                nc.gpsimd.indirect_dma_start(
                    out=gu[:],
                    out_offset=None,
                    in_=agg1f[:],
                    in_offset=bass.IndirectOffsetOnAxis(ap=idxu[:, 0:1], axis=0),
                )
                aT = []
                for fi in range(NFI1):
                    pt = ps.tile([P, P], bf16, tag="ps", name="pt")
                    nc.tensor.transpose(
                        out=pt[:], in_=gu[:, fi * P : (fi + 1) * P], identity=idt[:]
                    )
                    a = ap.tile([P, P], bf16, tag="aT", name="aTt", bufs=8)
                    nc.vector.tensor_copy(out=a[:], in_=pt[:])
                    aT.append(a)
                h1u_t = hp.tile([P, H], bf16, tag="hout")
                for fo in range(NFI1):
                    pz = ps.tile([P, D], f32, tag="ps", name="pz")
                    if use_b1:
                        nc.tensor.matmul(
                            out=pz[:],
                            lhsT=ones1[:1, :],
                            rhs=b1sb[:1, fo * D : (fo + 1) * D],
                            start=True,
                            stop=False,
                        )
                    for fi in range(NFI1):
                        nc.tensor.matmul(
                            out=pz[:],
                            lhsT=aT[fi][:],
                            rhs=w1sb[fi][:, fo * D : (fo + 1) * D],
                            start=(fi == 0 and not use_b1),
                            stop=(fi == NFI1 - 1),
                        )
                    nc.scalar.activation(
                        out=h1u_t[:, fo * D : (fo + 1) * D], in_=pz[:], func=relu
                    )
                nc.sync.dma_start(out=h1u[u * P : (u + 1) * P, :], in_=h1u_t[:])

            # ---------------- Layer 2 + Q (dtile pairs) --------------------
            NH = NFI2 // 2  # 8 feature chunks per half-row pass
            h1u_half = h1u[:].rearrange("n (h d) -> (n h) d", h=2)
            for tp in range(NDT // 2):
                aggT = [
                    ap.tile([P, 2 * DT], bf16, tag="aggT2", name="aggTt2", bufs=NFI2)
                    for _ in range(NFI2)
                ]
                for t2 in range(2):
                    t = tp * 2 + t2
                    idx_t = mp.tile([P, 2 * ECH], i32, tag="idx")
                    nc.sync.dma_start(out=idx_t[:], in_=idxL2_t[t])
                    s_t = sp.tile([P, ECH * DT], bf16, tag="s")
                    nc.sync.dma_start(out=s_t[:], in_=s_tab[t])

                    for hf in range(2):
                        pa = [ps.tile([P, DT], f32, tag="ps", name="pa2") for _ in range(NH)]
                        for c in range(ECH):
                            g = gp.tile([P, H // 2], bf16, tag="g")
                            nc.gpsimd.indirect_dma_start(
                                out=g[:],
                                out_offset=None,
                                in_=h1u_half,
                                in_offset=bass.IndirectOffsetOnAxis(
                                    ap=idx_t[:, hf * ECH + c : hf * ECH + c + 1], axis=0
                                ),
                            )
                            for j in range(NH):
                                nc.tensor.matmul(
                                    out=pa[j][:],
                                    lhsT=g[:, j * P : (j + 1) * P],
                                    rhs=s_t[:, c * DT : (c + 1) * DT],
                                    start=(c == 0),
                                    stop=(c == ECH - 1),
                                )
                        for j in range(NH):
                            fi = hf * NH + j
                            nc.vector.tensor_copy(
                                out=aggT[fi][:, t2 * DT : (t2 + 1) * DT], in_=pa[j][:]
                            )

                pq = [ps.tile([P, B], f32, tag="ps", name="pq") for _ in range(4)]
                for fo in range(NFI2):
                    pz = ps.tile([P, 2 * DT], f32, tag="ps", name="pz2")
                    for fi in range(NFI2):
                        nc.tensor.matmul(
                            out=pz[:],
                            lhsT=w2sb[fi][:, fo * P : (fo + 1) * P],
                            rhs=aggT[fi][:],
                            start=(fi == 0),
                            stop=(fi == NFI2 - 1),
                        )
                    h2t = ap.tile([P, 2 * DT], bf16, tag="h2", name="h2t", bufs=NFI2)
                    if use_b2:
                        nc.scalar.activation(
                            out=h2t[:], in_=pz[:], func=relu,
                            bias=b2sb[:, fo : fo + 1],
                        )
                    else:
                        nc.scalar.activation(out=h2t[:], in_=pz[:], func=relu)

                    for dh in range(4):
                        nc.tensor.matmul(
                            out=pq[dh][:],
                            lhsT=h2t[:, dh * P : (dh + 1) * P],
                            rhs=w3sb[fo][:],
                            start=(fo == 0),
                            stop=(fo == NFI2 - 1),
                        )
                for dh in range(4):
                    qn = mp.tile([P, B], bf16, tag="qn")
                    nc.vector.tensor_copy(out=qn[:], in_=pq[dh][:])
                    nc.sync.dma_start(
                        out=q_slab[tp * 2 * DT + dh * P : tp * 2 * DT + (dh + 1) * P, :],
                        in_=qn[:],
                    )

            nc.gpsimd.collective_compute(
                "AllGather",
                mybir.AluOpType.bypass,
                replica_groups=rg,
                ins=[q_slab[:]],
                outs=[q_full[:]],
            )

            # ---------------- Layer 3 (= output) ---------------------------
            for t in range(NDT):
                idx_t = mp.tile([P, ECH], i32, tag="idxq")
                nc.sync.dma_start(out=idx_t[:], in_=idxQ_t[t])
                s_t = sp.tile([P, ECH * DT], bf16, tag="s")
                nc.sync.dma_start(out=s_t[:], in_=s_tab[t])

                pa = ps.tile([B, DT], f32, tag="ps", name="pa3")
                for c in range(ECH):
                    g = gp.tile([P, B], bf16, tag="g3")
                    nc.gpsimd.indirect_dma_start(
                        out=g[:],
                        out_offset=None,
                        in_=q_full[:],
                        in_offset=bass.IndirectOffsetOnAxis(
                            ap=idx_t[:, c : c + 1], axis=0
                        ),
                    )
                    nc.tensor.matmul(
                        out=pa[:],
                        lhsT=g[:],
                        rhs=s_t[:, c * DT : (c + 1) * DT],
                        start=(c == 0),
                        stop=(c == ECH - 1),
                    )
                o_t = mp.tile([B, DT], f32, tag="ot")
                nc.vector.tensor_copy(out=o_t[:], in_=pa[:])
                nc.sync.dma_start(out=out_t[:, t * DT : (t + 1) * DT], in_=o_t[:])

    nc.finalize()
    return nc


_CACHE: dict = {}


def kernel(**inputs: np.ndarray) -> np.ndarray:
    import ml_dtypes

    nodes = np.asarray(inputs["nodes"], dtype=np.float32)
    edge_index = np.asarray(inputs["edge_index"])
    img = np.asarray(inputs["img"], dtype=np.float32)
    W1 = np.asarray(inputs["W1"], dtype=np.float32)
    b1 = np.asarray(inputs["b1"], dtype=np.float32)
    W2 = np.asarray(inputs["W2"], dtype=np.float32)
    b2 = np.asarray(inputs["b2"], dtype=np.float32)
    W3 = np.asarray(inputs["W3"], dtype=np.float32)
    b3 = np.asarray(inputs["b3"], dtype=np.float32)

    ECH, idxA, idxB, S = _preprocess(edge_index)
    S = S.astype(ml_dtypes.bfloat16)
    use_b1 = bool(np.any(b1))
    use_b2 = bool(np.any(b2))

    # per-core unique source rows + remapped layer-2 indices
    uniq = []   # [NCORES] arrays of unique slab-row ids
    idxL2 = []  # [NCORES][NDT, P, 2*ECH] int32 doubled half-row positions
    for k in range(NCORES):
        u, inv = np.unique(idxB[k], return_inverse=True)
        uniq.append(u.astype(np.int32))
        posk = inv.reshape(idxB[k].shape).astype(np.int32)  # [NDT, P, ECH]
        idxL2.append(np.concatenate([2 * posk, 2 * posk + 1], axis=2))
    NUCH = max(-(-len(u) // P) for u in uniq)

    key = (ECH, NUCH, use_b1, use_b2)
    if key not in _CACHE:
        _CACHE[key] = _build(ECH, NUCH, use_b1, use_b2)
    nc = _CACHE[key]

    w3img = (W3.astype(np.float32) @ img.astype(np.float32).T).astype(
        ml_dtypes.bfloat16
    )  # [H, B]
    outbias = img @ b3  # [B]

    nodes_r = nodes.astype(ml_dtypes.bfloat16)
    w1_r = W1.astype(ml_dtypes.bfloat16)
    w2_r = W2.astype(ml_dtypes.bfloat16)
    b1_r = b1.reshape(1, H).astype(ml_dtypes.bfloat16)
    b2_r = _round_fp32r(np.ascontiguousarray(b2.reshape(NFI2, P).T))
    ident = np.eye(P, dtype=ml_dtypes.bfloat16)

    in_maps = []
    for k in range(NCORES):
        g1 = nodes_r[idxA[k]].reshape(NDT, P, ECH * D)
        u_pad = np.zeros(NUCH * P, dtype=np.int32)
        u_pad[: len(uniq[k])] = uniq[k]
        in_maps.append(
            {
                "G1": np.ascontiguousarray(g1),
                "W1": w1_r,
                "W2": w2_r,
                "W3img": w3img,
                "b1": b1_r,
                "b2": b2_r,
                "IDENT": ident,
                "idxU": u_pad.reshape(NUCH, P),
                "idxL2": np.ascontiguousarray(idxL2[k]),
                "idxQ": np.ascontiguousarray(idxB[k]),
                "S": np.ascontiguousarray(S[k]),
            }
        )

    res = run_bass_kernel_spmd(nc, in_maps, core_ids=list(range(NCORES)))

    full = np.concatenate([res.results[k]["out"] for k in range(NCORES)], axis=1)
    n_ids = np.arange(N_SKIP, N)
    cols = (n_ids // NODES_PER) * SLAB + (n_ids % NODES_PER)
    out = full[:, cols] + outbias[:, None]
    return out.astype(np.float32)


if __name__ == "__main__":
    rng = np.random.default_rng(0)
    ins = {
        "nodes": rng.standard_normal((N, D)).astype(np.float32),
        "edge_index": rng.integers(0, N, size=(2, E)).astype(np.int64),
        "img": rng.standard_normal((B, D)).astype(np.float32),
        "W1": (rng.standard_normal((D, H)) * 0.02).astype(np.float32),
        "b1": np.zeros(H, np.float32),
        "W2": (rng.standard_normal((H, H)) * 0.02).astype(np.float32),
        "b2": np.zeros(H, np.float32),
        "W3": (rng.standard_normal((H, D)) * 0.02).astype(np.float32),
        "b3": np.zeros(D, np.float32),
    }
    out = kernel(**ins)
    print("out", out.shape, out.dtype, np.abs(out).mean())


# revision 13
# speedup vs baseline: 1.4673x; 1.0769x over previous
"""3-layer GCN + img@pair_embed.T for Trainium2, distributed over 8 NeuronCores.

Strategy (destination-sharded graph parallelism, agg1-exchange variant):
  - Each core owns a contiguous slab of destination nodes (3567, padded 3584).
  - Edges (plus self-loops) are bucketed per 256-destination tile and padded to
    128-edge chunks. Host builds per chunk a dense [128 edges x 256 dests]
    one-hot norm matrix S, so segment-sum aggregation becomes TensorE matmuls.
  - Layer-1 source rows are PRE-GATHERED ON HOST (X is a static input), and the
    layer-1 aggregation computes agg1 = A@X directly in node-row orientation
    (lhsT = S chunk), so agg1 [SLAB, 512] is written without any transpose.
  - KEY: the cross-core exchange moves agg1 (512 wide) instead of h1 (2048
    wide): ONE AllGather of [SLAB,512] -> [8*SLAB,512] (29MB out) instead of
    117MB. Each core then recomputes h1 = relu(agg1 @ W1) for only the unique
    source rows its layer-2/3 edges touch (~13k rows): gather agg1 rows,
    PE-transpose them into contraction layout, GEMM against resident W1.
  - Layer 2 gathers 1024-wide half-rows of the local recomputed h1_u in two
    passes (PSUM has only 8 accumulation banks), GEMMs in dtile pairs
    (free dim 512), and folds img into layer 3: W3img = W3@img.T, Q = h2@W3img.
  - Layer 3 aggregates 64-wide Q after a small Q AllGather.
  - Everything exchanged/gathered travels bf16; W1 float32r; W2/W3img bf16;
    PSUM accumulation fp32.
"""

import numpy as np

from concourse import bacc, bass, mybir
from concourse import tile as tile_mod
from concourse.bass_utils import run_bass_kernel_spmd

# Problem shapes (hardcoded per spec nn_GraphModel_26268019982828)
N = 28535
E = 113000
D = 512
H = 2048
B = 64
N_SKIP = 115 + 245  # attrs + objs; pair nodes are N_SKIP..N-1

NCORES = 8
NODES_PER = -(-N // NCORES)  # 3567
P = 128
DT = 256  # destination tile width
NDT = 14  # dest tiles per core
SLAB = NDT * DT  # 3584 padded dests per core
NFI1 = D // P  # 4 feature chunks of layer-1 width
NFI2 = H // P  # 16 feature chunks of hidden width

f32 = mybir.dt.float32
f32r = mybir.dt.float32r
bf16 = mybir.dt.bfloat16
i32 = mybir.dt.int32


def _round_fp32r(x: np.ndarray) -> np.ndarray:
    """Round-to-nearest-even fp32 -> fp32r (11-bit mantissa), numpy."""
    u = np.ascontiguousarray(x, dtype=np.float32).view(np.uint32)
    r = u + (0x7FF + ((u >> 12) & np.uint32(1)))
    r &= np.uint32(0xFFFFF000)
    return r.view(np.float32)


def _preprocess(edge_index: np.ndarray):
    """Sort/bucket edges by destination; build gather indices + S blocks."""
    src = np.concatenate([edge_index[0], np.arange(N, dtype=np.int64)])
    dst = np.concatenate([edge_index[1], np.arange(N, dtype=np.int64)])
    deg = np.bincount(dst, minlength=N).astype(np.float32)  # includes loops
    dinv = (1.0 / np.sqrt(deg)).astype(np.float32)
    norm = (dinv[src] * dinv[dst]).astype(np.float32)

    core = (dst // NODES_PER).astype(np.int64)
    local = (dst - core * NODES_PER).astype(np.int64)
    t_idx = local // DT
    d_local = local % DT
    bucket = core * NDT + t_idx

    order = np.argsort(bucket, kind="stable")
    src_s = src[order]
    bucket_s = bucket[order]
    dl_s = d_local[order]
    norm_s = norm[order]

    counts = np.bincount(bucket_s, minlength=NCORES * NDT)
    ECH = int(-(-counts.max() // P))

    idxA = np.zeros((NCORES, NDT, P, ECH), dtype=np.int32)
    idxB = np.zeros((NCORES, NDT, P, ECH), dtype=np.int32)
    S = np.zeros((NCORES, NDT, P, ECH * DT), dtype=np.float32)

    starts = np.zeros(NCORES * NDT + 1, dtype=np.int64)
    np.cumsum(counts, out=starts[1:])
    pos = np.arange(len(bucket_s)) - starts[bucket_s]
    c_idx = pos // P
    e_idx = pos % P

    ci = bucket_s // NDT
    ti = bucket_s % NDT
    srcB = (src_s // NODES_PER) * SLAB + (src_s % NODES_PER)
    idxA[ci, ti, e_idx, c_idx] = src_s.astype(np.int32)
    idxB[ci, ti, e_idx, c_idx] = srcB.astype(np.int32)
    S[ci, ti, e_idx, c_idx * DT + dl_s] = norm_s
    return ECH, idxA, idxB, S


def _build(ECH: int, NUCH: int, use_b1: bool, use_b2: bool):
    nc = bacc.Bacc("TRN2", target_bir_lowering=False, num_devices=NCORES)
    NU = NUCH * P  # padded unique-source rows per core

    g1_t = nc.dram_tensor("G1", [NDT, P, ECH * D], bf16, kind="ExternalInput")
    w1_t = nc.dram_tensor("W1", [D, H], bf16, kind="ExternalInput")
    w2_t = nc.dram_tensor("W2", [H, H], bf16, kind="ExternalInput")
    w3i_t = nc.dram_tensor("W3img", [H, B], bf16, kind="ExternalInput")
    b1_t = nc.dram_tensor("b1", [1, H], bf16, kind="ExternalInput")
    b2_t = nc.dram_tensor("b2", [P, NFI2], f32r, kind="ExternalInput")
    idxU_t = nc.dram_tensor("idxU", [NUCH, P], i32, kind="ExternalInput")
    idxL2_t = nc.dram_tensor("idxL2", [NDT, P, 2 * ECH], i32, kind="ExternalInput")
    idxQ_t = nc.dram_tensor("idxQ", [NDT, P, ECH], i32, kind="ExternalInput")
    s_tab = nc.dram_tensor("S", [NDT, P, ECH * DT], bf16, kind="ExternalInput")
    ident_t = nc.dram_tensor("IDENT", [P, P], bf16, kind="ExternalInput")
    out_t = nc.dram_tensor("out", [B, SLAB], f32, kind="ExternalOutput")

    agg1p = nc.dram_tensor("agg1p", [SLAB, D], bf16)
    agg1f = nc.dram_tensor("agg1f", [SLAB * NCORES, D], bf16, addr_space="Shared")
    h1u = nc.dram_tensor("h1u", [NU, H], bf16)
    q_slab = nc.dram_tensor("q_slab", [SLAB, B], bf16)
    q_full = nc.dram_tensor("q_full", [SLAB * NCORES, B], bf16, addr_space="Shared")

    rg = [list(range(NCORES))]

    with tile_mod.TileContext(nc) as tc:
        with (
            tc.tile_pool(name="w", bufs=1) as wp,
            tc.tile_pool(name="gio", bufs=6) as gp,
            tc.tile_pool(name="g1io", bufs=3) as g1p,
            tc.tile_pool(name="stab", bufs=3) as sp,
            tc.tile_pool(name="agg", bufs=1) as ap,
            tc.tile_pool(name="small", bufs=3) as mp,
            tc.tile_pool(name="hout", bufs=2) as hp,
            tc.tile_pool(name="consts", bufs=1) as cp,
            tc.tile_pool(name="ps", bufs=8, space="PSUM") as ps,
        ):
            # --- resident weights ---
            w1sb = []
            for fi in range(NFI1):
                w = wp.tile([P, H], bf16, tag="w1", name="wsb", bufs=NFI1)
                nc.sync.dma_start(out=w[:], in_=w1_t[fi * P : (fi + 1) * P, :])
                w1sb.append(w)
            w2sb = []
            for fi in range(NFI2):
                w = wp.tile([P, H], bf16, tag="w2", name="w2sb", bufs=NFI2)
                nc.sync.dma_start(out=w[:], in_=w2_t[fi * P : (fi + 1) * P, :])
                w2sb.append(w)
            w3sb = []
            for fo in range(NFI2):
                w = wp.tile([P, B], bf16, tag="w3", name="w3sb", bufs=NFI2)
                nc.sync.dma_start(out=w[:], in_=w3i_t[fo * P : (fo + 1) * P, :])
                w3sb.append(w)
            idt = cp.tile([P, P], bf16, tag="idt")
            nc.sync.dma_start(out=idt[:], in_=ident_t[:])
            if use_b1:
                b1sb = cp.tile([1, H], bf16, tag="b1")
                nc.sync.dma_start(out=b1sb[:], in_=b1_t[:])
                ones1 = cp.tile([1, P], bf16, tag="ones")
                nc.gpsimd.memset(ones1[:], 1.0)
            if use_b2:
                b2sb = cp.tile([P, NFI2], f32r, tag="b2")
                nc.sync.dma_start(out=b2sb[:], in_=b2_t[:])

            relu = mybir.ActivationFunctionType.Relu

            # ---------------- Layer 1: agg1 = A@X  (node-row orientation) ---
            for t in range(NDT):
                g1 = g1p.tile([P, ECH * D], bf16, tag="g1")
                nc.sync.dma_start(out=g1[:], in_=g1_t[t])
                s_t = sp.tile([P, ECH * DT], bf16, tag="s")
                nc.sync.dma_start(out=s_t[:], in_=s_tab[t])

                pd = [ps.tile([P, D], f32, tag="ps", name="pd") for _ in range(2)]
                for c in range(ECH):
                    for dh in range(2):
                        nc.tensor.matmul(
                            out=pd[dh][:],
                            lhsT=s_t[:, c * DT + dh * P : c * DT + (dh + 1) * P],
                            rhs=g1[:, c * D : (c + 1) * D],
                            start=(c == 0),
                            stop=(c == ECH - 1),
                        )
                for dh in range(2):
                    a1 = mp.tile([P, D], bf16, tag="a1")
                    nc.vector.tensor_copy(out=a1[:], in_=pd[dh][:])
                    nc.sync.dma_start(
                        out=agg1p[t * DT + dh * P : t * DT + (dh + 1) * P, :],
                        in_=a1[:],
                    )

            NOWN = SLAB // P  # 28 chunks of own-slab rows, recomputed locally

            def recompute_chunk(u, src_tab):
                idxu = mp.tile([P, 1], i32, tag="idxu")
                nc.sync.dma_start(out=idxu[:], in_=idxU_t[u : u + 1].rearrange("o p -> p o"))
                gu = gp.tile([P, D], bf16, tag="gu")
                nc.gpsimd.indirect_dma_start(
                    out=gu[:],
                    out_offset=None,
                    in_=src_tab[:],
                    in_offset=bass.IndirectOffsetOnAxis(ap=idxu[:, 0:1], axis=0),
                )
                aT = []
                for fi in range(NFI1):
                    pt = ps.tile([P, P], bf16, tag="ps", name="pt")
                    nc.tensor.transpose(
                        out=pt[:], in_=gu[:, fi * P : (fi + 1) * P], identity=idt[:]
                    )
                    a = ap.tile([P, P], bf16, tag="aT", name="aTt", bufs=8)
                    nc.vector.tensor_copy(out=a[:], in_=pt[:])
                    aT.append(a)
                h1u_t = hp.tile([P, H], bf16, tag="hout")
                pz = [ps.tile([P, D], f32, tag="ps", name="pz") for _ in range(NFI1)]
                if use_b1:
                    for fo in range(NFI1):
                        nc.tensor.matmul(
                            out=pz[fo][:],
                            lhsT=ones1[:1, :],
                            rhs=b1sb[:1, fo * D : (fo + 1) * D],
                            start=True,
                            stop=False,
                        )
                for fi in range(NFI1):
                    for fo in range(NFI1):
                        nc.tensor.matmul(
                            out=pz[fo][:],
                            lhsT=aT[fi][:],
                            rhs=w1sb[fi][:, fo * D : (fo + 1) * D],
                            start=(fi == 0 and not use_b1),
                            stop=(fi == NFI1 - 1),
                        )
                for fo in range(NFI1):
                    nc.scalar.activation(
                        out=h1u_t[:, fo * D : (fo + 1) * D], in_=pz[fo][:], func=relu
                    )
                nc.sync.dma_start(out=h1u[u * P : (u + 1) * P, :], in_=h1u_t[:])

            nc.gpsimd.collective_compute(
                "AllGather",
                mybir.AluOpType.bypass,
                replica_groups=rg,
                ins=[agg1p[:]],
                outs=[agg1f[:]],
            )

            # own-slab rows: local gather, fills the AllGather shadow
            for u in range(NOWN):
                recompute_chunk(u, agg1p)
            # remote rows: gated on the AllGather
            for u in range(NOWN, NUCH):
                recompute_chunk(u, agg1f)

            # ---------------- Layer 2 + Q (dtile pairs) --------------------
            NH = NFI2 // 2  # 8 feature chunks per half-row pass
            h1u_half = h1u[:].rearrange("n (h d) -> (n h) d", h=2)
            for tp in range(NDT // 2):
                aggT = [
                    ap.tile([P, 2 * DT], bf16, tag="aggT2", name="aggTt2", bufs=NFI2)
                    for _ in range(NFI2)
                ]
                for t2 in range(2):
                    t = tp * 2 + t2
                    idx_t = mp.tile([P, 2 * ECH], i32, tag="idx")
                    nc.sync.dma_start(out=idx_t[:], in_=idxL2_t[t])
                    s_t = sp.tile([P, ECH * DT], bf16, tag="s")
                    nc.sync.dma_start(out=s_t[:], in_=s_tab[t])

                    for hf in range(2):
                        pa = [ps.tile([P, DT], f32, tag="ps", name="pa2") for _ in range(NH)]
                        for c in range(ECH):
                            g = gp.tile([P, H // 2], bf16, tag="g")
                            nc.gpsimd.indirect_dma_start(
                                out=g[:],
                                out_offset=None,
                                in_=h1u_half,
                                in_offset=bass.IndirectOffsetOnAxis(
                                    ap=idx_t[:, hf * ECH + c : hf * ECH + c + 1], axis=0
                                ),
                            )
                            for j in range(NH):
                                nc.tensor.matmul(
                                    out=pa[j][:],
                                    lhsT=g[:, j * P : (j + 1) * P],
                                    rhs=s_t[:, c * DT : (c + 1) * DT],
                                    start=(c == 0),
                                    stop=(c == ECH - 1),
                                )
                        for j in range(NH):
                            fi = hf * NH + j
                            nc.vector.tensor_copy(
                                out=aggT[fi][:, t2 * DT : (t2 + 1) * DT], in_=pa[j][:]
                            )

                pq = [ps.tile([P, B], f32, tag="ps", name="pq") for _ in range(4)]
                for fo in range(NFI2):
                    pz = ps.tile([P, 2 * DT], f32, tag="ps", name="pz2")
                    for fi in range(NFI2):
                        nc.tensor.matmul(
                            out=pz[:],
                            lhsT=w2sb[fi][:, fo * P : (fo + 1) * P],
                            rhs=aggT[fi][:],
                            start=(fi == 0),
                            stop=(fi == NFI2 - 1),
                        )
                    h2t = ap.tile([P, 2 * DT], bf16, tag="h2", name="h2t", bufs=NFI2)
                    if use_b2:
                        nc.scalar.activation(
                            out=h2t[:], in_=pz[:], func=relu,
                            bias=b2sb[:, fo : fo + 1],
                        )
                    else:
                        nc.scalar.activation(out=h2t[:], in_=pz[:], func=relu)

                    for dh in range(4):
                        nc.tensor.matmul(
                            out=pq[dh][:],
                            lhsT=h2t[:, dh * P : (dh + 1) * P],
                            rhs=w3sb[fo][:],
                            start=(fo == 0),
                            stop=(fo == NFI2 - 1),
                        )
                for dh in range(4):
                    qn = mp.tile([P, B], bf16, tag="qn")
                    nc.vector.tensor_copy(out=qn[:], in_=pq[dh][:])
                    nc.sync.dma_start(
                        out=q_slab[tp * 2 * DT + dh * P : tp * 2 * DT + (dh + 1) * P, :],
                        in_=qn[:],
                    )

            nc.gpsimd.collective_compute(
                "AllGather",
                mybir.AluOpType.bypass,
                replica_groups=rg,
                ins=[q_slab[:]],
                outs=[q_full[:]],
            )

            # ---------------- Layer 3 (= output) ---------------------------
            for t in range(NDT):
                idx_t = mp.tile([P, ECH], i32, tag="idxq")
                nc.sync.dma_start(out=idx_t[:], in_=idxQ_t[t])
                s_t = sp.tile([P, ECH * DT], bf16, tag="s")
                nc.sync.dma_start(out=s_t[:], in_=s_tab[t])

                pa = ps.tile([B, DT], f32, tag="ps", name="pa3")
                for c in range(ECH):
                    g = gp.tile([P, B], bf16, tag="g3")
                    nc.gpsimd.indirect_dma_start(
                        out=g[:],
                        out_offset=None,
                        in_=q_full[:],
                        in_offset=bass.IndirectOffsetOnAxis(
                            ap=idx_t[:, c : c + 1], axis=0
                        ),
                    )
                    nc.tensor.matmul(
                        out=pa[:],
                        lhsT=g[:],
                        rhs=s_t[:, c * DT : (c + 1) * DT],
                        start=(c == 0),
                        stop=(c == ECH - 1),
                    )
                o_t = mp.tile([B, DT], f32, tag="ot")
                nc.vector.tensor_copy(out=o_t[:], in_=pa[:])
                nc.sync.dma_start(out=out_t[:, t * DT : (t + 1) * DT], in_=o_t[:])

    nc.finalize()
    return nc


_CACHE: dict = {}


def kernel(**inputs: np.ndarray) -> np.ndarray:
    import ml_dtypes

    nodes = np.asarray(inputs["nodes"], dtype=np.float32)
    edge_index = np.asarray(inputs["edge_index"])
    img = np.asarray(inputs["img"], dtype=np.float32)
    W1 = np.asarray(inputs["W1"], dtype=np.float32)
    b1 = np.asarray(inputs["b1"], dtype=np.float32)
    W2 = np.asarray(inputs["W2"], dtype=np.float32)
    b2 = np.asarray(inputs["b2"], dtype=np.float32)
    W3 = np.asarray(inputs["W3"], dtype=np.float32)
    b3 = np.asarray(inputs["b3"], dtype=np.float32)

    ECH, idxA, idxB, S = _preprocess(edge_index)
    S = S.astype(ml_dtypes.bfloat16)
    use_b1 = bool(np.any(b1))
    use_b2 = bool(np.any(b2))

    # per-core source rows: all 3584 own-slab rows first (recomputed locally,
    # hidden under the AllGather), then unique remote rows.
    uniq = []   # [NCORES] arrays of remote agg1f row ids, sorted
    idxL2 = []  # [NCORES][NDT, P, 2*ECH] int32 doubled half-row positions
    for k in range(NCORES):
        own_lo, own_hi = k * SLAB, (k + 1) * SLAB
        rem = np.unique(idxB[k])
        rem = rem[(rem < own_lo) | (rem >= own_hi)]
        uniq.append(rem.astype(np.int32))
        pos_map = np.zeros(NCORES * SLAB, dtype=np.int32)
        pos_map[own_lo:own_hi] = np.arange(SLAB)
        pos_map[rem] = SLAB + np.arange(len(rem))
        posk = pos_map[idxB[k]]  # [NDT, P, ECH]
        idxL2.append(np.concatenate([2 * posk, 2 * posk + 1], axis=2))
    NOWN = SLAB // P
    NUCH = NOWN + max(-(-len(u) // P) for u in uniq)

    key = (ECH, NUCH, use_b1, use_b2)
    if key not in _CACHE:
        _CACHE[key] = _build(ECH, NUCH, use_b1, use_b2)
    nc = _CACHE[key]

    w3img = (W3.astype(np.float32) @ img.astype(np.float32).T).astype(
        ml_dtypes.bfloat16
    )  # [H, B]
    outbias = img @ b3  # [B]

    nodes_r = nodes.astype(ml_dtypes.bfloat16)
    w1_r = W1.astype(ml_dtypes.bfloat16)
    w2_r = W2.astype(ml_dtypes.bfloat16)
    b1_r = b1.reshape(1, H).astype(ml_dtypes.bfloat16)
    b2_r = _round_fp32r(np.ascontiguousarray(b2.reshape(NFI2, P).T))
    ident = np.eye(P, dtype=ml_dtypes.bfloat16)

    in_maps = []
    for k in range(NCORES):
        g1 = nodes_r[idxA[k]].reshape(NDT, P, ECH * D)
        u_pad = np.zeros(NUCH * P, dtype=np.int32)
        u_pad[:SLAB] = np.arange(SLAB)  # own rows: local agg1p row ids
        u_pad[SLAB : SLAB + len(uniq[k])] = uniq[k]
        in_maps.append(
            {
                "G1": np.ascontiguousarray(g1),
                "W1": w1_r,
                "W2": w2_r,
                "W3img": w3img,
                "b1": b1_r,
                "b2": b2_r,
                "IDENT": ident,
                "idxU": u_pad.reshape(NUCH, P),
                "idxL2": np.ascontiguousarray(idxL2[k]),
                "idxQ": np.ascontiguousarray(idxB[k]),
                "S": np.ascontiguousarray(S[k]),
            }
        )

    res = run_bass_kernel_spmd(nc, in_maps, core_ids=list(range(NCORES)))

    full = np.concatenate([res.results[k]["out"] for k in range(NCORES)], axis=1)
    n_ids = np.arange(N_SKIP, N)
    cols = (n_ids // NODES_PER) * SLAB + (n_ids % NODES_PER)
    out = full[:, cols] + outbias[:, None]
    return out.astype(np.float32)


if __name__ == "__main__":
    rng = np.random.default_rng(0)
    ins = {
        "nodes": rng.standard_normal((N, D)).astype(np.float32),
        "edge_index": rng.integers(0, N, size=(2, E)).astype(np.int64),
        "img": rng.standard_normal((B, D)).astype(np.float32),
        "W1": (rng.standard_normal((D, H)) * 0.02).astype(np.float32),
        "b1": np.zeros(H, np.float32),
        "W2": (rng.standard_normal((H, H)) * 0.02).astype(np.float32),
        "b2": np.zeros(H, np.float32),
        "W3": (rng.standard_normal((H, D)) * 0.02).astype(np.float32),
        "b3": np.zeros(D, np.float32),
    }
    out = kernel(**ins)
    print("out", out.shape, out.dtype, np.abs(out).mean())


# revision 14
# speedup vs baseline: 1.5209x; 1.0365x over previous
"""3-layer GCN + img@pair_embed.T for Trainium2, distributed over 8 NeuronCores.

Strategy (destination-sharded graph parallelism, agg1-exchange variant):
  - Each core owns a contiguous slab of destination nodes (3567, padded 3584).
  - Edges (plus self-loops) are bucketed per 256-destination tile and padded to
    128-edge chunks. Host builds per chunk a dense [128 edges x 256 dests]
    one-hot norm matrix S, so segment-sum aggregation becomes TensorE matmuls.
  - Layer-1 source rows are PRE-GATHERED ON HOST (X is a static input), and the
    layer-1 aggregation computes agg1 = A@X directly in node-row orientation
    (lhsT = S chunk), so agg1 [SLAB, 512] is written without any transpose.
  - KEY: the cross-core exchange moves agg1 (512 wide) instead of h1 (2048
    wide): ONE AllGather of [SLAB,512] -> [8*SLAB,512] (29MB out) instead of
    117MB. Each core then recomputes h1 = relu(agg1 @ W1) for only the unique
    source rows its layer-2/3 edges touch (~13k rows): gather agg1 rows,
    PE-transpose them into contraction layout, GEMM against resident W1.
  - Layer 2 gathers 1024-wide half-rows of the local recomputed h1_u in two
    passes (PSUM has only 8 accumulation banks), GEMMs in dtile pairs
    (free dim 512), and folds img into layer 3: W3img = W3@img.T, Q = h2@W3img.
  - Layer 3 aggregates 64-wide Q after a small Q AllGather.
  - Everything exchanged/gathered travels bf16; W1 float32r; W2/W3img bf16;
    PSUM accumulation fp32.
"""

import numpy as np

from concourse import bacc, bass, mybir
from concourse import tile as tile_mod
from concourse.bass_utils import run_bass_kernel_spmd

# Problem shapes (hardcoded per spec nn_GraphModel_26268019982828)
N = 28535
E = 113000
D = 512
H = 2048
B = 64
N_SKIP = 115 + 245  # attrs + objs; pair nodes are N_SKIP..N-1

NCORES = 8
NODES_PER = -(-N // NCORES)  # 3567
P = 128
DT = 256  # destination tile width
NDT = 14  # dest tiles per core
SLAB = NDT * DT  # 3584 padded dests per core
NFI1 = D // P  # 4 feature chunks of layer-1 width
NFI2 = H // P  # 16 feature chunks of hidden width

f32 = mybir.dt.float32
f32r = mybir.dt.float32r
bf16 = mybir.dt.bfloat16
i32 = mybir.dt.int32


def _round_fp32r(x: np.ndarray) -> np.ndarray:
    """Round-to-nearest-even fp32 -> fp32r (11-bit mantissa), numpy."""
    u = np.ascontiguousarray(x, dtype=np.float32).view(np.uint32)
    r = u + (0x7FF + ((u >> 12) & np.uint32(1)))
    r &= np.uint32(0xFFFFF000)
    return r.view(np.float32)


def _preprocess(edge_index: np.ndarray):
    """Sort/bucket edges by destination; build gather indices + S blocks."""
    src = np.concatenate([edge_index[0], np.arange(N, dtype=np.int64)])
    dst = np.concatenate([edge_index[1], np.arange(N, dtype=np.int64)])
    deg = np.bincount(dst, minlength=N).astype(np.float32)  # includes loops
    dinv = (1.0 / np.sqrt(deg)).astype(np.float32)
    norm = (dinv[src] * dinv[dst]).astype(np.float32)

    core = (dst // NODES_PER).astype(np.int64)
    local = (dst - core * NODES_PER).astype(np.int64)
    t_idx = local // DT
    d_local = local % DT
    bucket = core * NDT + t_idx

    order = np.argsort(bucket, kind="stable")
    src_s = src[order]
    bucket_s = bucket[order]
    dl_s = d_local[order]
    norm_s = norm[order]

    counts = np.bincount(bucket_s, minlength=NCORES * NDT)
    ECH = int(-(-counts.max() // P))

    idxA = np.zeros((NCORES, NDT, P, ECH), dtype=np.int32)
    idxB = np.zeros((NCORES, NDT, P, ECH), dtype=np.int32)
    S = np.zeros((NCORES, NDT, P, ECH * DT), dtype=np.float32)

    starts = np.zeros(NCORES * NDT + 1, dtype=np.int64)
    np.cumsum(counts, out=starts[1:])
    pos = np.arange(len(bucket_s)) - starts[bucket_s]
    c_idx = pos // P
    e_idx = pos % P

    ci = bucket_s // NDT
    ti = bucket_s % NDT
    srcB = (src_s // NODES_PER) * SLAB + (src_s % NODES_PER)
    idxA[ci, ti, e_idx, c_idx] = src_s.astype(np.int32)
    idxB[ci, ti, e_idx, c_idx] = srcB.astype(np.int32)
    S[ci, ti, e_idx, c_idx * DT + dl_s] = norm_s
    return ECH, idxA, idxB, S


def _build(ECH: int, NUCH: int, use_b1: bool, use_b2: bool):
    nc = bacc.Bacc("TRN2", target_bir_lowering=False, num_devices=NCORES)
    NU = NUCH * P  # padded unique-source rows per core

    g1_t = nc.dram_tensor("G1", [NDT, P, ECH * D], bf16, kind="ExternalInput")
    w1_t = nc.dram_tensor("W1", [D, H], bf16, kind="ExternalInput")
    w2_t = nc.dram_tensor("W2", [H, H], bf16, kind="ExternalInput")
    w3i_t = nc.dram_tensor("W3img", [H, B], bf16, kind="ExternalInput")
    b1_t = nc.dram_tensor("b1", [1, H], bf16, kind="ExternalInput")
    b2_t = nc.dram_tensor("b2", [P, NFI2], f32r, kind="ExternalInput")
    idxU_t = nc.dram_tensor("idxU", [NUCH, P], i32, kind="ExternalInput")
    idxL2_t = nc.dram_tensor("idxL2", [NDT, P, 2 * ECH], i32, kind="ExternalInput")
    idxQ_t = nc.dram_tensor("idxQ", [NDT, P, ECH], i32, kind="ExternalInput")
    s_tab = nc.dram_tensor("S", [NDT, P, ECH * DT], bf16, kind="ExternalInput")
    ident_t = nc.dram_tensor("IDENT", [P, P], bf16, kind="ExternalInput")
    out_t = nc.dram_tensor("out", [B, SLAB], f32, kind="ExternalOutput")

    agg1p = nc.dram_tensor("agg1p", [SLAB, D], bf16)
    agg1f = nc.dram_tensor("agg1f", [SLAB * NCORES, D], bf16, addr_space="Shared")
    h1u = nc.dram_tensor("h1u", [NU, H], bf16)
    q_slab = nc.dram_tensor("q_slab", [SLAB, B], bf16)
    q_full = nc.dram_tensor("q_full", [SLAB * NCORES, B], bf16, addr_space="Shared")

    rg = [list(range(NCORES))]

    with tile_mod.TileContext(nc) as tc:
        with (
            tc.tile_pool(name="w", bufs=1) as wp,
            tc.tile_pool(name="gio", bufs=6) as gp,
            tc.tile_pool(name="g1io", bufs=3) as g1p,
            tc.tile_pool(name="stab", bufs=3) as sp,
            tc.tile_pool(name="agg", bufs=1) as ap,
            tc.tile_pool(name="small", bufs=3) as mp,
            tc.tile_pool(name="hout", bufs=2) as hp,
            tc.tile_pool(name="consts", bufs=1) as cp,
            tc.tile_pool(name="ps", bufs=8, space="PSUM") as ps,
        ):
            # --- resident weights ---
            w1sb = []
            for fi in range(NFI1):
                w = wp.tile([P, H], bf16, tag="w1", name="wsb", bufs=NFI1)
                nc.sync.dma_start(out=w[:], in_=w1_t[fi * P : (fi + 1) * P, :])
                w1sb.append(w)
            idt = cp.tile([P, P], bf16, tag="idt")
            nc.sync.dma_start(out=idt[:], in_=ident_t[:])
            if use_b1:
                b1sb = cp.tile([1, H], bf16, tag="b1")
                nc.sync.dma_start(out=b1sb[:], in_=b1_t[:])
                ones1 = cp.tile([1, P], bf16, tag="ones")
                nc.gpsimd.memset(ones1[:], 1.0)
            if use_b2:
                b2sb = cp.tile([P, NFI2], f32r, tag="b2")
                nc.sync.dma_start(out=b2sb[:], in_=b2_t[:])

            relu = mybir.ActivationFunctionType.Relu

            # ---------------- Layer 1: agg1 = A@X  (node-row orientation) ---
            for t in range(NDT):
                g1 = g1p.tile([P, ECH * D], bf16, tag="g1")
                nc.sync.dma_start(out=g1[:], in_=g1_t[t])
                s_t = sp.tile([P, ECH * DT], bf16, tag="s")
                nc.sync.dma_start(out=s_t[:], in_=s_tab[t])

                pd = [ps.tile([P, D], f32, tag="ps", name="pd") for _ in range(2)]
                for c in range(ECH):
                    for dh in range(2):
                        nc.tensor.matmul(
                            out=pd[dh][:],
                            lhsT=s_t[:, c * DT + dh * P : c * DT + (dh + 1) * P],
                            rhs=g1[:, c * D : (c + 1) * D],
                            start=(c == 0),
                            stop=(c == ECH - 1),
                        )
                for dh in range(2):
                    a1 = mp.tile([P, D], bf16, tag="a1")
                    nc.vector.tensor_copy(out=a1[:], in_=pd[dh][:])
                    nc.sync.dma_start(
                        out=agg1p[t * DT + dh * P : t * DT + (dh + 1) * P, :],
                        in_=a1[:],
                    )

            NOWN = SLAB // P  # 28 chunks of own-slab rows, recomputed locally

            def rec_front(u, src_tab):
                """Gather + transpose chunk u; returns aT tiles."""
                idxu = mp.tile([P, 1], i32, tag="idxu")
                nc.sync.dma_start(out=idxu[:], in_=idxU_t[u : u + 1].rearrange("o p -> p o"))
                gu = gp.tile([P, D], bf16, tag="gu")
                nc.gpsimd.indirect_dma_start(
                    out=gu[:],
                    out_offset=None,
                    in_=src_tab[:],
                    in_offset=bass.IndirectOffsetOnAxis(ap=idxu[:, 0:1], axis=0),
                )
                aT = []
                for fi in range(NFI1):
                    pt = ps.tile([P, P], bf16, tag="ps", name="pt")
                    nc.tensor.transpose(
                        out=pt[:], in_=gu[:, fi * P : (fi + 1) * P], identity=idt[:]
                    )
                    a = ap.tile([P, P], bf16, tag="aT", name="aTt", bufs=8)
                    if fi % 2 == 0:
                        nc.vector.tensor_copy(out=a[:], in_=pt[:])
                    else:
                        nc.scalar.activation(
                            out=a[:], in_=pt[:],
                            func=mybir.ActivationFunctionType.Copy,
                        )
                    aT.append(a)
                return aT

            def rec_back(u, aT):
                """GEMM + relu + writeback for chunk u."""
                h1u_t = hp.tile([P, H], bf16, tag="hout")
                pz = [ps.tile([P, D], f32, tag="ps", name="pz") for _ in range(NFI1)]
                if use_b1:
                    for fo in range(NFI1):
                        nc.tensor.matmul(
                            out=pz[fo][:],
                            lhsT=ones1[:1, :],
                            rhs=b1sb[:1, fo * D : (fo + 1) * D],
                            start=True,
                            stop=False,
                        )
                for fi in range(NFI1):
                    for fo in range(NFI1):
                        nc.tensor.matmul(
                            out=pz[fo][:],
                            lhsT=aT[fi][:],
                            rhs=w1sb[fi][:, fo * D : (fo + 1) * D],
                            start=(fi == 0 and not use_b1),
                            stop=(fi == NFI1 - 1),
                        )
                for fo in range(NFI1):
                    nc.scalar.activation(
                        out=h1u_t[:, fo * D : (fo + 1) * D], in_=pz[fo][:], func=relu
                    )
                nc.sync.dma_start(out=h1u[u * P : (u + 1) * P, :], in_=h1u_t[:])

            nc.gpsimd.collective_compute(
                "AllGather",
                mybir.AluOpType.bypass,
                replica_groups=rg,
                ins=[agg1p[:]],
                outs=[agg1f[:]],
            )

            # W2/W3 resident loads: issued after L1's input stream so they
            # ride the AllGather shadow instead of delaying the first dtile.
            w2sb = []
            for fi in range(NFI2):
                w = wp.tile([P, H], bf16, tag="w2", name="w2sb", bufs=NFI2)
                nc.sync.dma_start(out=w[:], in_=w2_t[fi * P : (fi + 1) * P, :])
                w2sb.append(w)
            w3sb = []
            for fo in range(NFI2):
                w = wp.tile([P, B], bf16, tag="w3", name="w3sb", bufs=NFI2)
                nc.sync.dma_start(out=w[:], in_=w3i_t[fo * P : (fo + 1) * P, :])
                w3sb.append(w)

            # software-pipelined: transpose(u) overlaps GEMM(u-1); own-slab
            # chunks (local agg1p) run inside the AllGather shadow.
            prev = None
            for u in range(NUCH):
                aT = rec_front(u, agg1p if u < NOWN else agg1f)
                if prev is not None:
                    rec_back(prev[0], prev[1])
                prev = (u, aT)
            rec_back(prev[0], prev[1])

            # ---------------- Layer 2 + Q (dtile pairs) --------------------
            NH = NFI2 // 2  # 8 feature chunks per half-row pass
            h1u_half = h1u[:].rearrange("n (h d) -> (n h) d", h=2)
            for tp in range(NDT // 2):
                aggT = [
                    ap.tile([P, 2 * DT], bf16, tag="aggT2", name="aggTt2", bufs=NFI2)
                    for _ in range(NFI2)
                ]
                for t2 in range(2):
                    t = tp * 2 + t2
                    idx_t = mp.tile([P, 2 * ECH], i32, tag="idx")
                    nc.sync.dma_start(out=idx_t[:], in_=idxL2_t[t])
                    s_t = sp.tile([P, ECH * DT], bf16, tag="s")
                    nc.sync.dma_start(out=s_t[:], in_=s_tab[t])

                    for hf in range(2):
                        pa = [ps.tile([P, DT], f32, tag="ps", name="pa2") for _ in range(NH)]
                        for c in range(ECH):
                            g = gp.tile([P, H // 2], bf16, tag="g")
                            nc.gpsimd.indirect_dma_start(
                                out=g[:],
                                out_offset=None,
                                in_=h1u_half,
                                in_offset=bass.IndirectOffsetOnAxis(
                                    ap=idx_t[:, hf * ECH + c : hf * ECH + c + 1], axis=0
                                ),
                            )
                            for j in range(NH):
                                nc.tensor.matmul(
                                    out=pa[j][:],
                                    lhsT=g[:, j * P : (j + 1) * P],
                                    rhs=s_t[:, c * DT : (c + 1) * DT],
                                    start=(c == 0),
                                    stop=(c == ECH - 1),
                                )
                        for j in range(NH):
                            fi = hf * NH + j
                            if j % 2 == 0:
                                nc.vector.tensor_copy(
                                    out=aggT[fi][:, t2 * DT : (t2 + 1) * DT], in_=pa[j][:]
                                )
                            else:
                                nc.scalar.activation(
                                    out=aggT[fi][:, t2 * DT : (t2 + 1) * DT],
                                    in_=pa[j][:],
                                    func=mybir.ActivationFunctionType.Copy,
                                )

                pq = [ps.tile([P, B], f32, tag="ps", name="pq") for _ in range(4)]
                for fo in range(NFI2):
                    pz = ps.tile([P, 2 * DT], f32, tag="ps", name="pz2")
                    for fi in range(NFI2):
                        nc.tensor.matmul(
                            out=pz[:],
                            lhsT=w2sb[fi][:, fo * P : (fo + 1) * P],
                            rhs=aggT[fi][:],
                            start=(fi == 0),
                            stop=(fi == NFI2 - 1),
                        )
                    h2t = ap.tile([P, 2 * DT], bf16, tag="h2", name="h2t", bufs=NFI2)
                    if use_b2:
                        nc.scalar.activation(
                            out=h2t[:], in_=pz[:], func=relu,
                            bias=b2sb[:, fo : fo + 1],
                        )
                    else:
                        nc.scalar.activation(out=h2t[:], in_=pz[:], func=relu)

                    for dh in range(4):
                        nc.tensor.matmul(
                            out=pq[dh][:],
                            lhsT=h2t[:, dh * P : (dh + 1) * P],
                            rhs=w3sb[fo][:],
                            start=(fo == 0),
                            stop=(fo == NFI2 - 1),
                        )
                for dh in range(4):
                    qn = mp.tile([P, B], bf16, tag="qn")
                    nc.vector.tensor_copy(out=qn[:], in_=pq[dh][:])
                    nc.sync.dma_start(
                        out=q_slab[tp * 2 * DT + dh * P : tp * 2 * DT + (dh + 1) * P, :],
                        in_=qn[:],
                    )

            nc.gpsimd.collective_compute(
                "AllGather",
                mybir.AluOpType.bypass,
                replica_groups=rg,
                ins=[q_slab[:]],
                outs=[q_full[:]],
            )

            # ---------------- Layer 3 (= output) ---------------------------
            for t in range(NDT):
                idx_t = mp.tile([P, ECH], i32, tag="idxq")
                nc.sync.dma_start(out=idx_t[:], in_=idxQ_t[t])
                s_t = sp.tile([P, ECH * DT], bf16, tag="s")
                nc.sync.dma_start(out=s_t[:], in_=s_tab[t])

                pa = ps.tile([B, DT], f32, tag="ps", name="pa3")
                for c in range(ECH):
                    g = gp.tile([P, B], bf16, tag="g3")
                    nc.gpsimd.indirect_dma_start(
                        out=g[:],
                        out_offset=None,
                        in_=q_full[:],
                        in_offset=bass.IndirectOffsetOnAxis(
                            ap=idx_t[:, c : c + 1], axis=0
                        ),
                    )
                    nc.tensor.matmul(
                        out=pa[:],
                        lhsT=g[:],
                        rhs=s_t[:, c * DT : (c + 1) * DT],
                        start=(c == 0),
                        stop=(c == ECH - 1),
                    )
                o_t = mp.tile([B, DT], f32, tag="ot")
                nc.vector.tensor_copy(out=o_t[:], in_=pa[:])
                nc.sync.dma_start(out=out_t[:, t * DT : (t + 1) * DT], in_=o_t[:])

    nc.finalize()
    return nc


_CACHE: dict = {}


def kernel(**inputs: np.ndarray) -> np.ndarray:
    import ml_dtypes

    nodes = np.asarray(inputs["nodes"], dtype=np.float32)
    edge_index = np.asarray(inputs["edge_index"])
    img = np.asarray(inputs["img"], dtype=np.float32)
    W1 = np.asarray(inputs["W1"], dtype=np.float32)
    b1 = np.asarray(inputs["b1"], dtype=np.float32)
    W2 = np.asarray(inputs["W2"], dtype=np.float32)
    b2 = np.asarray(inputs["b2"], dtype=np.float32)
    W3 = np.asarray(inputs["W3"], dtype=np.float32)
    b3 = np.asarray(inputs["b3"], dtype=np.float32)

    ECH, idxA, idxB, S = _preprocess(edge_index)
    S = S.astype(ml_dtypes.bfloat16)
    use_b1 = bool(np.any(b1))
    use_b2 = bool(np.any(b2))

    # per-core source rows: all 3584 own-slab rows first (recomputed locally,
    # hidden under the AllGather), then unique remote rows.
    uniq = []   # [NCORES] arrays of remote agg1f row ids, sorted
    idxL2 = []  # [NCORES][NDT, P, 2*ECH] int32 doubled half-row positions
    for k in range(NCORES):
        own_lo, own_hi = k * SLAB, (k + 1) * SLAB
        rem = np.unique(idxB[k])
        rem = rem[(rem < own_lo) | (rem >= own_hi)]
        uniq.append(rem.astype(np.int32))
        pos_map = np.zeros(NCORES * SLAB, dtype=np.int32)
        pos_map[own_lo:own_hi] = np.arange(SLAB)
        pos_map[rem] = SLAB + np.arange(len(rem))
        posk = pos_map[idxB[k]]  # [NDT, P, ECH]
        idxL2.append(np.concatenate([2 * posk, 2 * posk + 1], axis=2))
    NOWN = SLAB // P
    NUCH = NOWN + max(-(-len(u) // P) for u in uniq)

    key = (ECH, NUCH, use_b1, use_b2)
    if key not in _CACHE:
        _CACHE[key] = _build(ECH, NUCH, use_b1, use_b2)
    nc = _CACHE[key]

    w3img = (W3.astype(np.float32) @ img.astype(np.float32).T).astype(
        ml_dtypes.bfloat16
    )  # [H, B]
    outbias = img @ b3  # [B]

    nodes_r = nodes.astype(ml_dtypes.bfloat16)
    w1_r = W1.astype(ml_dtypes.bfloat16)
    w2_r = W2.astype(ml_dtypes.bfloat16)
    b1_r = b1.reshape(1, H).astype(ml_dtypes.bfloat16)
    b2_r = _round_fp32r(np.ascontiguousarray(b2.reshape(NFI2, P).T))
    ident = np.eye(P, dtype=ml_dtypes.bfloat16)

    in_maps = []
    for k in range(NCORES):
        g1 = nodes_r[idxA[k]].reshape(NDT, P, ECH * D)
        u_pad = np.zeros(NUCH * P, dtype=np.int32)
        u_pad[:SLAB] = np.arange(SLAB)  # own rows: local agg1p row ids
        u_pad[SLAB : SLAB + len(uniq[k])] = uniq[k]
        in_maps.append(
            {
                "G1": np.ascontiguousarray(g1),
                "W1": w1_r,
                "W2": w2_r,
                "W3img": w3img,
                "b1": b1_r,
                "b2": b2_r,
                "IDENT": ident,
                "idxU": u_pad.reshape(NUCH, P),
                "idxL2": np.ascontiguousarray(idxL2[k]),
                "idxQ": np.ascontiguousarray(idxB[k]),
                "S": np.ascontiguousarray(S[k]),
            }
        )

    res = run_bass_kernel_spmd(nc, in_maps, core_ids=list(range(NCORES)))

    full = np.concatenate([res.results[k]["out"] for k in range(NCORES)], axis=1)
    n_ids = np.arange(N_SKIP, N)
    cols = (n_ids // NODES_PER) * SLAB + (n_ids % NODES_PER)
    out = full[:, cols] + outbias[:, None]
    return out.astype(np.float32)


if __name__ == "__main__":
    rng = np.random.default_rng(0)
    ins = {
        "nodes": rng.standard_normal((N, D)).astype(np.float32),
        "edge_index": rng.integers(0, N, size=(2, E)).astype(np.int64),
        "img": rng.standard_normal((B, D)).astype(np.float32),
        "W1": (rng.standard_normal((D, H)) * 0.02).astype(np.float32),
        "b1": np.zeros(H, np.float32),
        "W2": (rng.standard_normal((H, H)) * 0.02).astype(np.float32),
        "b2": np.zeros(H, np.float32),
        "W3": (rng.standard_normal((H, D)) * 0.02).astype(np.float32),
        "b3": np.zeros(D, np.float32),
    }
    out = kernel(**ins)
    print("out", out.shape, out.dtype, np.abs(out).mean())


# revision 15
# speedup vs baseline: 1.5344x; 1.0089x over previous
"""3-layer GCN + img@pair_embed.T for Trainium2, distributed over 8 NeuronCores.

Strategy (destination-sharded graph parallelism, agg1-exchange variant):
  - Each core owns a contiguous slab of destination nodes (3567, padded 3584).
  - Edges (plus self-loops) are bucketed per 256-destination tile and padded to
    128-edge chunks. Host builds per chunk a dense [128 edges x 256 dests]
    one-hot norm matrix S, so segment-sum aggregation becomes TensorE matmuls.
  - Layer-1 source rows are PRE-GATHERED ON HOST (X is a static input), and the
    layer-1 aggregation computes agg1 = A@X directly in node-row orientation
    (lhsT = S chunk), so agg1 [SLAB, 512] is written without any transpose.
  - KEY: the cross-core exchange moves agg1 (512 wide) instead of h1 (2048
    wide): ONE AllGather of [SLAB,512] -> [8*SLAB,512] (29MB out) instead of
    117MB. Each core then recomputes h1 = relu(agg1 @ W1) for only the unique
    source rows its layer-2/3 edges touch (~13k rows): gather agg1 rows,
    PE-transpose them into contraction layout, GEMM against resident W1.
  - Layer 2 gathers 1024-wide half-rows of the local recomputed h1_u in two
    passes (PSUM has only 8 accumulation banks), GEMMs in dtile pairs
    (free dim 512), and folds img into layer 3: W3img = W3@img.T, Q = h2@W3img.
  - Layer 3 aggregates 64-wide Q after a small Q AllGather.
  - Everything exchanged/gathered travels bf16; W1 float32r; W2/W3img bf16;
    PSUM accumulation fp32.
"""

import numpy as np

from concourse import bacc, bass, mybir
from concourse import tile as tile_mod
from concourse.bass_utils import run_bass_kernel_spmd

# Problem shapes (hardcoded per spec nn_GraphModel_26268019982828)
N = 28535
E = 113000
D = 512
H = 2048
B = 64
N_SKIP = 115 + 245  # attrs + objs; pair nodes are N_SKIP..N-1

NCORES = 8
NODES_PER = -(-N // NCORES)  # 3567
P = 128
DT = 256  # destination tile width
NDT = 14  # dest tiles per core
SLAB = NDT * DT  # 3584 padded dests per core
NFI1 = D // P  # 4 feature chunks of layer-1 width
NFI2 = H // P  # 16 feature chunks of hidden width

f32 = mybir.dt.float32
f32r = mybir.dt.float32r
bf16 = mybir.dt.bfloat16
i32 = mybir.dt.int32


def _round_fp32r(x: np.ndarray) -> np.ndarray:
    """Round-to-nearest-even fp32 -> fp32r (11-bit mantissa), numpy."""
    u = np.ascontiguousarray(x, dtype=np.float32).view(np.uint32)
    r = u + (0x7FF + ((u >> 12) & np.uint32(1)))
    r &= np.uint32(0xFFFFF000)
    return r.view(np.float32)


def _preprocess(edge_index: np.ndarray):
    """Sort/bucket edges by destination; build gather indices + S blocks."""
    src = np.concatenate([edge_index[0], np.arange(N, dtype=np.int64)])
    dst = np.concatenate([edge_index[1], np.arange(N, dtype=np.int64)])
    deg = np.bincount(dst, minlength=N).astype(np.float32)  # includes loops
    dinv = (1.0 / np.sqrt(deg)).astype(np.float32)
    norm = (dinv[src] * dinv[dst]).astype(np.float32)

    core = (dst // NODES_PER).astype(np.int64)
    local = (dst - core * NODES_PER).astype(np.int64)
    t_idx = local // DT
    d_local = local % DT
    bucket = core * NDT + t_idx

    # secondary key: source's q-half (slab row >= 1792) so layer-3 chunks
    # whose sources all sit in the first half can run under the 2nd Q gather
    is_b = (src % NODES_PER) >= (SLAB // 2)
    order = np.argsort(bucket * 2 + is_b, kind="stable")
    src_s = src[order]
    bucket_s = bucket[order]
    dl_s = d_local[order]
    norm_s = norm[order]

    counts = np.bincount(bucket_s, minlength=NCORES * NDT)
    ECH = int(-(-counts.max() // P))

    idxA = np.zeros((NCORES, NDT, P, ECH), dtype=np.int32)
    idxB = np.zeros((NCORES, NDT, P, ECH), dtype=np.int32)
    S = np.zeros((NCORES, NDT, P, ECH * DT), dtype=np.float32)

    starts = np.zeros(NCORES * NDT + 1, dtype=np.int64)
    np.cumsum(counts, out=starts[1:])
    pos = np.arange(len(bucket_s)) - starts[bucket_s]
    c_idx = pos // P
    e_idx = pos % P

    ci = bucket_s // NDT
    ti = bucket_s % NDT
    srcB = (src_s // NODES_PER) * SLAB + (src_s % NODES_PER)
    idxA[ci, ti, e_idx, c_idx] = src_s.astype(np.int32)
    idxB[ci, ti, e_idx, c_idx] = srcB.astype(np.int32)
    S[ci, ti, e_idx, c_idx * DT + dl_s] = norm_s
    # per (core, dtile): edge count and A-half count (pads count as A only
    # when a whole trailing chunk is padding)
    cntA = np.bincount(bucket_s[~is_b[order]], minlength=NCORES * NDT).reshape(
        NCORES, NDT
    )
    cnt = counts.reshape(NCORES, NDT)
    gate = []  # gate[t][c] True -> chunk c reads the A-prefix on every core
    for t in range(NDT):
        row = []
        for c in range(ECH):
            ok = True
            for k in range(NCORES):
                if not (
                    (c + 1) * P <= cntA[k, t] or c * P >= cnt[k, t]
                ):
                    ok = False
                    break
            row.append(ok)
        gate.append(tuple(row))
    return ECH, idxA, idxB, S, tuple(gate)


def _build(ECH: int, NUCH: int, gateQ, use_b1: bool, use_b2: bool):
    nc = bacc.Bacc("TRN2", target_bir_lowering=False, num_devices=NCORES)
    NU = NUCH * P  # padded unique-source rows per core

    g1_t = nc.dram_tensor("G1", [NDT, P, ECH * D], bf16, kind="ExternalInput")
    w1_t = nc.dram_tensor("W1", [D, H], bf16, kind="ExternalInput")
    w2_t = nc.dram_tensor("W2", [H, H], bf16, kind="ExternalInput")
    w3i_t = nc.dram_tensor("W3img", [H, B], bf16, kind="ExternalInput")
    b1_t = nc.dram_tensor("b1", [1, H], bf16, kind="ExternalInput")
    b2_t = nc.dram_tensor("b2", [P, NFI2], f32r, kind="ExternalInput")
    idxU_t = nc.dram_tensor("idxU", [NUCH, P], i32, kind="ExternalInput")
    idxL2_t = nc.dram_tensor("idxL2", [NDT, P, 2 * ECH], i32, kind="ExternalInput")
    idxQ_t = nc.dram_tensor("idxQ", [NDT, P, ECH], i32, kind="ExternalInput")
    s_tab = nc.dram_tensor("S", [NDT, P, ECH * DT], bf16, kind="ExternalInput")
    ident_t = nc.dram_tensor("IDENT", [P, P], bf16, kind="ExternalInput")
    out_t = nc.dram_tensor("out", [B, SLAB], f32, kind="ExternalOutput")

    agg1p = nc.dram_tensor("agg1p", [SLAB, D], bf16)
    agg1f = nc.dram_tensor("agg1f", [SLAB * NCORES, D], bf16, addr_space="Shared")
    h1u = nc.dram_tensor("h1u", [NU, H], bf16)
    q_slab = nc.dram_tensor("q_slab", [SLAB, B], bf16)
    q_full = nc.dram_tensor("q_full", [SLAB * NCORES, B], bf16, addr_space="Shared")

    rg = [list(range(NCORES))]

    with tile_mod.TileContext(nc) as tc:
        with (
            tc.tile_pool(name="w", bufs=1) as wp,
            tc.tile_pool(name="gio", bufs=6) as gp,
            tc.tile_pool(name="g1io", bufs=3) as g1p,
            tc.tile_pool(name="stab", bufs=3) as sp,
            tc.tile_pool(name="agg", bufs=1) as ap,
            tc.tile_pool(name="small", bufs=3) as mp,
            tc.tile_pool(name="hout", bufs=2) as hp,
            tc.tile_pool(name="consts", bufs=1) as cp,
            tc.tile_pool(name="ps", bufs=8, space="PSUM") as ps,
        ):
            # --- resident weights ---
            w1sb = []
            for fi in range(NFI1):
                w = wp.tile([P, H], bf16, tag="w1", name="wsb", bufs=NFI1)
                nc.sync.dma_start(out=w[:], in_=w1_t[fi * P : (fi + 1) * P, :])
                w1sb.append(w)
            idt = cp.tile([P, P], bf16, tag="idt")
            nc.sync.dma_start(out=idt[:], in_=ident_t[:])
            if use_b1:
                b1sb = cp.tile([1, H], bf16, tag="b1")
                nc.sync.dma_start(out=b1sb[:], in_=b1_t[:])
                ones1 = cp.tile([1, P], bf16, tag="ones")
                nc.gpsimd.memset(ones1[:], 1.0)
            if use_b2:
                b2sb = cp.tile([P, NFI2], f32r, tag="b2")
                nc.sync.dma_start(out=b2sb[:], in_=b2_t[:])

            relu = mybir.ActivationFunctionType.Relu

            # ---------------- Layer 1: agg1 = A@X  (node-row orientation) ---
            for tp in range(NDT // 2):
                g1s, sts = [], []
                for t2 in range(2):
                    t = tp * 2 + t2
                    g1 = g1p.tile([P, ECH * D], bf16, tag="g1")
                    nc.sync.dma_start(out=g1[:], in_=g1_t[t])
                    s_t = sp.tile([P, ECH * DT], bf16, tag="s")
                    nc.sync.dma_start(out=s_t[:], in_=s_tab[t])
                    g1s.append(g1)
                    sts.append(s_t)
                pd = [ps.tile([P, D], f32, tag="ps", name="pd") for _ in range(4)]
                for t2 in range(2):
                    for c in range(ECH):
                        for dh in range(2):
                            nc.tensor.matmul(
                                out=pd[t2 * 2 + dh][:],
                                lhsT=sts[t2][:, c * DT + dh * P : c * DT + (dh + 1) * P],
                                rhs=g1s[t2][:, c * D : (c + 1) * D],
                                start=(c == 0),
                                stop=(c == ECH - 1),
                            )
                for t2 in range(2):
                    for dh in range(2):
                        a1 = mp.tile([P, D], bf16, tag="a1")
                        if dh % 2 == 0:
                            nc.vector.tensor_copy(out=a1[:], in_=pd[t2 * 2 + dh][:])
                        else:
                            nc.scalar.activation(
                                out=a1[:], in_=pd[t2 * 2 + dh][:],
                                func=mybir.ActivationFunctionType.Copy,
                            )
                        nc.sync.dma_start(
                            out=agg1p[
                                (tp * 2 + t2) * DT + dh * P : (tp * 2 + t2) * DT + (dh + 1) * P, :
                            ],
                            in_=a1[:],
                        )

            NOWN = SLAB // P  # 28 chunks of own-slab rows, recomputed locally

            def rec_front(u, src_tab):
                """Gather + transpose chunk u; returns aT tiles."""
                idxu = mp.tile([P, 1], i32, tag="idxu")
                nc.sync.dma_start(out=idxu[:], in_=idxU_t[u : u + 1].rearrange("o p -> p o"))
                gu = gp.tile([P, D], bf16, tag="gu")
                nc.gpsimd.indirect_dma_start(
                    out=gu[:],
                    out_offset=None,
                    in_=src_tab[:],
                    in_offset=bass.IndirectOffsetOnAxis(ap=idxu[:, 0:1], axis=0),
                )
                aT = []
                for fi in range(NFI1):
                    pt = ps.tile([P, P], bf16, tag="ps", name="pt")
                    nc.tensor.transpose(
                        out=pt[:], in_=gu[:, fi * P : (fi + 1) * P], identity=idt[:]
                    )
                    a = ap.tile([P, P], bf16, tag="aT", name="aTt", bufs=8)
                    if fi % 2 == 0:
                        nc.vector.tensor_copy(out=a[:], in_=pt[:])
                    else:
                        nc.scalar.activation(
                            out=a[:], in_=pt[:],
                            func=mybir.ActivationFunctionType.Copy,
                        )
                    aT.append(a)
                return aT

            def rec_back(u, aT):
                """GEMM + relu + writeback for chunk u."""
                h1u_t = hp.tile([P, H], bf16, tag="hout")
                pz = [ps.tile([P, D], f32, tag="ps", name="pz") for _ in range(NFI1)]
                if use_b1:
                    for fo in range(NFI1):
                        nc.tensor.matmul(
                            out=pz[fo][:],
                            lhsT=ones1[:1, :],
                            rhs=b1sb[:1, fo * D : (fo + 1) * D],
                            start=True,
                            stop=False,
                        )
                for fi in range(NFI1):
                    for fo in range(NFI1):
                        nc.tensor.matmul(
                            out=pz[fo][:],
                            lhsT=aT[fi][:],
                            rhs=w1sb[fi][:, fo * D : (fo + 1) * D],
                            start=(fi == 0 and not use_b1),
                            stop=(fi == NFI1 - 1),
                        )
                for fo in range(NFI1):
                    nc.scalar.activation(
                        out=h1u_t[:, fo * D : (fo + 1) * D], in_=pz[fo][:], func=relu
                    )
                nc.sync.dma_start(out=h1u[u * P : (u + 1) * P, :], in_=h1u_t[:])

            nc.gpsimd.collective_compute(
                "AllGather",
                mybir.AluOpType.bypass,
                replica_groups=rg,
                ins=[agg1p[:]],
                outs=[agg1f[:]],
            )

            # W2/W3 resident loads: issued after L1's input stream so they
            # ride the AllGather shadow instead of delaying the first dtile.
            w2sb = []
            for fi in range(NFI2):
                w = wp.tile([P, H], bf16, tag="w2", name="w2sb", bufs=NFI2)
                nc.sync.dma_start(out=w[:], in_=w2_t[fi * P : (fi + 1) * P, :])
                w2sb.append(w)
            w3sb = []
            for fo in range(NFI2):
                w = wp.tile([P, B], bf16, tag="w3", name="w3sb", bufs=NFI2)
                nc.sync.dma_start(out=w[:], in_=w3i_t[fo * P : (fo + 1) * P, :])
                w3sb.append(w)

            # software-pipelined: transpose(u) overlaps GEMM(u-1); own-slab
            # chunks (local agg1p) run inside the AllGather shadow.
            prev = None
            for u in range(NUCH):
                aT = rec_front(u, agg1p if u < NOWN else agg1f)
                if prev is not None:
                    rec_back(prev[0], prev[1])
                prev = (u, aT)
            rec_back(prev[0], prev[1])

            # ---------------- Layer 2 + Q (dtile pairs) --------------------
            NH = NFI2 // 2  # 8 feature chunks per half-row pass
            h1u_half = h1u[:].rearrange("n (h d) -> (n h) d", h=2)
            for tp in range(NDT // 2):
                aggT = [
                    ap.tile([P, 2 * DT], bf16, tag="aggT2", name="aggTt2", bufs=NFI2)
                    for _ in range(NFI2)
                ]
                for t2 in range(2):
                    t = tp * 2 + t2
                    idx_t = mp.tile([P, 2 * ECH], i32, tag="idx")
                    nc.sync.dma_start(out=idx_t[:], in_=idxL2_t[t])
                    s_t = sp.tile([P, ECH * DT], bf16, tag="s")
                    nc.sync.dma_start(out=s_t[:], in_=s_tab[t])

                    for hf in range(2):
                        pa = [ps.tile([P, DT], f32, tag="ps", name="pa2") for _ in range(NH)]
                        for c in range(ECH):
                            g = gp.tile([P, H // 2], bf16, tag="g")
                            nc.gpsimd.indirect_dma_start(
                                out=g[:],
                                out_offset=None,
                                in_=h1u_half,
                                in_offset=bass.IndirectOffsetOnAxis(
                                    ap=idx_t[:, hf * ECH + c : hf * ECH + c + 1], axis=0
                                ),
                            )
                            for j in range(NH):
                                nc.tensor.matmul(
                                    out=pa[j][:],
                                    lhsT=g[:, j * P : (j + 1) * P],
                                    rhs=s_t[:, c * DT : (c + 1) * DT],
                                    start=(c == 0),
                                    stop=(c == ECH - 1),
                                )
                        for j in range(NH):
                            fi = hf * NH + j
                            if j % 2 == 0:
                                nc.vector.tensor_copy(
                                    out=aggT[fi][:, t2 * DT : (t2 + 1) * DT], in_=pa[j][:]
                                )
                            else:
                                nc.scalar.activation(
                                    out=aggT[fi][:, t2 * DT : (t2 + 1) * DT],
                                    in_=pa[j][:],
                                    func=mybir.ActivationFunctionType.Copy,
                                )

                pq = [ps.tile([P, B], f32, tag="ps", name="pq") for _ in range(4)]
                for fo in range(NFI2):
                    pz = ps.tile([P, 2 * DT], f32, tag="ps", name="pz2")
                    for fi in range(NFI2):
                        nc.tensor.matmul(
                            out=pz[:],
                            lhsT=w2sb[fi][:, fo * P : (fo + 1) * P],
                            rhs=aggT[fi][:],
                            start=(fi == 0),
                            stop=(fi == NFI2 - 1),
                        )
                    h2t = ap.tile([P, 2 * DT], bf16, tag="h2", name="h2t", bufs=NFI2)
                    if use_b2:
                        nc.scalar.activation(
                            out=h2t[:], in_=pz[:], func=relu,
                            bias=b2sb[:, fo : fo + 1],
                        )
                    else:
                        nc.scalar.activation(out=h2t[:], in_=pz[:], func=relu)

                    for dh in range(4):
                        nc.tensor.matmul(
                            out=pq[dh][:],
                            lhsT=h2t[:, dh * P : (dh + 1) * P],
                            rhs=w3sb[fo][:],
                            start=(fo == 0),
                            stop=(fo == NFI2 - 1),
                        )
                for dh in range(4):
                    qn = mp.tile([P, B], bf16, tag="qn")
                    nc.vector.tensor_copy(out=qn[:], in_=pq[dh][:])
                    nc.sync.dma_start(
                        out=q_slab[tp * 2 * DT + dh * P : tp * 2 * DT + (dh + 1) * P, :],
                        in_=qn[:],
                    )
                if tp == 3:
                    # first q half complete (rows 0..1791): start its gather
                    nc.gpsimd.collective_compute(
                        "AllGather",
                        mybir.AluOpType.bypass,
                        replica_groups=rg,
                        ins=[q_slab[0 : SLAB // 2, :]],
                        outs=[q_full[0 : SLAB * NCORES // 2, :]],
                    )
            nc.gpsimd.collective_compute(
                "AllGather",
                mybir.AluOpType.bypass,
                replica_groups=rg,
                ins=[q_slab[SLAB // 2 :, :]],
                outs=[q_full[SLAB * NCORES // 2 :, :]],
            )


            # ---------------- Layer 3 (= output) ---------------------------
            for t in range(NDT):
                idx_t = mp.tile([P, ECH], i32, tag="idxq")
                nc.sync.dma_start(out=idx_t[:], in_=idxQ_t[t])
                s_t = sp.tile([P, ECH * DT], bf16, tag="s")
                nc.sync.dma_start(out=s_t[:], in_=s_tab[t])

                pa = ps.tile([B, DT], f32, tag="ps", name="pa3")
                for c in range(ECH):
                    g = gp.tile([P, B], bf16, tag="g3")
                    src_ap = (
                        q_full[0 : SLAB * NCORES // 2, :]
                        if gateQ[t][c]
                        else q_full[:]
                    )
                    nc.gpsimd.indirect_dma_start(
                        out=g[:],
                        out_offset=None,
                        in_=src_ap,
                        in_offset=bass.IndirectOffsetOnAxis(
                            ap=idx_t[:, c : c + 1], axis=0
                        ),
                    )
                    nc.tensor.matmul(
                        out=pa[:],
                        lhsT=g[:],
                        rhs=s_t[:, c * DT : (c + 1) * DT],
                        start=(c == 0),
                        stop=(c == ECH - 1),
                    )
                o_t = mp.tile([B, DT], f32, tag="ot")
                nc.vector.tensor_copy(out=o_t[:], in_=pa[:])
                nc.sync.dma_start(out=out_t[:, t * DT : (t + 1) * DT], in_=o_t[:])

    nc.finalize()
    return nc


_CACHE: dict = {}


def kernel(**inputs: np.ndarray) -> np.ndarray:
    import ml_dtypes

    nodes = np.asarray(inputs["nodes"], dtype=np.float32)
    edge_index = np.asarray(inputs["edge_index"])
    img = np.asarray(inputs["img"], dtype=np.float32)
    W1 = np.asarray(inputs["W1"], dtype=np.float32)
    b1 = np.asarray(inputs["b1"], dtype=np.float32)
    W2 = np.asarray(inputs["W2"], dtype=np.float32)
    b2 = np.asarray(inputs["b2"], dtype=np.float32)
    W3 = np.asarray(inputs["W3"], dtype=np.float32)
    b3 = np.asarray(inputs["b3"], dtype=np.float32)

    ECH, idxA, idxB, S, gateQ = _preprocess(edge_index)
    S = S.astype(ml_dtypes.bfloat16)
    use_b1 = bool(np.any(b1))
    use_b2 = bool(np.any(b2))

    # per-core source rows: all 3584 own-slab rows first (recomputed locally,
    # hidden under the AllGather), then unique remote rows.
    uniq = []   # [NCORES] arrays of remote agg1f row ids, sorted
    idxL2 = []  # [NCORES][NDT, P, 2*ECH] int32 doubled half-row positions
    for k in range(NCORES):
        own_lo, own_hi = k * SLAB, (k + 1) * SLAB
        rem = np.unique(idxB[k])
        rem = rem[(rem < own_lo) | (rem >= own_hi)]
        uniq.append(rem.astype(np.int32))
        pos_map = np.zeros(NCORES * SLAB, dtype=np.int32)
        pos_map[own_lo:own_hi] = np.arange(SLAB)
        pos_map[rem] = SLAB + np.arange(len(rem))
        posk = pos_map[idxB[k]]  # [NDT, P, ECH]
        idxL2.append(np.concatenate([2 * posk, 2 * posk + 1], axis=2))
    NOWN = SLAB // P
    NUCH = NOWN + max(-(-len(u) // P) for u in uniq)
    # q_full is half-major: rows [0:14336) = cores' slab rows 0..1791,
    # rows [14336:) = cores' slab rows 1792..3583
    q_core = idxB // SLAB
    q_r = idxB % SLAB
    HS = SLAB // 2
    idxQ = np.where(
        q_r < HS,
        q_core * HS + q_r,
        NCORES * HS + q_core * HS + (q_r - HS),
    ).astype(np.int32)

    key = (ECH, NUCH, gateQ, use_b1, use_b2)
    if key not in _CACHE:
        _CACHE[key] = _build(ECH, NUCH, gateQ, use_b1, use_b2)
    nc = _CACHE[key]

    w3img = (W3.astype(np.float32) @ img.astype(np.float32).T).astype(
        ml_dtypes.bfloat16
    )  # [H, B]
    outbias = img @ b3  # [B]

    nodes_r = nodes.astype(ml_dtypes.bfloat16)
    w1_r = W1.astype(ml_dtypes.bfloat16)
    w2_r = W2.astype(ml_dtypes.bfloat16)
    b1_r = b1.reshape(1, H).astype(ml_dtypes.bfloat16)
    b2_r = _round_fp32r(np.ascontiguousarray(b2.reshape(NFI2, P).T))
    ident = np.eye(P, dtype=ml_dtypes.bfloat16)

    in_maps = []
    for k in range(NCORES):
        g1 = nodes_r[idxA[k]].reshape(NDT, P, ECH * D)
        u_pad = np.zeros(NUCH * P, dtype=np.int32)
        u_pad[:SLAB] = np.arange(SLAB)  # own rows: local agg1p row ids
        u_pad[SLAB : SLAB + len(uniq[k])] = uniq[k]
        in_maps.append(
            {
                "G1": np.ascontiguousarray(g1),
                "W1": w1_r,
                "W2": w2_r,
                "W3img": w3img,
                "b1": b1_r,
                "b2": b2_r,
                "IDENT": ident,
                "idxU": u_pad.reshape(NUCH, P),
                "idxL2": np.ascontiguousarray(idxL2[k]),
                "idxQ": np.ascontiguousarray(idxQ[k]),
                "S": np.ascontiguousarray(S[k]),
            }
        )

    res = run_bass_kernel_spmd(nc, in_maps, core_ids=list(range(NCORES)))

    full = np.concatenate([res.results[k]["out"] for k in range(NCORES)], axis=1)
    n_ids = np.arange(N_SKIP, N)
    cols = (n_ids // NODES_PER) * SLAB + (n_ids % NODES_PER)
    out = full[:, cols] + outbias[:, None]
    return out.astype(np.float32)


if __name__ == "__main__":
    rng = np.random.default_rng(0)
    ins = {
        "nodes": rng.standard_normal((N, D)).astype(np.float32),
        "edge_index": rng.integers(0, N, size=(2, E)).astype(np.int64),
        "img": rng.standard_normal((B, D)).astype(np.float32),
        "W1": (rng.standard_normal((D, H)) * 0.02).astype(np.float32),
        "b1": np.zeros(H, np.float32),
        "W2": (rng.standard_normal((H, H)) * 0.02).astype(np.float32),
        "b2": np.zeros(H, np.float32),
        "W3": (rng.standard_normal((H, D)) * 0.02).astype(np.float32),
        "b3": np.zeros(D, np.float32),
    }
    out = kernel(**ins)
    print("out", out.shape, out.dtype, np.abs(out).mean())


# revision 16
# speedup vs baseline: 1.5754x; 1.0267x over previous
"""3-layer GCN + img@pair_embed.T for Trainium2, distributed over 8 NeuronCores.

Strategy (destination-sharded graph parallelism, agg1-exchange variant):
  - Each core owns a contiguous slab of destination nodes (3567, padded 3584).
  - Edges (plus self-loops) are bucketed per 256-destination tile and padded to
    128-edge chunks. Host builds per chunk a dense [128 edges x 256 dests]
    one-hot norm matrix S, so segment-sum aggregation becomes TensorE matmuls.
  - Layer-1 source rows are PRE-GATHERED ON HOST (X is a static input), and the
    layer-1 aggregation computes agg1 = A@X directly in node-row orientation
    (lhsT = S chunk), so agg1 [SLAB, 512] is written without any transpose.
  - KEY: the cross-core exchange moves agg1 (512 wide) instead of h1 (2048
    wide): ONE AllGather of [SLAB,512] -> [8*SLAB,512] (29MB out) instead of
    117MB. Each core then recomputes h1 = relu(agg1 @ W1) for only the unique
    source rows its layer-2/3 edges touch (~13k rows): gather agg1 rows,
    PE-transpose them into contraction layout, GEMM against resident W1.
  - Layer 2 gathers 1024-wide half-rows of the local recomputed h1_u in two
    passes (PSUM has only 8 accumulation banks), GEMMs in dtile pairs
    (free dim 512), and folds img into layer 3: W3img = W3@img.T, Q = h2@W3img.
  - Layer 3 aggregates 64-wide Q after a small Q AllGather.
  - Everything exchanged/gathered travels bf16; W1 float32r; W2/W3img bf16;
    PSUM accumulation fp32.
"""

import numpy as np

from concourse import bacc, bass, mybir
from concourse import tile as tile_mod
from concourse.bass_utils import run_bass_kernel_spmd

# Problem shapes (hardcoded per spec nn_GraphModel_26268019982828)
N = 28535
E = 113000
D = 512
H = 2048
B = 64
N_SKIP = 115 + 245  # attrs + objs; pair nodes are N_SKIP..N-1

NCORES = 8
NODES_PER = -(-N // NCORES)  # 3567
P = 128
DT = 256  # destination tile width
NDT = 14  # dest tiles per core
SLAB = NDT * DT  # 3584 padded dests per core
NFI1 = D // P  # 4 feature chunks of layer-1 width
NFI2 = H // P  # 16 feature chunks of hidden width

f32 = mybir.dt.float32
f32r = mybir.dt.float32r
bf16 = mybir.dt.bfloat16
i32 = mybir.dt.int32


def _round_fp32r(x: np.ndarray) -> np.ndarray:
    """Round-to-nearest-even fp32 -> fp32r (11-bit mantissa), numpy."""
    u = np.ascontiguousarray(x, dtype=np.float32).view(np.uint32)
    r = u + (0x7FF + ((u >> 12) & np.uint32(1)))
    r &= np.uint32(0xFFFFF000)
    return r.view(np.float32)


def _preprocess(edge_index: np.ndarray):
    """Sort/bucket edges by destination; build gather indices + S blocks."""
    src = np.concatenate([edge_index[0], np.arange(N, dtype=np.int64)])
    dst = np.concatenate([edge_index[1], np.arange(N, dtype=np.int64)])
    deg = np.bincount(dst, minlength=N).astype(np.float32)  # includes loops
    dinv = (1.0 / np.sqrt(deg)).astype(np.float32)
    norm = (dinv[src] * dinv[dst]).astype(np.float32)

    core = (dst // NODES_PER).astype(np.int64)
    local = (dst - core * NODES_PER).astype(np.int64)
    t_idx = local // DT
    d_local = local % DT
    bucket = core * NDT + t_idx

    # secondary key: source's q-half (slab row >= 1792) so layer-3 chunks
    # whose sources all sit in the first half can run under the 2nd Q gather
    is_b = (src % NODES_PER) >= (SLAB // 2)
    order = np.argsort(bucket * 2 + is_b, kind="stable")
    src_s = src[order]
    bucket_s = bucket[order]
    dl_s = d_local[order]
    norm_s = norm[order]

    counts = np.bincount(bucket_s, minlength=NCORES * NDT)
    ECH = int(-(-counts.max() // P))

    idxA = np.zeros((NCORES, NDT, P, ECH), dtype=np.int32)
    idxB = np.zeros((NCORES, NDT, P, ECH), dtype=np.int32)
    S = np.zeros((NCORES, NDT, P, ECH * DT), dtype=np.float32)

    starts = np.zeros(NCORES * NDT + 1, dtype=np.int64)
    np.cumsum(counts, out=starts[1:])
    pos = np.arange(len(bucket_s)) - starts[bucket_s]
    c_idx = pos // P
    e_idx = pos % P

    ci = bucket_s // NDT
    ti = bucket_s % NDT
    srcB = (src_s // NODES_PER) * SLAB + (src_s % NODES_PER)
    idxA[ci, ti, e_idx, c_idx] = src_s.astype(np.int32)
    idxB[ci, ti, e_idx, c_idx] = srcB.astype(np.int32)
    S[ci, ti, e_idx, c_idx * DT + dl_s] = norm_s
    # per (core, dtile): edge count and A-half count (pads count as A only
    # when a whole trailing chunk is padding)
    cntA = np.bincount(bucket_s[~is_b[order]], minlength=NCORES * NDT).reshape(
        NCORES, NDT
    )
    cnt = counts.reshape(NCORES, NDT)
    gate = []  # gate[t][c] True -> chunk c reads the A-prefix on every core
    for t in range(NDT):
        row = []
        for c in range(ECH):
            ok = True
            for k in range(NCORES):
                if not (
                    (c + 1) * P <= cntA[k, t] or c * P >= cnt[k, t]
                ):
                    ok = False
                    break
            row.append(ok)
        gate.append(tuple(row))
    return ECH, idxA, idxB, S, tuple(gate)


def _build(ECH: int, NUCH: int, gateQ, use_b1: bool, use_b2: bool):
    nc = bacc.Bacc("TRN2", target_bir_lowering=False, num_devices=NCORES)
    NU = NUCH * P  # padded unique-source rows per core

    g1_t = nc.dram_tensor("G1", [NDT, P, ECH * D], bf16, kind="ExternalInput")
    w1_t = nc.dram_tensor("W1", [D, H], bf16, kind="ExternalInput")
    w2_t = nc.dram_tensor("W2", [H, H], bf16, kind="ExternalInput")
    w3i_t = nc.dram_tensor("W3img", [H, B], bf16, kind="ExternalInput")
    b1_t = nc.dram_tensor("b1", [1, H], bf16, kind="ExternalInput")
    b2_t = nc.dram_tensor("b2", [P, NFI2], f32r, kind="ExternalInput")
    idxU_t = nc.dram_tensor("idxU", [NUCH, P], i32, kind="ExternalInput")
    idxL2_t = nc.dram_tensor("idxL2", [NDT, P, 2 * ECH], i32, kind="ExternalInput")
    idxQ2_t = nc.dram_tensor("idxQ2", [P, NDT * ECH], i32, kind="ExternalInput")
    s_tab = nc.dram_tensor("S", [NDT, P, ECH * DT], bf16, kind="ExternalInput")
    ident_t = nc.dram_tensor("IDENT", [P, P], bf16, kind="ExternalInput")
    out_t = nc.dram_tensor("out", [B, SLAB], f32, kind="ExternalOutput")

    agg1p = nc.dram_tensor("agg1p", [SLAB, D], bf16)
    agg1f = nc.dram_tensor("agg1f", [SLAB * NCORES, D], bf16, addr_space="Shared")
    h1u = nc.dram_tensor("h1u", [NU, H], bf16)
    q_slab = nc.dram_tensor("q_slab", [SLAB, B], bf16)
    q_full = nc.dram_tensor("q_full", [SLAB * NCORES, B], bf16, addr_space="Shared")

    rg = [list(range(NCORES))]

    with tile_mod.TileContext(nc) as tc:
        with (
            tc.tile_pool(name="w", bufs=1) as wp,
            tc.tile_pool(name="gio", bufs=6) as gp,
            tc.tile_pool(name="g1io", bufs=2) as g1p,
            tc.tile_pool(name="stab", bufs=3) as sp,
            tc.tile_pool(name="agg", bufs=1) as ap,
            tc.tile_pool(name="small", bufs=3) as mp,
            tc.tile_pool(name="hout", bufs=2) as hp,
            tc.tile_pool(name="consts", bufs=1) as cp,
            tc.tile_pool(name="ps", bufs=8, space="PSUM") as ps,
        ):
            # --- resident weights ---
            w1sb = []
            for fi in range(NFI1):
                w = wp.tile([P, H], bf16, tag="w1", name="wsb", bufs=NFI1)
                nc.sync.dma_start(out=w[:], in_=w1_t[fi * P : (fi + 1) * P, :])
                w1sb.append(w)
            idt = cp.tile([P, P], bf16, tag="idt")
            nc.sync.dma_start(out=idt[:], in_=ident_t[:])
            if use_b1:
                b1sb = cp.tile([1, H], bf16, tag="b1")
                nc.sync.dma_start(out=b1sb[:], in_=b1_t[:])
                ones1 = cp.tile([1, P], bf16, tag="ones")
                nc.gpsimd.memset(ones1[:], 1.0)
            if use_b2:
                b2sb = cp.tile([P, NFI2], f32r, tag="b2")
                nc.sync.dma_start(out=b2sb[:], in_=b2_t[:])

            relu = mybir.ActivationFunctionType.Relu

            # ---------------- Layer 1: agg1 = A@X  (node-row orientation) ---
            for tp in range(NDT // 2):
                g1s, sts = [], []
                for t2 in range(2):
                    t = tp * 2 + t2
                    g1 = g1p.tile([P, ECH * D], bf16, tag="g1")
                    nc.sync.dma_start(out=g1[:], in_=g1_t[t])
                    s_t = sp.tile([P, ECH * DT], bf16, tag="s")
                    nc.sync.dma_start(out=s_t[:], in_=s_tab[t])
                    g1s.append(g1)
                    sts.append(s_t)
                pd = [ps.tile([P, D], f32, tag="ps", name="pd") for _ in range(4)]
                for t2 in range(2):
                    for c in range(ECH):
                        for dh in range(2):
                            nc.tensor.matmul(
                                out=pd[t2 * 2 + dh][:],
                                lhsT=sts[t2][:, c * DT + dh * P : c * DT + (dh + 1) * P],
                                rhs=g1s[t2][:, c * D : (c + 1) * D],
                                start=(c == 0),
                                stop=(c == ECH - 1),
                            )
                for t2 in range(2):
                    for dh in range(2):
                        a1 = mp.tile([P, D], bf16, tag="a1")
                        if dh % 2 == 0:
                            nc.vector.tensor_copy(out=a1[:], in_=pd[t2 * 2 + dh][:])
                        else:
                            nc.scalar.activation(
                                out=a1[:], in_=pd[t2 * 2 + dh][:],
                                func=mybir.ActivationFunctionType.Copy,
                            )
                        nc.sync.dma_start(
                            out=agg1p[
                                (tp * 2 + t2) * DT + dh * P : (tp * 2 + t2) * DT + (dh + 1) * P, :
                            ],
                            in_=a1[:],
                        )

            NOWN = SLAB // P  # 28 chunks of own-slab rows, recomputed locally

            def rec_front(u, src_tab):
                """Gather + transpose chunk u; returns aT tiles."""
                idxu = mp.tile([P, 1], i32, tag="idxu")
                nc.sync.dma_start(out=idxu[:], in_=idxU_t[u : u + 1].rearrange("o p -> p o"))
                gu = gp.tile([P, D], bf16, tag="gu")
                nc.gpsimd.indirect_dma_start(
                    out=gu[:],
                    out_offset=None,
                    in_=src_tab[:],
                    in_offset=bass.IndirectOffsetOnAxis(ap=idxu[:, 0:1], axis=0),
                )
                aT = []
                for fi in range(NFI1):
                    pt = ps.tile([P, P], bf16, tag="ps", name="pt")
                    nc.tensor.transpose(
                        out=pt[:], in_=gu[:, fi * P : (fi + 1) * P], identity=idt[:]
                    )
                    a = ap.tile([P, P], bf16, tag="aT", name="aTt", bufs=8)
                    if fi % 2 == 0:
                        nc.vector.tensor_copy(out=a[:], in_=pt[:])
                    else:
                        nc.scalar.activation(
                            out=a[:], in_=pt[:],
                            func=mybir.ActivationFunctionType.Copy,
                        )
                    aT.append(a)
                return aT

            def rec_back(u, aT):
                """GEMM + relu + writeback for chunk u."""
                h1u_t = hp.tile([P, H], bf16, tag="hout")
                pz = [ps.tile([P, D], f32, tag="ps", name="pz") for _ in range(NFI1)]
                if use_b1:
                    for fo in range(NFI1):
                        nc.tensor.matmul(
                            out=pz[fo][:],
                            lhsT=ones1[:1, :],
                            rhs=b1sb[:1, fo * D : (fo + 1) * D],
                            start=True,
                            stop=False,
                        )
                for fi in range(NFI1):
                    for fo in range(NFI1):
                        nc.tensor.matmul(
                            out=pz[fo][:],
                            lhsT=aT[fi][:],
                            rhs=w1sb[fi][:, fo * D : (fo + 1) * D],
                            start=(fi == 0 and not use_b1),
                            stop=(fi == NFI1 - 1),
                        )
                for fo in range(NFI1):
                    nc.scalar.activation(
                        out=h1u_t[:, fo * D : (fo + 1) * D], in_=pz[fo][:], func=relu
                    )
                nc.sync.dma_start(out=h1u[u * P : (u + 1) * P, :], in_=h1u_t[:])

            nc.gpsimd.collective_compute(
                "AllGather",
                mybir.AluOpType.bypass,
                replica_groups=rg,
                ins=[agg1p[:]],
                outs=[agg1f[:]],
            )

            # W2/W3 resident loads: issued after L1's input stream so they
            # ride the AllGather shadow instead of delaying the first dtile.
            w2sb = []
            for fi in range(NFI2):
                w = wp.tile([P, H], bf16, tag="w2", name="w2sb", bufs=NFI2)
                nc.sync.dma_start(out=w[:], in_=w2_t[fi * P : (fi + 1) * P, :])
                w2sb.append(w)
            w3sb = []
            for fo in range(NFI2):
                w = wp.tile([P, B], bf16, tag="w3", name="w3sb", bufs=NFI2)
                nc.sync.dma_start(out=w[:], in_=w3i_t[fo * P : (fo + 1) * P, :])
                w3sb.append(w)

            # software-pipelined: transpose(u) overlaps GEMM(u-1); own-slab
            # chunks (local agg1p) run inside the AllGather shadow.
            prev = None
            for u in range(NUCH):
                aT = rec_front(u, agg1p if u < NOWN else agg1f)
                if prev is not None:
                    rec_back(prev[0], prev[1])
                prev = (u, aT)
            rec_back(prev[0], prev[1])

            # ---------------- Layer 2 + Q (dtile pairs) --------------------
            NH = NFI2 // 2  # 8 feature chunks per half-row pass
            h1u_half = h1u[:].rearrange("n (h d) -> (n h) d", h=2)
            for tp in range(NDT // 2):
                aggT = [
                    ap.tile([P, 2 * DT], bf16, tag="aggT2", name="aggTt2", bufs=NFI2)
                    for _ in range(NFI2)
                ]
                for t2 in range(2):
                    t = tp * 2 + t2
                    idx_t = mp.tile([P, 2 * ECH], i32, tag="idx")
                    nc.sync.dma_start(out=idx_t[:], in_=idxL2_t[t])
                    s_t = sp.tile([P, ECH * DT], bf16, tag="s")
                    nc.sync.dma_start(out=s_t[:], in_=s_tab[t])

                    for hf in range(2):
                        pa = [ps.tile([P, DT], f32, tag="ps", name="pa2") for _ in range(NH)]
                        for c in range(ECH):
                            g = gp.tile([P, H // 2], bf16, tag="g")
                            nc.gpsimd.indirect_dma_start(
                                out=g[:],
                                out_offset=None,
                                in_=h1u_half,
                                in_offset=bass.IndirectOffsetOnAxis(
                                    ap=idx_t[:, hf * ECH + c : hf * ECH + c + 1], axis=0
                                ),
                            )
                            for j in range(NH):
                                nc.tensor.matmul(
                                    out=pa[j][:],
                                    lhsT=g[:, j * P : (j + 1) * P],
                                    rhs=s_t[:, c * DT : (c + 1) * DT],
                                    start=(c == 0),
                                    stop=(c == ECH - 1),
                                )
                        for j in range(NH):
                            fi = hf * NH + j
                            if j % 2 == 0:
                                nc.vector.tensor_copy(
                                    out=aggT[fi][:, t2 * DT : (t2 + 1) * DT], in_=pa[j][:]
                                )
                            else:
                                nc.scalar.activation(
                                    out=aggT[fi][:, t2 * DT : (t2 + 1) * DT],
                                    in_=pa[j][:],
                                    func=mybir.ActivationFunctionType.Copy,
                                )

                pq = [ps.tile([P, B], f32, tag="ps", name="pq") for _ in range(4)]
                for fo in range(NFI2):
                    pz = ps.tile([P, 2 * DT], f32, tag="ps", name="pz2")
                    for fi in range(NFI2):
                        nc.tensor.matmul(
                            out=pz[:],
                            lhsT=w2sb[fi][:, fo * P : (fo + 1) * P],
                            rhs=aggT[fi][:],
                            start=(fi == 0),
                            stop=(fi == NFI2 - 1),
                        )
                    h2t = ap.tile([P, 2 * DT], bf16, tag="h2", name="h2t", bufs=NFI2)
                    if use_b2:
                        nc.scalar.activation(
                            out=h2t[:], in_=pz[:], func=relu,
                            bias=b2sb[:, fo : fo + 1],
                        )
                    else:
                        nc.scalar.activation(out=h2t[:], in_=pz[:], func=relu)

                    for dh in range(4):
                        nc.tensor.matmul(
                            out=pq[dh][:],
                            lhsT=h2t[:, dh * P : (dh + 1) * P],
                            rhs=w3sb[fo][:],
                            start=(fo == 0),
                            stop=(fo == NFI2 - 1),
                        )
                for dh in range(4):
                    qn = mp.tile([P, B], bf16, tag="qn")
                    nc.vector.tensor_copy(out=qn[:], in_=pq[dh][:])
                    nc.sync.dma_start(
                        out=q_slab[tp * 2 * DT + dh * P : tp * 2 * DT + (dh + 1) * P, :],
                        in_=qn[:],
                    )
                if tp == 3:
                    # first q half complete (rows 0..1791): start its gather
                    nc.gpsimd.collective_compute(
                        "AllGather",
                        mybir.AluOpType.bypass,
                        replica_groups=rg,
                        ins=[q_slab[0 : SLAB // 2, :]],
                        outs=[q_full[0 : SLAB * NCORES // 2, :]],
                    )
            nc.gpsimd.collective_compute(
                "AllGather",
                mybir.AluOpType.bypass,
                replica_groups=rg,
                ins=[q_slab[SLAB // 2 :, :]],
                outs=[q_full[SLAB * NCORES // 2 :, :]],
            )


            # ---------------- Layer 3 (= output) ---------------------------
            # SBUF fp32 accumulators per dtile; single-shot matmul per chunk +
            # DVE add. Chunk order is A-half-gated chunks (all dtiles) first,
            # so their gathers run under the second Q AllGather, then B chunks.
            idxall = cp.tile([P, NDT * ECH], i32, tag="idxall")
            nc.sync.dma_start(out=idxall[:], in_=idxQ2_t[:])
            acc = [
                ap.tile([B, DT], f32, tag="acc", name="acc3", bufs=NDT)
                for _ in range(NDT)
            ]
            first = [True] * NDT
            chunksA = [(t, c) for t in range(NDT) for c in range(ECH) if gateQ[t][c]]
            chunksB = [(t, c) for t in range(NDT) for c in range(ECH) if not gateQ[t][c]]
            for t, c in chunksA + chunksB:
                s3 = mp.tile([P, DT], bf16, tag="s3", bufs=8)
                nc.sync.dma_start(
                    out=s3[:], in_=s_tab[t][:, c * DT : (c + 1) * DT]
                )
                g = gp.tile([P, B], bf16, tag="g3", bufs=12)
                src_ap = (
                    q_full[0 : SLAB * NCORES // 2, :]
                    if gateQ[t][c]
                    else q_full[:]
                )
                nc.gpsimd.indirect_dma_start(
                    out=g[:],
                    out_offset=None,
                    in_=src_ap,
                    in_offset=bass.IndirectOffsetOnAxis(
                        ap=idxall[:, t * ECH + c : t * ECH + c + 1], axis=0
                    ),
                )
                pa = ps.tile([B, DT], f32, tag="ps", name="pa3")
                nc.tensor.matmul(
                    out=pa[:], lhsT=g[:], rhs=s3[:], start=True, stop=True
                )
                if first[t]:
                    nc.vector.tensor_copy(out=acc[t][:], in_=pa[:])
                    first[t] = False
                else:
                    nc.vector.tensor_tensor(
                        out=acc[t][:], in0=acc[t][:], in1=pa[:],
                        op=mybir.AluOpType.add,
                    )
            for t in range(NDT):
                nc.sync.dma_start(out=out_t[:, t * DT : (t + 1) * DT], in_=acc[t][:])

    nc.finalize()
    return nc


_CACHE: dict = {}


def kernel(**inputs: np.ndarray) -> np.ndarray:
    import ml_dtypes

    nodes = np.asarray(inputs["nodes"], dtype=np.float32)
    edge_index = np.asarray(inputs["edge_index"])
    img = np.asarray(inputs["img"], dtype=np.float32)
    W1 = np.asarray(inputs["W1"], dtype=np.float32)
    b1 = np.asarray(inputs["b1"], dtype=np.float32)
    W2 = np.asarray(inputs["W2"], dtype=np.float32)
    b2 = np.asarray(inputs["b2"], dtype=np.float32)
    W3 = np.asarray(inputs["W3"], dtype=np.float32)
    b3 = np.asarray(inputs["b3"], dtype=np.float32)

    ECH, idxA, idxB, S, gateQ = _preprocess(edge_index)
    S = S.astype(ml_dtypes.bfloat16)
    use_b1 = bool(np.any(b1))
    use_b2 = bool(np.any(b2))

    # per-core source rows: all 3584 own-slab rows first (recomputed locally,
    # hidden under the AllGather), then unique remote rows.
    uniq = []   # [NCORES] arrays of remote agg1f row ids, sorted
    idxL2 = []  # [NCORES][NDT, P, 2*ECH] int32 doubled half-row positions
    for k in range(NCORES):
        own_lo, own_hi = k * SLAB, (k + 1) * SLAB
        rem = np.unique(idxB[k])
        rem = rem[(rem < own_lo) | (rem >= own_hi)]
        uniq.append(rem.astype(np.int32))
        pos_map = np.zeros(NCORES * SLAB, dtype=np.int32)
        pos_map[own_lo:own_hi] = np.arange(SLAB)
        pos_map[rem] = SLAB + np.arange(len(rem))
        posk = pos_map[idxB[k]]  # [NDT, P, ECH]
        idxL2.append(np.concatenate([2 * posk, 2 * posk + 1], axis=2))
    NOWN = SLAB // P
    NUCH = NOWN + max(-(-len(u) // P) for u in uniq)
    # q_full is half-major: rows [0:14336) = cores' slab rows 0..1791,
    # rows [14336:) = cores' slab rows 1792..3583
    q_core = idxB // SLAB
    q_r = idxB % SLAB
    HS = SLAB // 2
    idxQ = np.where(
        q_r < HS,
        q_core * HS + q_r,
        NCORES * HS + q_core * HS + (q_r - HS),
    ).astype(np.int32)

    key = (ECH, NUCH, gateQ, use_b1, use_b2)
    if key not in _CACHE:
        _CACHE[key] = _build(ECH, NUCH, gateQ, use_b1, use_b2)
    nc = _CACHE[key]

    w3img = (W3.astype(np.float32) @ img.astype(np.float32).T).astype(
        ml_dtypes.bfloat16
    )  # [H, B]
    outbias = img @ b3  # [B]

    nodes_r = nodes.astype(ml_dtypes.bfloat16)
    w1_r = W1.astype(ml_dtypes.bfloat16)
    w2_r = W2.astype(ml_dtypes.bfloat16)
    b1_r = b1.reshape(1, H).astype(ml_dtypes.bfloat16)
    b2_r = _round_fp32r(np.ascontiguousarray(b2.reshape(NFI2, P).T))
    ident = np.eye(P, dtype=ml_dtypes.bfloat16)

    in_maps = []
    for k in range(NCORES):
        g1 = nodes_r[idxA[k]].reshape(NDT, P, ECH * D)
        u_pad = np.zeros(NUCH * P, dtype=np.int32)
        u_pad[:SLAB] = np.arange(SLAB)  # own rows: local agg1p row ids
        u_pad[SLAB : SLAB + len(uniq[k])] = uniq[k]
        in_maps.append(
            {
                "G1": np.ascontiguousarray(g1),
                "W1": w1_r,
                "W2": w2_r,
                "W3img": w3img,
                "b1": b1_r,
                "b2": b2_r,
                "IDENT": ident,
                "idxU": u_pad.reshape(NUCH, P),
                "idxL2": np.ascontiguousarray(idxL2[k]),
                "idxQ2": np.ascontiguousarray(
                    idxQ[k].transpose(1, 0, 2).reshape(P, NDT * ECH)
                ),
                "S": np.ascontiguousarray(S[k]),
            }
        )

    res = run_bass_kernel_spmd(nc, in_maps, core_ids=list(range(NCORES)))

    full = np.concatenate([res.results[k]["out"] for k in range(NCORES)], axis=1)
    n_ids = np.arange(N_SKIP, N)
    cols = (n_ids // NODES_PER) * SLAB + (n_ids % NODES_PER)
    out = full[:, cols] + outbias[:, None]
    return out.astype(np.float32)


if __name__ == "__main__":
    rng = np.random.default_rng(0)
    ins = {
        "nodes": rng.standard_normal((N, D)).astype(np.float32),
        "edge_index": rng.integers(0, N, size=(2, E)).astype(np.int64),
        "img": rng.standard_normal((B, D)).astype(np.float32),
        "W1": (rng.standard_normal((D, H)) * 0.02).astype(np.float32),
        "b1": np.zeros(H, np.float32),
        "W2": (rng.standard_normal((H, H)) * 0.02).astype(np.float32),
        "b2": np.zeros(H, np.float32),
        "W3": (rng.standard_normal((H, D)) * 0.02).astype(np.float32),
        "b3": np.zeros(D, np.float32),
    }
    out = kernel(**ins)
    print("out", out.shape, out.dtype, np.abs(out).mean())


# revision 17
# speedup vs baseline: 1.6175x; 1.0267x over previous
"""3-layer GCN + img@pair_embed.T for Trainium2, distributed over 8 NeuronCores.

Strategy (destination-sharded graph parallelism, agg1-exchange variant):
  - Each core owns a contiguous slab of destination nodes (3567, padded 3584).
  - Edges (plus self-loops) are bucketed per 256-destination tile and padded to
    128-edge chunks. Host builds per chunk a dense [128 edges x 256 dests]
    one-hot norm matrix S, so segment-sum aggregation becomes TensorE matmuls.
  - Layer-1 source rows are PRE-GATHERED ON HOST (X is a static input), and the
    layer-1 aggregation computes agg1 = A@X directly in node-row orientation
    (lhsT = S chunk), so agg1 [SLAB, 512] is written without any transpose.
  - KEY: the cross-core exchange moves agg1 (512 wide) instead of h1 (2048
    wide): ONE AllGather of [SLAB,512] -> [8*SLAB,512] (29MB out) instead of
    117MB. Each core then recomputes h1 = relu(agg1 @ W1) for only the unique
    source rows its layer-2/3 edges touch (~13k rows): gather agg1 rows,
    PE-transpose them into contraction layout, GEMM against resident W1.
  - Layer 2 gathers 1024-wide half-rows of the local recomputed h1_u in two
    passes (PSUM has only 8 accumulation banks), GEMMs in dtile pairs
    (free dim 512), and folds img into layer 3: W3img = W3@img.T, Q = h2@W3img.
  - Layer 3 aggregates 64-wide Q after a small Q AllGather.
  - Everything exchanged/gathered travels bf16; W1 float32r; W2/W3img bf16;
    PSUM accumulation fp32.
"""

import numpy as np

from concourse import bacc, bass, mybir
from concourse import tile as tile_mod
from concourse.bass_utils import run_bass_kernel_spmd

# Problem shapes (hardcoded per spec nn_GraphModel_26268019982828)
N = 28535
E = 113000
D = 512
H = 2048
B = 64
N_SKIP = 115 + 245  # attrs + objs; pair nodes are N_SKIP..N-1

NCORES = 8
NODES_PER = -(-N // NCORES)  # 3567
P = 128
DT = 256  # destination tile width
NDT = 14  # dest tiles per core
SLAB = NDT * DT  # 3584 padded dests per core
NFI1 = D // P  # 4 feature chunks of layer-1 width
NFI2 = H // P  # 16 feature chunks of hidden width

f32 = mybir.dt.float32
f32r = mybir.dt.float32r
bf16 = mybir.dt.bfloat16
i32 = mybir.dt.int32


def _round_fp32r(x: np.ndarray) -> np.ndarray:
    """Round-to-nearest-even fp32 -> fp32r (11-bit mantissa), numpy."""
    u = np.ascontiguousarray(x, dtype=np.float32).view(np.uint32)
    r = u + (0x7FF + ((u >> 12) & np.uint32(1)))
    r &= np.uint32(0xFFFFF000)
    return r.view(np.float32)


def _preprocess(edge_index: np.ndarray):
    """Sort/bucket edges by destination; build gather indices + S blocks."""
    src = np.concatenate([edge_index[0], np.arange(N, dtype=np.int64)])
    dst = np.concatenate([edge_index[1], np.arange(N, dtype=np.int64)])
    deg = np.bincount(dst, minlength=N).astype(np.float32)  # includes loops
    dinv = (1.0 / np.sqrt(deg)).astype(np.float32)
    norm = (dinv[src] * dinv[dst]).astype(np.float32)

    core = (dst // NODES_PER).astype(np.int64)
    local = (dst - core * NODES_PER).astype(np.int64)
    t_idx = local // DT
    d_local = local % DT
    bucket = core * NDT + t_idx

    # secondary key: source's q-quarter so layer-3 chunks gate on the
    # earliest quarter AllGather that covers all their sources
    quart = (src % NODES_PER) // (SLAB // 4)
    order = np.argsort(bucket * 4 + quart, kind="stable")
    src_s = src[order]
    bucket_s = bucket[order]
    dl_s = d_local[order]
    norm_s = norm[order]

    counts = np.bincount(bucket_s, minlength=NCORES * NDT)
    ECH = int(-(-counts.max() // P))

    idxA = np.zeros((NCORES, NDT, P, ECH), dtype=np.int32)
    idxB = np.zeros((NCORES, NDT, P, ECH), dtype=np.int32)
    S = np.zeros((NCORES, NDT, P, ECH * DT), dtype=np.float32)

    starts = np.zeros(NCORES * NDT + 1, dtype=np.int64)
    np.cumsum(counts, out=starts[1:])
    pos = np.arange(len(bucket_s)) - starts[bucket_s]
    c_idx = pos // P
    e_idx = pos % P

    ci = bucket_s // NDT
    ti = bucket_s % NDT
    srcB = (src_s // NODES_PER) * SLAB + (src_s % NODES_PER)
    idxA[ci, ti, e_idx, c_idx] = src_s.astype(np.int32)
    idxB[ci, ti, e_idx, c_idx] = srcB.astype(np.int32)
    S[ci, ti, e_idx, c_idx * DT + dl_s] = norm_s
    # gate[t][c] = max source-quarter of chunk c across cores (pads -> 0)
    quart_s = quart[order]
    cnt = counts.reshape(NCORES, NDT)
    qmax = np.zeros((NCORES, NDT, ECH), dtype=np.int64)
    for k in range(NCORES):
        for t in range(NDT):
            b = k * NDT + t
            qs = quart_s[starts[b] : starts[b] + cnt[k, t]]
            for c in range(ECH):
                last = min((c + 1) * P, cnt[k, t]) - 1
                qmax[k, t, c] = qs[last] if last >= c * P else 0
    gate = tuple(
        tuple(int(qmax[:, t, c].max()) for c in range(ECH)) for t in range(NDT)
    )
    return ECH, idxA, idxB, S, gate


def _build(ECH: int, NUCH: int, gateQ, use_b1: bool, use_b2: bool):
    nc = bacc.Bacc("TRN2", target_bir_lowering=False, num_devices=NCORES)
    NU = NUCH * P  # padded unique-source rows per core

    g1_t = nc.dram_tensor("G1", [NDT, P, ECH * D], bf16, kind="ExternalInput")
    w1_t = nc.dram_tensor("W1", [D, H], bf16, kind="ExternalInput")
    w2_t = nc.dram_tensor("W2", [H, H], bf16, kind="ExternalInput")
    w3i_t = nc.dram_tensor("W3img", [H, B], bf16, kind="ExternalInput")
    b1_t = nc.dram_tensor("b1", [1, H], bf16, kind="ExternalInput")
    b2_t = nc.dram_tensor("b2", [P, NFI2], f32r, kind="ExternalInput")
    idxU_t = nc.dram_tensor("idxU", [NUCH, P], i32, kind="ExternalInput")
    idxL2_t = nc.dram_tensor("idxL2", [NDT, P, 2 * ECH], i32, kind="ExternalInput")
    idxQ2_t = nc.dram_tensor("idxQ2", [P, NDT * ECH], i32, kind="ExternalInput")
    s_tab = nc.dram_tensor("S", [NDT, P, ECH * DT], bf16, kind="ExternalInput")
    ident_t = nc.dram_tensor("IDENT", [P, P], bf16, kind="ExternalInput")
    out_t = nc.dram_tensor("out", [B, SLAB], f32, kind="ExternalOutput")

    agg1p = nc.dram_tensor("agg1p", [SLAB, D], bf16)
    agg1f = nc.dram_tensor("agg1f", [SLAB * NCORES, D], bf16, addr_space="Shared")
    h1u = nc.dram_tensor("h1u", [NU, H], bf16)
    q_slab = nc.dram_tensor("q_slab", [SLAB, B], bf16)
    q_full = nc.dram_tensor("q_full", [SLAB * NCORES, B], bf16, addr_space="Shared")

    rg = [list(range(NCORES))]

    with tile_mod.TileContext(nc) as tc:
        with (
            tc.tile_pool(name="w", bufs=1) as wp,
            tc.tile_pool(name="gio", bufs=6) as gp,
            tc.tile_pool(name="g1io", bufs=2) as g1p,
            tc.tile_pool(name="stab", bufs=3) as sp,
            tc.tile_pool(name="agg", bufs=1) as ap,
            tc.tile_pool(name="small", bufs=3) as mp,
            tc.tile_pool(name="hout", bufs=2) as hp,
            tc.tile_pool(name="consts", bufs=1) as cp,
            tc.tile_pool(name="ps", bufs=8, space="PSUM") as ps,
        ):
            # --- resident weights ---
            w1sb = []
            for fi in range(NFI1):
                w = wp.tile([P, H], bf16, tag="w1", name="wsb", bufs=NFI1)
                nc.sync.dma_start(out=w[:], in_=w1_t[fi * P : (fi + 1) * P, :])
                w1sb.append(w)
            idt = cp.tile([P, P], bf16, tag="idt")
            nc.sync.dma_start(out=idt[:], in_=ident_t[:])
            if use_b1:
                b1sb = cp.tile([1, H], bf16, tag="b1")
                nc.sync.dma_start(out=b1sb[:], in_=b1_t[:])
                ones1 = cp.tile([1, P], bf16, tag="ones")
                nc.gpsimd.memset(ones1[:], 1.0)
            if use_b2:
                b2sb = cp.tile([P, NFI2], f32r, tag="b2")
                nc.sync.dma_start(out=b2sb[:], in_=b2_t[:])

            relu = mybir.ActivationFunctionType.Relu

            # ---------------- Layer 1: agg1 = A@X  (node-row orientation) ---
            for tp in range(NDT // 2):
                g1s, sts = [], []
                for t2 in range(2):
                    t = tp * 2 + t2
                    g1 = g1p.tile([P, ECH * D], bf16, tag="g1")
                    nc.sync.dma_start(out=g1[:], in_=g1_t[t])
                    s_t = sp.tile([P, ECH * DT], bf16, tag="s")
                    nc.sync.dma_start(out=s_t[:], in_=s_tab[t])
                    g1s.append(g1)
                    sts.append(s_t)
                pd = [ps.tile([P, D], f32, tag="ps", name="pd") for _ in range(4)]
                for t2 in range(2):
                    for c in range(ECH):
                        for dh in range(2):
                            nc.tensor.matmul(
                                out=pd[t2 * 2 + dh][:],
                                lhsT=sts[t2][:, c * DT + dh * P : c * DT + (dh + 1) * P],
                                rhs=g1s[t2][:, c * D : (c + 1) * D],
                                start=(c == 0),
                                stop=(c == ECH - 1),
                            )
                for t2 in range(2):
                    for dh in range(2):
                        a1 = mp.tile([P, D], bf16, tag="a1")
                        if dh % 2 == 0:
                            nc.vector.tensor_copy(out=a1[:], in_=pd[t2 * 2 + dh][:])
                        else:
                            nc.scalar.activation(
                                out=a1[:], in_=pd[t2 * 2 + dh][:],
                                func=mybir.ActivationFunctionType.Copy,
                            )
                        nc.sync.dma_start(
                            out=agg1p[
                                (tp * 2 + t2) * DT + dh * P : (tp * 2 + t2) * DT + (dh + 1) * P, :
                            ],
                            in_=a1[:],
                        )

            NOWN = SLAB // P  # 28 chunks of own-slab rows, recomputed locally

            def rec_front(u, src_tab):
                """Gather + transpose chunk u; returns aT tiles."""
                idxu = mp.tile([P, 1], i32, tag="idxu")
                nc.sync.dma_start(out=idxu[:], in_=idxU_t[u : u + 1].rearrange("o p -> p o"))
                gu = gp.tile([P, D], bf16, tag="gu")
                nc.gpsimd.indirect_dma_start(
                    out=gu[:],
                    out_offset=None,
                    in_=src_tab[:],
                    in_offset=bass.IndirectOffsetOnAxis(ap=idxu[:, 0:1], axis=0),
                )
                aT = []
                for fi in range(NFI1):
                    pt = ps.tile([P, P], bf16, tag="ps", name="pt")
                    nc.tensor.transpose(
                        out=pt[:], in_=gu[:, fi * P : (fi + 1) * P], identity=idt[:]
                    )
                    a = ap.tile([P, P], bf16, tag="aT", name="aTt", bufs=8)
                    if fi % 2 == 0:
                        nc.vector.tensor_copy(out=a[:], in_=pt[:])
                    else:
                        nc.scalar.activation(
                            out=a[:], in_=pt[:],
                            func=mybir.ActivationFunctionType.Copy,
                        )
                    aT.append(a)
                return aT

            def rec_back(u, aT):
                """GEMM + relu + writeback for chunk u."""
                h1u_t = hp.tile([P, H], bf16, tag="hout")
                pz = [ps.tile([P, D], f32, tag="ps", name="pz") for _ in range(NFI1)]
                if use_b1:
                    for fo in range(NFI1):
                        nc.tensor.matmul(
                            out=pz[fo][:],
                            lhsT=ones1[:1, :],
                            rhs=b1sb[:1, fo * D : (fo + 1) * D],
                            start=True,
                            stop=False,
                        )
                for fi in range(NFI1):
                    for fo in range(NFI1):
                        nc.tensor.matmul(
                            out=pz[fo][:],
                            lhsT=aT[fi][:],
                            rhs=w1sb[fi][:, fo * D : (fo + 1) * D],
                            start=(fi == 0 and not use_b1),
                            stop=(fi == NFI1 - 1),
                        )
                for fo in range(NFI1):
                    nc.scalar.activation(
                        out=h1u_t[:, fo * D : (fo + 1) * D], in_=pz[fo][:], func=relu
                    )
                nc.sync.dma_start(out=h1u[u * P : (u + 1) * P, :], in_=h1u_t[:])

            nc.gpsimd.collective_compute(
                "AllGather",
                mybir.AluOpType.bypass,
                replica_groups=rg,
                ins=[agg1p[:]],
                outs=[agg1f[:]],
            )

            # W2/W3 resident loads: issued after L1's input stream so they
            # ride the AllGather shadow instead of delaying the first dtile.
            w2sb = []
            for fi in range(NFI2):
                w = wp.tile([P, H], bf16, tag="w2", name="w2sb", bufs=NFI2)
                nc.sync.dma_start(out=w[:], in_=w2_t[fi * P : (fi + 1) * P, :])
                w2sb.append(w)
            w3sb = []
            for fo in range(NFI2):
                w = wp.tile([P, B], bf16, tag="w3", name="w3sb", bufs=NFI2)
                nc.sync.dma_start(out=w[:], in_=w3i_t[fo * P : (fo + 1) * P, :])
                w3sb.append(w)

            # software-pipelined: transpose(u) overlaps GEMM(u-1); own-slab
            # chunks (local agg1p) run inside the AllGather shadow.
            prev = None
            for u in range(NUCH):
                aT = rec_front(u, agg1p if u < NOWN else agg1f)
                if prev is not None:
                    rec_back(prev[0], prev[1])
                prev = (u, aT)
            rec_back(prev[0], prev[1])

            # ---------------- Layer 2 + Q (dtile pairs) --------------------
            NH = NFI2 // 2  # 8 feature chunks per half-row pass
            h1u_half = h1u[:].rearrange("n (h d) -> (n h) d", h=2)
            for tp in range(NDT // 2):
                aggT = [
                    ap.tile([P, 2 * DT], bf16, tag="aggT2", name="aggTt2", bufs=NFI2)
                    for _ in range(NFI2)
                ]
                for t2 in range(2):
                    t = tp * 2 + t2
                    idx_t = mp.tile([P, 2 * ECH], i32, tag="idx")
                    nc.sync.dma_start(out=idx_t[:], in_=idxL2_t[t])
                    s_t = sp.tile([P, ECH * DT], bf16, tag="s")
                    nc.sync.dma_start(out=s_t[:], in_=s_tab[t])

                    for hf in range(2):
                        pa = [ps.tile([P, DT], f32, tag="ps", name="pa2") for _ in range(NH)]
                        for c in range(ECH):
                            g = gp.tile([P, H // 2], bf16, tag="g")
                            nc.gpsimd.indirect_dma_start(
                                out=g[:],
                                out_offset=None,
                                in_=h1u_half,
                                in_offset=bass.IndirectOffsetOnAxis(
                                    ap=idx_t[:, hf * ECH + c : hf * ECH + c + 1], axis=0
                                ),
                            )
                            for j in range(NH):
                                nc.tensor.matmul(
                                    out=pa[j][:],
                                    lhsT=g[:, j * P : (j + 1) * P],
                                    rhs=s_t[:, c * DT : (c + 1) * DT],
                                    start=(c == 0),
                                    stop=(c == ECH - 1),
                                )
                        for j in range(NH):
                            fi = hf * NH + j
                            if j % 2 == 0:
                                nc.vector.tensor_copy(
                                    out=aggT[fi][:, t2 * DT : (t2 + 1) * DT], in_=pa[j][:]
                                )
                            else:
                                nc.scalar.activation(
                                    out=aggT[fi][:, t2 * DT : (t2 + 1) * DT],
                                    in_=pa[j][:],
                                    func=mybir.ActivationFunctionType.Copy,
                                )

                pq = [ps.tile([P, B], f32, tag="ps", name="pq") for _ in range(4)]
                for fo in range(NFI2):
                    pz = ps.tile([P, 2 * DT], f32, tag="ps", name="pz2")
                    for fi in range(NFI2):
                        nc.tensor.matmul(
                            out=pz[:],
                            lhsT=w2sb[fi][:, fo * P : (fo + 1) * P],
                            rhs=aggT[fi][:],
                            start=(fi == 0),
                            stop=(fi == NFI2 - 1),
                        )
                    h2t = ap.tile([P, 2 * DT], bf16, tag="h2", name="h2t", bufs=NFI2)
                    if use_b2:
                        nc.scalar.activation(
                            out=h2t[:], in_=pz[:], func=relu,
                            bias=b2sb[:, fo : fo + 1],
                        )
                    else:
                        nc.scalar.activation(out=h2t[:], in_=pz[:], func=relu)

                    for dh in range(4):
                        nc.tensor.matmul(
                            out=pq[dh][:],
                            lhsT=h2t[:, dh * P : (dh + 1) * P],
                            rhs=w3sb[fo][:],
                            start=(fo == 0),
                            stop=(fo == NFI2 - 1),
                        )
                for dh in range(4):
                    qn = mp.tile([P, B], bf16, tag="qn")
                    nc.vector.tensor_copy(out=qn[:], in_=pq[dh][:])
                    nc.sync.dma_start(
                        out=q_slab[tp * 2 * DT + dh * P : tp * 2 * DT + (dh + 1) * P, :],
                        in_=qn[:],
                    )
                QS = SLAB // 4
                for j in range(4):
                    # quarter j spans rows [j*896,(j+1)*896): ready once the
                    # pair covering its last row has been written
                    if tp == ((j + 1) * QS - 1) // (2 * DT):
                        nc.gpsimd.collective_compute(
                            "AllGather",
                            mybir.AluOpType.bypass,
                            replica_groups=rg,
                            ins=[q_slab[j * QS : (j + 1) * QS, :]],
                            outs=[
                                q_full[
                                    j * QS * NCORES : (j + 1) * QS * NCORES, :
                                ]
                            ],
                        )


            # ---------------- Layer 3 (= output) ---------------------------
            # SBUF fp32 accumulators per dtile; single-shot matmul per chunk +
            # DVE add. Chunk order is A-half-gated chunks (all dtiles) first,
            # so their gathers run under the second Q AllGather, then B chunks.
            idxall = cp.tile([P, NDT * ECH], i32, tag="idxall")
            nc.sync.dma_start(out=idxall[:], in_=idxQ2_t[:])
            acc = [
                ap.tile([B, DT], f32, tag="acc", name="acc3", bufs=NDT)
                for _ in range(NDT)
            ]
            first = [True] * NDT
            ordered = sorted(
                ((t, c) for t in range(NDT) for c in range(ECH)),
                key=lambda tc: gateQ[tc[0]][tc[1]],
            )
            for t, c in ordered:
                s3 = mp.tile([P, DT], bf16, tag="s3", bufs=8)
                nc.sync.dma_start(
                    out=s3[:], in_=s_tab[t][:, c * DT : (c + 1) * DT]
                )
                g = gp.tile([P, B], bf16, tag="g3", bufs=12)
                gq = gateQ[t][c]
                src_ap = q_full[0 : (gq + 1) * (SLAB // 4) * NCORES, :]
                nc.gpsimd.indirect_dma_start(
                    out=g[:],
                    out_offset=None,
                    in_=src_ap,
                    in_offset=bass.IndirectOffsetOnAxis(
                        ap=idxall[:, t * ECH + c : t * ECH + c + 1], axis=0
                    ),
                )
                pa = ps.tile([B, DT], f32, tag="ps", name="pa3")
                nc.tensor.matmul(
                    out=pa[:], lhsT=g[:], rhs=s3[:], start=True, stop=True
                )
                if first[t]:
                    nc.vector.tensor_copy(out=acc[t][:], in_=pa[:])
                    first[t] = False
                else:
                    nc.vector.tensor_tensor(
                        out=acc[t][:], in0=acc[t][:], in1=pa[:],
                        op=mybir.AluOpType.add,
                    )
            for t in range(NDT):
                nc.sync.dma_start(out=out_t[:, t * DT : (t + 1) * DT], in_=acc[t][:])

    nc.finalize()
    return nc


_CACHE: dict = {}


def kernel(**inputs: np.ndarray) -> np.ndarray:
    import ml_dtypes

    nodes = np.asarray(inputs["nodes"], dtype=np.float32)
    edge_index = np.asarray(inputs["edge_index"])
    img = np.asarray(inputs["img"], dtype=np.float32)
    W1 = np.asarray(inputs["W1"], dtype=np.float32)
    b1 = np.asarray(inputs["b1"], dtype=np.float32)
    W2 = np.asarray(inputs["W2"], dtype=np.float32)
    b2 = np.asarray(inputs["b2"], dtype=np.float32)
    W3 = np.asarray(inputs["W3"], dtype=np.float32)
    b3 = np.asarray(inputs["b3"], dtype=np.float32)

    ECH, idxA, idxB, S, gateQ = _preprocess(edge_index)
    S = S.astype(ml_dtypes.bfloat16)
    use_b1 = bool(np.any(b1))
    use_b2 = bool(np.any(b2))

    # per-core source rows: all 3584 own-slab rows first (recomputed locally,
    # hidden under the AllGather), then unique remote rows.
    uniq = []   # [NCORES] arrays of remote agg1f row ids, sorted
    idxL2 = []  # [NCORES][NDT, P, 2*ECH] int32 doubled half-row positions
    for k in range(NCORES):
        own_lo, own_hi = k * SLAB, (k + 1) * SLAB
        rem = np.unique(idxB[k])
        rem = rem[(rem < own_lo) | (rem >= own_hi)]
        uniq.append(rem.astype(np.int32))
        pos_map = np.zeros(NCORES * SLAB, dtype=np.int32)
        pos_map[own_lo:own_hi] = np.arange(SLAB)
        pos_map[rem] = SLAB + np.arange(len(rem))
        posk = pos_map[idxB[k]]  # [NDT, P, ECH]
        idxL2.append(np.concatenate([2 * posk, 2 * posk + 1], axis=2))
    NOWN = SLAB // P
    NUCH = NOWN + max(-(-len(u) // P) for u in uniq)
    # q_full is quarter-major: rows [j*7168:(j+1)*7168) hold quarter j
    # (cores' slab rows j*896..j*896+895, core-major within the quarter)
    q_core = idxB // SLAB
    q_r = idxB % SLAB
    QS = SLAB // 4
    idxQ = (
        (q_r // QS) * (NCORES * QS) + q_core * QS + (q_r % QS)
    ).astype(np.int32)

    key = (ECH, NUCH, gateQ, use_b1, use_b2)
    if key not in _CACHE:
        _CACHE[key] = _build(ECH, NUCH, gateQ, use_b1, use_b2)
    nc = _CACHE[key]

    w3img = (W3.astype(np.float32) @ img.astype(np.float32).T).astype(
        ml_dtypes.bfloat16
    )  # [H, B]
    outbias = img @ b3  # [B]

    nodes_r = nodes.astype(ml_dtypes.bfloat16)
    w1_r = W1.astype(ml_dtypes.bfloat16)
    w2_r = W2.astype(ml_dtypes.bfloat16)
    b1_r = b1.reshape(1, H).astype(ml_dtypes.bfloat16)
    b2_r = _round_fp32r(np.ascontiguousarray(b2.reshape(NFI2, P).T))
    ident = np.eye(P, dtype=ml_dtypes.bfloat16)

    in_maps = []
    for k in range(NCORES):
        g1 = nodes_r[idxA[k]].reshape(NDT, P, ECH * D)
        u_pad = np.zeros(NUCH * P, dtype=np.int32)
        u_pad[:SLAB] = np.arange(SLAB)  # own rows: local agg1p row ids
        u_pad[SLAB : SLAB + len(uniq[k])] = uniq[k]
        in_maps.append(
            {
                "G1": np.ascontiguousarray(g1),
                "W1": w1_r,
                "W2": w2_r,
                "W3img": w3img,
                "b1": b1_r,
                "b2": b2_r,
                "IDENT": ident,
                "idxU": u_pad.reshape(NUCH, P),
                "idxL2": np.ascontiguousarray(idxL2[k]),
                "idxQ2": np.ascontiguousarray(
                    idxQ[k].transpose(1, 0, 2).reshape(P, NDT * ECH)
                ),
                "S": np.ascontiguousarray(S[k]),
            }
        )

    res = run_bass_kernel_spmd(nc, in_maps, core_ids=list(range(NCORES)))

    full = np.concatenate([res.results[k]["out"] for k in range(NCORES)], axis=1)
    n_ids = np.arange(N_SKIP, N)
    cols = (n_ids // NODES_PER) * SLAB + (n_ids % NODES_PER)
    out = full[:, cols] + outbias[:, None]
    return out.astype(np.float32)


if __name__ == "__main__":
    rng = np.random.default_rng(0)
    ins = {
        "nodes": rng.standard_normal((N, D)).astype(np.float32),
        "edge_index": rng.integers(0, N, size=(2, E)).astype(np.int64),
        "img": rng.standard_normal((B, D)).astype(np.float32),
        "W1": (rng.standard_normal((D, H)) * 0.02).astype(np.float32),
        "b1": np.zeros(H, np.float32),
        "W2": (rng.standard_normal((H, H)) * 0.02).astype(np.float32),
        "b2": np.zeros(H, np.float32),
        "W3": (rng.standard_normal((H, D)) * 0.02).astype(np.float32),
        "b3": np.zeros(D, np.float32),
    }
    out = kernel(**ins)
    print("out", out.shape, out.dtype, np.abs(out).mean())


# revision 18
# speedup vs baseline: 1.6313x; 1.0085x over previous
"""3-layer GCN + img@pair_embed.T for Trainium2, distributed over 8 NeuronCores.

Strategy (destination-sharded graph parallelism, agg1-exchange variant):
  - Each core owns a contiguous slab of destination nodes (3567, padded 3584).
  - Edges (plus self-loops) are bucketed per 256-destination tile and padded to
    128-edge chunks. Host builds per chunk a dense [128 edges x 256 dests]
    one-hot norm matrix S, so segment-sum aggregation becomes TensorE matmuls.
  - Layer-1 source rows are PRE-GATHERED ON HOST (X is a static input), and the
    layer-1 aggregation computes agg1 = A@X directly in node-row orientation
    (lhsT = S chunk), so agg1 [SLAB, 512] is written without any transpose.
  - KEY: the cross-core exchange moves agg1 (512 wide) instead of h1 (2048
    wide): ONE AllGather of [SLAB,512] -> [8*SLAB,512] (29MB out) instead of
    117MB. Each core then recomputes h1 = relu(agg1 @ W1) for only the unique
    source rows its layer-2/3 edges touch (~13k rows): gather agg1 rows,
    PE-transpose them into contraction layout, GEMM against resident W1.
  - Layer 2 gathers 1024-wide half-rows of the local recomputed h1_u in two
    passes (PSUM has only 8 accumulation banks), GEMMs in dtile pairs
    (free dim 512), and folds img into layer 3: W3img = W3@img.T, Q = h2@W3img.
  - Layer 3 aggregates 64-wide Q after a small Q AllGather.
  - Everything exchanged/gathered travels bf16; W1 float32r; W2/W3img bf16;
    PSUM accumulation fp32.
"""

import numpy as np

from concourse import bacc, bass, mybir
from concourse import tile as tile_mod
from concourse.bass_utils import run_bass_kernel_spmd

# Problem shapes (hardcoded per spec nn_GraphModel_26268019982828)
N = 28535
E = 113000
D = 512
H = 2048
B = 64
N_SKIP = 115 + 245  # attrs + objs; pair nodes are N_SKIP..N-1

NCORES = 8
NODES_PER = -(-N // NCORES)  # 3567
P = 128
DT = 256  # destination tile width
NDT = 14  # dest tiles per core
SLAB = NDT * DT  # 3584 padded dests per core
NFI1 = D // P  # 4 feature chunks of layer-1 width
NFI2 = H // P  # 16 feature chunks of hidden width

f32 = mybir.dt.float32
f32r = mybir.dt.float32r
bf16 = mybir.dt.bfloat16
i32 = mybir.dt.int32


def _round_fp32r(x: np.ndarray) -> np.ndarray:
    """Round-to-nearest-even fp32 -> fp32r (11-bit mantissa), numpy."""
    u = np.ascontiguousarray(x, dtype=np.float32).view(np.uint32)
    r = u + (0x7FF + ((u >> 12) & np.uint32(1)))
    r &= np.uint32(0xFFFFF000)
    return r.view(np.float32)


def _preprocess(edge_index: np.ndarray):
    """Sort/bucket edges by destination; build gather indices + S blocks."""
    src = np.concatenate([edge_index[0], np.arange(N, dtype=np.int64)])
    dst = np.concatenate([edge_index[1], np.arange(N, dtype=np.int64)])
    deg = np.bincount(dst, minlength=N).astype(np.float32)  # includes loops
    dinv = (1.0 / np.sqrt(deg)).astype(np.float32)
    norm = (dinv[src] * dinv[dst]).astype(np.float32)

    core = (dst // NODES_PER).astype(np.int64)
    local = (dst - core * NODES_PER).astype(np.int64)
    t_idx = local // DT
    d_local = local % DT
    bucket = core * NDT + t_idx

    # secondary key: source's q-quarter so layer-3 chunks gate on the
    # earliest quarter AllGather that covers all their sources
    quart = (src % NODES_PER) // (SLAB // 4)
    order = np.argsort(bucket * 4 + quart, kind="stable")
    src_s = src[order]
    bucket_s = bucket[order]
    dl_s = d_local[order]
    norm_s = norm[order]

    counts = np.bincount(bucket_s, minlength=NCORES * NDT)
    ECH = int(-(-counts.max() // P))

    idxA = np.zeros((NCORES, NDT, P, ECH), dtype=np.int32)
    idxB = np.zeros((NCORES, NDT, P, ECH), dtype=np.int32)
    S = np.zeros((NCORES, NDT, P, ECH * DT), dtype=np.float32)

    starts = np.zeros(NCORES * NDT + 1, dtype=np.int64)
    np.cumsum(counts, out=starts[1:])
    pos = np.arange(len(bucket_s)) - starts[bucket_s]
    c_idx = pos // P
    e_idx = pos % P

    ci = bucket_s // NDT
    ti = bucket_s % NDT
    srcB = (src_s // NODES_PER) * SLAB + (src_s % NODES_PER)
    idxA[ci, ti, e_idx, c_idx] = src_s.astype(np.int32)
    idxB[ci, ti, e_idx, c_idx] = srcB.astype(np.int32)
    S[ci, ti, e_idx, c_idx * DT + dl_s] = norm_s
    # gate[t][c] = max source-quarter of chunk c across cores (pads -> 0)
    quart_s = quart[order]
    cnt = counts.reshape(NCORES, NDT)
    qmax = np.zeros((NCORES, NDT, ECH), dtype=np.int64)
    for k in range(NCORES):
        for t in range(NDT):
            b = k * NDT + t
            qs = quart_s[starts[b] : starts[b] + cnt[k, t]]
            for c in range(ECH):
                last = min((c + 1) * P, cnt[k, t]) - 1
                qmax[k, t, c] = qs[last] if last >= c * P else 0
    gate = tuple(
        tuple(int(qmax[:, t, c].max()) for c in range(ECH)) for t in range(NDT)
    )
    return ECH, idxA, idxB, S, gate


def _build(ECH: int, NUCH: int, gateQ, use_b1: bool, use_b2: bool):
    nc = bacc.Bacc("TRN2", target_bir_lowering=False, num_devices=NCORES)
    NU = NUCH * P  # padded unique-source rows per core

    g1_t = nc.dram_tensor("G1", [NDT, P, ECH * D], bf16, kind="ExternalInput")
    w1_t = nc.dram_tensor("W1", [D, H], bf16, kind="ExternalInput")
    w2_t = nc.dram_tensor("W2", [H, H], bf16, kind="ExternalInput")
    w3i_t = nc.dram_tensor("W3img", [H, B], bf16, kind="ExternalInput")
    b1_t = nc.dram_tensor("b1", [1, H], bf16, kind="ExternalInput")
    b2_t = nc.dram_tensor("b2", [P, NFI2], f32r, kind="ExternalInput")
    idxU_t = nc.dram_tensor("idxUT", [P, NUCH], i32, kind="ExternalInput")
    idxL2_t = nc.dram_tensor("idxL2", [NDT, P, 2 * ECH], i32, kind="ExternalInput")
    idxQ2_t = nc.dram_tensor("idxQ2", [P, NDT * ECH], i32, kind="ExternalInput")
    s_tab = nc.dram_tensor("S", [NDT, P, ECH * DT], bf16, kind="ExternalInput")
    ident_t = nc.dram_tensor("IDENT", [P, P], bf16, kind="ExternalInput")
    out_t = nc.dram_tensor("out", [B, SLAB], f32, kind="ExternalOutput")

    agg1p = nc.dram_tensor("agg1p", [SLAB, D], bf16)
    agg1f = nc.dram_tensor("agg1f", [SLAB * NCORES, D], bf16, addr_space="Shared")
    h1u = nc.dram_tensor("h1u", [NU, H], bf16)
    q_slab = nc.dram_tensor("q_slab", [SLAB, B], bf16)
    q_full = nc.dram_tensor("q_full", [SLAB * NCORES, B], bf16, addr_space="Shared")

    rg = [list(range(NCORES))]

    with tile_mod.TileContext(nc) as tc:
        with (
            tc.tile_pool(name="w", bufs=1) as wp,
            tc.tile_pool(name="gio", bufs=6) as gp,
            tc.tile_pool(name="g1io", bufs=2) as g1p,
            tc.tile_pool(name="stab", bufs=3) as sp,
            tc.tile_pool(name="agg", bufs=1) as ap,
            tc.tile_pool(name="small", bufs=3) as mp,
            tc.tile_pool(name="hout", bufs=2) as hp,
            tc.tile_pool(name="consts", bufs=1) as cp,
            tc.tile_pool(name="ps", bufs=8, space="PSUM") as ps,
        ):
            # --- resident weights ---
            w1sb = []
            for fi in range(NFI1):
                w = wp.tile([P, H], bf16, tag="w1", name="wsb", bufs=NFI1)
                nc.sync.dma_start(out=w[:], in_=w1_t[fi * P : (fi + 1) * P, :])
                w1sb.append(w)
            idxUall = cp.tile([P, NUCH], i32, tag="idxUall")
            nc.sync.dma_start(out=idxUall[:], in_=idxU_t[:])
            idt = cp.tile([P, P], bf16, tag="idt")
            nc.sync.dma_start(out=idt[:], in_=ident_t[:])
            if use_b1:
                b1sb = cp.tile([1, H], bf16, tag="b1")
                nc.sync.dma_start(out=b1sb[:], in_=b1_t[:])
                ones1 = cp.tile([1, P], bf16, tag="ones")
                nc.gpsimd.memset(ones1[:], 1.0)
            if use_b2:
                b2sb = cp.tile([P, NFI2], f32r, tag="b2")
                nc.sync.dma_start(out=b2sb[:], in_=b2_t[:])

            relu = mybir.ActivationFunctionType.Relu

            # ---------------- Layer 1: agg1 = A@X  (node-row orientation) ---
            for tp in range(NDT // 2):
                g1s, sts = [], []
                for t2 in range(2):
                    t = tp * 2 + t2
                    g1 = g1p.tile([P, ECH * D], bf16, tag="g1")
                    nc.sync.dma_start(out=g1[:], in_=g1_t[t])
                    s_t = sp.tile([P, ECH * DT], bf16, tag="s")
                    nc.sync.dma_start(out=s_t[:], in_=s_tab[t])
                    g1s.append(g1)
                    sts.append(s_t)
                pd = [ps.tile([P, D], f32, tag="ps", name="pd") for _ in range(4)]
                for t2 in range(2):
                    for c in range(ECH):
                        for dh in range(2):
                            nc.tensor.matmul(
                                out=pd[t2 * 2 + dh][:],
                                lhsT=sts[t2][:, c * DT + dh * P : c * DT + (dh + 1) * P],
                                rhs=g1s[t2][:, c * D : (c + 1) * D],
                                start=(c == 0),
                                stop=(c == ECH - 1),
                            )
                for t2 in range(2):
                    for dh in range(2):
                        a1 = mp.tile([P, D], bf16, tag="a1")
                        if dh % 2 == 0:
                            nc.vector.tensor_copy(out=a1[:], in_=pd[t2 * 2 + dh][:])
                        else:
                            nc.scalar.activation(
                                out=a1[:], in_=pd[t2 * 2 + dh][:],
                                func=mybir.ActivationFunctionType.Copy,
                            )
                        nc.sync.dma_start(
                            out=agg1p[
                                (tp * 2 + t2) * DT + dh * P : (tp * 2 + t2) * DT + (dh + 1) * P, :
                            ],
                            in_=a1[:],
                        )

            NOWN = SLAB // P  # 28 chunks of own-slab rows, recomputed locally

            def rec_front(u, src_tab):
                """Gather + transpose chunk u; returns aT tiles."""
                gu = gp.tile([P, D], bf16, tag="gu")
                nc.gpsimd.indirect_dma_start(
                    out=gu[:],
                    out_offset=None,
                    in_=src_tab[:],
                    in_offset=bass.IndirectOffsetOnAxis(ap=idxUall[:, u : u + 1], axis=0),
                )
                aT = []
                for fi in range(NFI1):
                    pt = ps.tile([P, P], bf16, tag="ps", name="pt")
                    nc.tensor.transpose(
                        out=pt[:], in_=gu[:, fi * P : (fi + 1) * P], identity=idt[:]
                    )
                    a = ap.tile([P, P], bf16, tag="aT", name="aTt", bufs=8)
                    if fi % 2 == 0:
                        nc.vector.tensor_copy(out=a[:], in_=pt[:])
                    else:
                        nc.scalar.activation(
                            out=a[:], in_=pt[:],
                            func=mybir.ActivationFunctionType.Copy,
                        )
                    aT.append(a)
                return aT

            def rec_back(u, aT):
                """GEMM + relu + writeback for chunk u."""
                h1u_t = hp.tile([P, H], bf16, tag="hout")
                pz = [ps.tile([P, D], f32, tag="ps", name="pz") for _ in range(NFI1)]
                if use_b1:
                    for fo in range(NFI1):
                        nc.tensor.matmul(
                            out=pz[fo][:],
                            lhsT=ones1[:1, :],
                            rhs=b1sb[:1, fo * D : (fo + 1) * D],
                            start=True,
                            stop=False,
                        )
                for fi in range(NFI1):
                    for fo in range(NFI1):
                        nc.tensor.matmul(
                            out=pz[fo][:],
                            lhsT=aT[fi][:],
                            rhs=w1sb[fi][:, fo * D : (fo + 1) * D],
                            start=(fi == 0 and not use_b1),
                            stop=(fi == NFI1 - 1),
                        )
                for fo in range(NFI1):
                    nc.scalar.activation(
                        out=h1u_t[:, fo * D : (fo + 1) * D], in_=pz[fo][:], func=relu
                    )
                nc.sync.dma_start(out=h1u[u * P : (u + 1) * P, :], in_=h1u_t[:])

            nc.gpsimd.collective_compute(
                "AllGather",
                mybir.AluOpType.bypass,
                replica_groups=rg,
                ins=[agg1p[:]],
                outs=[agg1f[:]],
            )

            # W2/W3 resident loads: issued after L1's input stream so they
            # ride the AllGather shadow instead of delaying the first dtile.
            w2sb = []
            for fi in range(NFI2):
                w = wp.tile([P, H], bf16, tag="w2", name="w2sb", bufs=NFI2)
                nc.sync.dma_start(out=w[:], in_=w2_t[fi * P : (fi + 1) * P, :])
                w2sb.append(w)
            w3sb = []
            for fo in range(NFI2):
                w = wp.tile([P, B], bf16, tag="w3", name="w3sb", bufs=NFI2)
                nc.sync.dma_start(out=w[:], in_=w3i_t[fo * P : (fo + 1) * P, :])
                w3sb.append(w)

            # software-pipelined: transpose(u) overlaps GEMM(u-1); own-slab
            # chunks (local agg1p) run inside the AllGather shadow.
            prev = None
            for u in range(NUCH):
                aT = rec_front(u, agg1p if u < NOWN else agg1f)
                if prev is not None:
                    rec_back(prev[0], prev[1])
                prev = (u, aT)
            rec_back(prev[0], prev[1])

            # ---------------- Layer 2 + Q (dtile pairs) --------------------
            NH = NFI2 // 2  # 8 feature chunks per half-row pass
            h1u_half = h1u[:].rearrange("n (h d) -> (n h) d", h=2)
            QS = SLAB // 4

            def q_quarter_ag(j):
                nc.gpsimd.collective_compute(
                    "AllGather",
                    mybir.AluOpType.bypass,
                    replica_groups=rg,
                    ins=[q_slab[j * QS : (j + 1) * QS, :]],
                    outs=[q_full[j * QS * NCORES : (j + 1) * QS * NCORES, :]],
                )

            for tp in range(NDT // 2):
                # issue quarter AllGathers one pair after their rows complete,
                # so the collective's SEQ wait never stalls the gpsimd queue
                for j in range(4):
                    if tp == ((j + 1) * QS - 1) // (2 * DT) + 1:
                        q_quarter_ag(j)
                aggT = [
                    ap.tile([P, 2 * DT], bf16, tag="aggT2", name="aggTt2", bufs=NFI2)
                    for _ in range(NFI2)
                ]
                for t2 in range(2):
                    t = tp * 2 + t2
                    idx_t = mp.tile([P, 2 * ECH], i32, tag="idx")
                    nc.sync.dma_start(out=idx_t[:], in_=idxL2_t[t])
                    s_t = sp.tile([P, ECH * DT], bf16, tag="s")
                    nc.sync.dma_start(out=s_t[:], in_=s_tab[t])

                    for hf in range(2):
                        pa = [ps.tile([P, DT], f32, tag="ps", name="pa2") for _ in range(NH)]
                        for c in range(ECH):
                            g = gp.tile([P, H // 2], bf16, tag="g")
                            nc.gpsimd.indirect_dma_start(
                                out=g[:],
                                out_offset=None,
                                in_=h1u_half,
                                in_offset=bass.IndirectOffsetOnAxis(
                                    ap=idx_t[:, hf * ECH + c : hf * ECH + c + 1], axis=0
                                ),
                            )
                            for j in range(NH):
                                nc.tensor.matmul(
                                    out=pa[j][:],
                                    lhsT=g[:, j * P : (j + 1) * P],
                                    rhs=s_t[:, c * DT : (c + 1) * DT],
                                    start=(c == 0),
                                    stop=(c == ECH - 1),
                                )
                        for j in range(NH):
                            fi = hf * NH + j
                            if j % 2 == 0:
                                nc.vector.tensor_copy(
                                    out=aggT[fi][:, t2 * DT : (t2 + 1) * DT], in_=pa[j][:]
                                )
                            else:
                                nc.scalar.activation(
                                    out=aggT[fi][:, t2 * DT : (t2 + 1) * DT],
                                    in_=pa[j][:],
                                    func=mybir.ActivationFunctionType.Copy,
                                )

                pq = [ps.tile([P, B], f32, tag="ps", name="pq") for _ in range(4)]
                for fo in range(NFI2):
                    pz = ps.tile([P, 2 * DT], f32, tag="ps", name="pz2")
                    for fi in range(NFI2):
                        nc.tensor.matmul(
                            out=pz[:],
                            lhsT=w2sb[fi][:, fo * P : (fo + 1) * P],
                            rhs=aggT[fi][:],
                            start=(fi == 0),
                            stop=(fi == NFI2 - 1),
                        )
                    h2t = ap.tile([P, 2 * DT], bf16, tag="h2", name="h2t", bufs=NFI2)
                    if use_b2:
                        nc.scalar.activation(
                            out=h2t[:], in_=pz[:], func=relu,
                            bias=b2sb[:, fo : fo + 1],
                        )
                    else:
                        nc.scalar.activation(out=h2t[:], in_=pz[:], func=relu)

                    for dh in range(4):
                        nc.tensor.matmul(
                            out=pq[dh][:],
                            lhsT=h2t[:, dh * P : (dh + 1) * P],
                            rhs=w3sb[fo][:],
                            start=(fo == 0),
                            stop=(fo == NFI2 - 1),
                        )
                for dh in range(4):
                    qn = mp.tile([P, B], bf16, tag="qn")
                    nc.vector.tensor_copy(out=qn[:], in_=pq[dh][:])
                    nc.sync.dma_start(
                        out=q_slab[tp * 2 * DT + dh * P : tp * 2 * DT + (dh + 1) * P, :],
                        in_=qn[:],
                    )



            q_quarter_ag(3)

            # ---------------- Layer 3 (= output) ---------------------------
            # SBUF fp32 accumulators per dtile; single-shot matmul per chunk +
            # DVE add. Chunk order is A-half-gated chunks (all dtiles) first,
            # so their gathers run under the second Q AllGather, then B chunks.
            idxall = cp.tile([P, NDT * ECH], i32, tag="idxall")
            nc.sync.dma_start(out=idxall[:], in_=idxQ2_t[:])
            acc = [
                ap.tile([B, DT], f32, tag="acc", name="acc3", bufs=NDT)
                for _ in range(NDT)
            ]
            first = [True] * NDT
            ordered = sorted(
                ((t, c) for t in range(NDT) for c in range(ECH)),
                key=lambda tc: gateQ[tc[0]][tc[1]],
            )
            for t, c in ordered:
                s3 = mp.tile([P, DT], bf16, tag="s3", bufs=8)
                nc.sync.dma_start(
                    out=s3[:], in_=s_tab[t][:, c * DT : (c + 1) * DT]
                )
                g = gp.tile([P, B], bf16, tag="g3", bufs=12)
                gq = gateQ[t][c]
                src_ap = q_full[0 : (gq + 1) * (SLAB // 4) * NCORES, :]
                nc.gpsimd.indirect_dma_start(
                    out=g[:],
                    out_offset=None,
                    in_=src_ap,
                    in_offset=bass.IndirectOffsetOnAxis(
                        ap=idxall[:, t * ECH + c : t * ECH + c + 1], axis=0
                    ),
                )
                pa = ps.tile([B, DT], f32, tag="ps", name="pa3")
                nc.tensor.matmul(
                    out=pa[:], lhsT=g[:], rhs=s3[:], start=True, stop=True
                )
                if first[t]:
                    nc.vector.tensor_copy(out=acc[t][:], in_=pa[:])
                    first[t] = False
                else:
                    nc.vector.tensor_tensor(
                        out=acc[t][:], in0=acc[t][:], in1=pa[:],
                        op=mybir.AluOpType.add,
                    )
            for t in range(NDT):
                nc.sync.dma_start(out=out_t[:, t * DT : (t + 1) * DT], in_=acc[t][:])

    nc.finalize()
    return nc


_CACHE: dict = {}


def kernel(**inputs: np.ndarray) -> np.ndarray:
    import ml_dtypes

    nodes = np.asarray(inputs["nodes"], dtype=np.float32)
    edge_index = np.asarray(inputs["edge_index"])
    img = np.asarray(inputs["img"], dtype=np.float32)
    W1 = np.asarray(inputs["W1"], dtype=np.float32)
    b1 = np.asarray(inputs["b1"], dtype=np.float32)
    W2 = np.asarray(inputs["W2"], dtype=np.float32)
    b2 = np.asarray(inputs["b2"], dtype=np.float32)
    W3 = np.asarray(inputs["W3"], dtype=np.float32)
    b3 = np.asarray(inputs["b3"], dtype=np.float32)

    ECH, idxA, idxB, S, gateQ = _preprocess(edge_index)
    S = S.astype(ml_dtypes.bfloat16)
    use_b1 = bool(np.any(b1))
    use_b2 = bool(np.any(b2))

    # per-core source rows: all 3584 own-slab rows first (recomputed locally,
    # hidden under the AllGather), then unique remote rows.
    uniq = []   # [NCORES] arrays of remote agg1f row ids, sorted
    idxL2 = []  # [NCORES][NDT, P, 2*ECH] int32 doubled half-row positions
    for k in range(NCORES):
        own_lo, own_hi = k * SLAB, (k + 1) * SLAB
        rem = np.unique(idxB[k])
        rem = rem[(rem < own_lo) | (rem >= own_hi)]
        uniq.append(rem.astype(np.int32))
        pos_map = np.zeros(NCORES * SLAB, dtype=np.int32)
        pos_map[own_lo:own_hi] = np.arange(SLAB)
        pos_map[rem] = SLAB + np.arange(len(rem))
        posk = pos_map[idxB[k]]  # [NDT, P, ECH]
        idxL2.append(np.concatenate([2 * posk, 2 * posk + 1], axis=2))
    NOWN = SLAB // P
    NUCH = NOWN + max(-(-len(u) // P) for u in uniq)
    # q_full is quarter-major: rows [j*7168:(j+1)*7168) hold quarter j
    # (cores' slab rows j*896..j*896+895, core-major within the quarter)
    q_core = idxB // SLAB
    q_r = idxB % SLAB
    QS = SLAB // 4
    idxQ = (
        (q_r // QS) * (NCORES * QS) + q_core * QS + (q_r % QS)
    ).astype(np.int32)

    key = (ECH, NUCH, gateQ, use_b1, use_b2)
    if key not in _CACHE:
        _CACHE[key] = _build(ECH, NUCH, gateQ, use_b1, use_b2)
    nc = _CACHE[key]

    w3img = (W3.astype(np.float32) @ img.astype(np.float32).T).astype(
        ml_dtypes.bfloat16
    )  # [H, B]
    outbias = img @ b3  # [B]

    nodes_r = nodes.astype(ml_dtypes.bfloat16)
    w1_r = W1.astype(ml_dtypes.bfloat16)
    w2_r = W2.astype(ml_dtypes.bfloat16)
    b1_r = b1.reshape(1, H).astype(ml_dtypes.bfloat16)
    b2_r = _round_fp32r(np.ascontiguousarray(b2.reshape(NFI2, P).T))
    ident = np.eye(P, dtype=ml_dtypes.bfloat16)

    in_maps = []
    for k in range(NCORES):
        g1 = nodes_r[idxA[k]].reshape(NDT, P, ECH * D)
        u_pad = np.zeros(NUCH * P, dtype=np.int32)
        u_pad[:SLAB] = np.arange(SLAB)  # own rows: local agg1p row ids
        u_pad[SLAB : SLAB + len(uniq[k])] = uniq[k]
        in_maps.append(
            {
                "G1": np.ascontiguousarray(g1),
                "W1": w1_r,
                "W2": w2_r,
                "W3img": w3img,
                "b1": b1_r,
                "b2": b2_r,
                "IDENT": ident,
                "idxUT": np.ascontiguousarray(u_pad.reshape(NUCH, P).T),
                "idxL2": np.ascontiguousarray(idxL2[k]),
                "idxQ2": np.ascontiguousarray(
                    idxQ[k].transpose(1, 0, 2).reshape(P, NDT * ECH)
                ),
                "S": np.ascontiguousarray(S[k]),
            }
        )

    res = run_bass_kernel_spmd(nc, in_maps, core_ids=list(range(NCORES)))

    full = np.concatenate([res.results[k]["out"] for k in range(NCORES)], axis=1)
    n_ids = np.arange(N_SKIP, N)
    cols = (n_ids // NODES_PER) * SLAB + (n_ids % NODES_PER)
    out = full[:, cols] + outbias[:, None]
    return out.astype(np.float32)


if __name__ == "__main__":
    rng = np.random.default_rng(0)
    ins = {
        "nodes": rng.standard_normal((N, D)).astype(np.float32),
        "edge_index": rng.integers(0, N, size=(2, E)).astype(np.int64),
        "img": rng.standard_normal((B, D)).astype(np.float32),
        "W1": (rng.standard_normal((D, H)) * 0.02).astype(np.float32),
        "b1": np.zeros(H, np.float32),
        "W2": (rng.standard_normal((H, H)) * 0.02).astype(np.float32),
        "b2": np.zeros(H, np.float32),
        "W3": (rng.standard_normal((H, D)) * 0.02).astype(np.float32),
        "b3": np.zeros(D, np.float32),
    }
    out = kernel(**ins)
    print("out", out.shape, out.dtype, np.abs(out).mean())


# revision 19
# speedup vs baseline: 1.6355x; 1.0026x over previous
"""3-layer GCN + img@pair_embed.T for Trainium2, distributed over 8 NeuronCores.

Strategy (destination-sharded graph parallelism, agg1-exchange variant):
  - Each core owns a contiguous slab of destination nodes (3567, padded 3584).
  - Edges (plus self-loops) are bucketed per 256-destination tile and padded to
    128-edge chunks. Host builds per chunk a dense [128 edges x 256 dests]
    one-hot norm matrix S, so segment-sum aggregation becomes TensorE matmuls.
  - Layer-1 source rows are PRE-GATHERED ON HOST (X is a static input), and the
    layer-1 aggregation computes agg1 = A@X directly in node-row orientation
    (lhsT = S chunk), so agg1 [SLAB, 512] is written without any transpose.
  - KEY: the cross-core exchange moves agg1 (512 wide) instead of h1 (2048
    wide): ONE AllGather of [SLAB,512] -> [8*SLAB,512] (29MB out) instead of
    117MB. Each core then recomputes h1 = relu(agg1 @ W1) for only the unique
    source rows its layer-2/3 edges touch (~13k rows): gather agg1 rows,
    PE-transpose them into contraction layout, GEMM against resident W1.
  - Layer 2 gathers 1024-wide half-rows of the local recomputed h1_u in two
    passes (PSUM has only 8 accumulation banks), GEMMs in dtile pairs
    (free dim 512), and folds img into layer 3: W3img = W3@img.T, Q = h2@W3img.
  - Layer 3 aggregates 64-wide Q after a small Q AllGather.
  - Everything exchanged/gathered travels bf16; W1 float32r; W2/W3img bf16;
    PSUM accumulation fp32.
"""

import numpy as np

from concourse import bacc, bass, mybir
from concourse import tile as tile_mod
from concourse.bass_utils import run_bass_kernel_spmd

# Problem shapes (hardcoded per spec nn_GraphModel_26268019982828)
N = 28535
E = 113000
D = 512
H = 2048
B = 64
N_SKIP = 115 + 245  # attrs + objs; pair nodes are N_SKIP..N-1

NCORES = 8
NODES_PER = -(-N // NCORES)  # 3567
P = 128
DT = 256  # destination tile width
NDT = 14  # dest tiles per core
SLAB = NDT * DT  # 3584 padded dests per core
NFI1 = D // P  # 4 feature chunks of layer-1 width
NFI2 = H // P  # 16 feature chunks of hidden width

f32 = mybir.dt.float32
f32r = mybir.dt.float32r
bf16 = mybir.dt.bfloat16
i32 = mybir.dt.int32


def _round_fp32r(x: np.ndarray) -> np.ndarray:
    """Round-to-nearest-even fp32 -> fp32r (11-bit mantissa), numpy."""
    u = np.ascontiguousarray(x, dtype=np.float32).view(np.uint32)
    r = u + (0x7FF + ((u >> 12) & np.uint32(1)))
    r &= np.uint32(0xFFFFF000)
    return r.view(np.float32)


def _preprocess(edge_index: np.ndarray):
    """Sort/bucket edges by destination; build gather indices + S blocks."""
    src = np.concatenate([edge_index[0], np.arange(N, dtype=np.int64)])
    dst = np.concatenate([edge_index[1], np.arange(N, dtype=np.int64)])
    deg = np.bincount(dst, minlength=N).astype(np.float32)  # includes loops
    dinv = (1.0 / np.sqrt(deg)).astype(np.float32)
    norm = (dinv[src] * dinv[dst]).astype(np.float32)

    core = (dst // NODES_PER).astype(np.int64)
    local = (dst - core * NODES_PER).astype(np.int64)
    t_idx = local // DT
    d_local = local % DT
    bucket = core * NDT + t_idx

    # secondary key: source's q-quarter so layer-3 chunks gate on the
    # earliest quarter AllGather that covers all their sources
    quart = (src % NODES_PER) // (SLAB // 4)
    order = np.argsort(bucket * 4 + quart, kind="stable")
    src_s = src[order]
    bucket_s = bucket[order]
    dl_s = d_local[order]
    norm_s = norm[order]

    counts = np.bincount(bucket_s, minlength=NCORES * NDT)
    ECH = int(-(-counts.max() // P))

    idxA = np.zeros((NCORES, NDT, P, ECH), dtype=np.int32)
    idxB = np.zeros((NCORES, NDT, P, ECH), dtype=np.int32)
    S = np.zeros((NCORES, NDT, P, ECH * DT), dtype=np.float32)

    starts = np.zeros(NCORES * NDT + 1, dtype=np.int64)
    np.cumsum(counts, out=starts[1:])
    pos = np.arange(len(bucket_s)) - starts[bucket_s]
    c_idx = pos // P
    e_idx = pos % P

    ci = bucket_s // NDT
    ti = bucket_s % NDT
    srcB = (src_s // NODES_PER) * SLAB + (src_s % NODES_PER)
    idxA[ci, ti, e_idx, c_idx] = src_s.astype(np.int32)
    idxB[ci, ti, e_idx, c_idx] = srcB.astype(np.int32)
    S[ci, ti, e_idx, c_idx * DT + dl_s] = norm_s
    # gate[t][c] = max source-quarter of chunk c across cores (pads -> 0)
    quart_s = quart[order]
    cnt = counts.reshape(NCORES, NDT)
    qmax = np.zeros((NCORES, NDT, ECH), dtype=np.int64)
    for k in range(NCORES):
        for t in range(NDT):
            b = k * NDT + t
            qs = quart_s[starts[b] : starts[b] + cnt[k, t]]
            for c in range(ECH):
                last = min((c + 1) * P, cnt[k, t]) - 1
                qmax[k, t, c] = qs[last] if last >= c * P else 0
    gate = tuple(
        tuple(int(qmax[:, t, c].max()) for c in range(ECH)) for t in range(NDT)
    )
    return ECH, idxA, idxB, S, gate


def _build(ECH: int, NUCH: int, gateQ, use_b1: bool, use_b2: bool):
    nc = bacc.Bacc("TRN2", target_bir_lowering=False, num_devices=NCORES)
    NU = NUCH * P  # padded unique-source rows per core

    g1_t = nc.dram_tensor("G1", [NDT, P, ECH * D], bf16, kind="ExternalInput")
    w1_t = nc.dram_tensor("W1", [D, H], bf16, kind="ExternalInput")
    w2_t = nc.dram_tensor("W2", [H, H], bf16, kind="ExternalInput")
    w3i_t = nc.dram_tensor("W3img", [H, B], bf16, kind="ExternalInput")
    b1_t = nc.dram_tensor("b1", [1, H], bf16, kind="ExternalInput")
    b2_t = nc.dram_tensor("b2", [P, NFI2], f32r, kind="ExternalInput")
    idxU_t = nc.dram_tensor("idxUT", [P, NUCH], i32, kind="ExternalInput")
    idxL2_t = nc.dram_tensor("idxL2", [NDT, P, 2 * ECH], i32, kind="ExternalInput")
    idxQ2_t = nc.dram_tensor("idxQ2", [P, NDT * ECH], i32, kind="ExternalInput")
    s_tab = nc.dram_tensor("S", [NDT, P, ECH * DT], bf16, kind="ExternalInput")
    ident_t = nc.dram_tensor("IDENT", [P, P], bf16, kind="ExternalInput")
    out_t = nc.dram_tensor("out", [B, SLAB], f32, kind="ExternalOutput")

    agg1p = nc.dram_tensor("agg1p", [SLAB, D], bf16)
    agg1f = nc.dram_tensor("agg1f", [SLAB * NCORES, D], bf16, addr_space="Shared")
    h1u = nc.dram_tensor("h1u", [NU, H], bf16)
    q_slab = nc.dram_tensor("q_slab", [SLAB, B], bf16)
    q_full = nc.dram_tensor("q_full", [SLAB * NCORES, B], bf16, addr_space="Shared")

    rg = [list(range(NCORES))]

    with tile_mod.TileContext(nc) as tc:
        with (
            tc.tile_pool(name="w", bufs=1) as wp,
            tc.tile_pool(name="gio", bufs=6) as gp,
            tc.tile_pool(name="g1io", bufs=2) as g1p,
            tc.tile_pool(name="stab", bufs=3) as sp,
            tc.tile_pool(name="agg", bufs=1) as ap,
            tc.tile_pool(name="small", bufs=3) as mp,
            tc.tile_pool(name="hout", bufs=2) as hp,
            tc.tile_pool(name="consts", bufs=1) as cp,
            tc.tile_pool(name="ps", bufs=8, space="PSUM") as ps,
        ):
            # --- resident weights ---
            w1sb = []
            for fi in range(NFI1):
                w = wp.tile([P, H], bf16, tag="w1", name="wsb", bufs=NFI1)
                nc.sync.dma_start(out=w[:], in_=w1_t[fi * P : (fi + 1) * P, :])
                w1sb.append(w)
            idxUall = cp.tile([P, NUCH], i32, tag="idxUall")
            nc.sync.dma_start(out=idxUall[:], in_=idxU_t[:])
            idt = cp.tile([P, P], bf16, tag="idt")
            nc.sync.dma_start(out=idt[:], in_=ident_t[:])
            if use_b1:
                b1sb = cp.tile([1, H], bf16, tag="b1")
                nc.sync.dma_start(out=b1sb[:], in_=b1_t[:])
                ones1 = cp.tile([1, P], bf16, tag="ones")
                nc.gpsimd.memset(ones1[:], 1.0)
            if use_b2:
                b2sb = cp.tile([P, NFI2], f32r, tag="b2")
                nc.sync.dma_start(out=b2sb[:], in_=b2_t[:])

            relu = mybir.ActivationFunctionType.Relu

            # ---------------- Layer 1: agg1 = A@X  (node-row orientation) ---
            for tp in range(NDT // 2):
                g1s, sts = [], []
                for t2 in range(2):
                    t = tp * 2 + t2
                    g1 = g1p.tile([P, ECH * D], bf16, tag="g1")
                    nc.sync.dma_start(out=g1[:], in_=g1_t[t])
                    s_t = sp.tile([P, ECH * DT], bf16, tag="s")
                    nc.sync.dma_start(out=s_t[:], in_=s_tab[t])
                    g1s.append(g1)
                    sts.append(s_t)
                pd = [ps.tile([P, D], f32, tag="ps", name="pd") for _ in range(4)]
                for t2 in range(2):
                    for c in range(ECH):
                        for dh in range(2):
                            nc.tensor.matmul(
                                out=pd[t2 * 2 + dh][:],
                                lhsT=sts[t2][:, c * DT + dh * P : c * DT + (dh + 1) * P],
                                rhs=g1s[t2][:, c * D : (c + 1) * D],
                                start=(c == 0),
                                stop=(c == ECH - 1),
                            )
                for t2 in range(2):
                    for dh in range(2):
                        a1 = mp.tile([P, D], bf16, tag="a1")
                        if dh % 2 == 0:
                            nc.vector.tensor_copy(out=a1[:], in_=pd[t2 * 2 + dh][:])
                        else:
                            nc.scalar.activation(
                                out=a1[:], in_=pd[t2 * 2 + dh][:],
                                func=mybir.ActivationFunctionType.Copy,
                            )
                        nc.sync.dma_start(
                            out=agg1p[
                                (tp * 2 + t2) * DT + dh * P : (tp * 2 + t2) * DT + (dh + 1) * P, :
                            ],
                            in_=a1[:],
                        )

            NOWN = SLAB // P  # 28 chunks of own-slab rows, recomputed locally

            def rec_front(u, src_tab):
                """Gather + transpose chunk u; returns aT tiles."""
                gu = gp.tile([P, D], bf16, tag="gu")
                nc.gpsimd.indirect_dma_start(
                    out=gu[:],
                    out_offset=None,
                    in_=src_tab[:],
                    in_offset=bass.IndirectOffsetOnAxis(ap=idxUall[:, u : u + 1], axis=0),
                )
                aT = []
                for fi in range(NFI1):
                    pt = ps.tile([P, P], bf16, tag="ps", name="pt")
                    nc.tensor.transpose(
                        out=pt[:], in_=gu[:, fi * P : (fi + 1) * P], identity=idt[:]
                    )
                    a = ap.tile([P, P], bf16, tag="aT", name="aTt", bufs=8)
                    if fi % 2 == 0:
                        nc.vector.tensor_copy(out=a[:], in_=pt[:])
                    else:
                        nc.scalar.activation(
                            out=a[:], in_=pt[:],
                            func=mybir.ActivationFunctionType.Copy,
                        )
                    aT.append(a)
                return aT

            def rec_back(u, aT):
                """GEMM + relu + writeback for chunk u."""
                h1u_t = hp.tile([P, H], bf16, tag="hout")
                pz = [ps.tile([P, D], f32, tag="ps", name="pz") for _ in range(NFI1)]
                if use_b1:
                    for fo in range(NFI1):
                        nc.tensor.matmul(
                            out=pz[fo][:],
                            lhsT=ones1[:1, :],
                            rhs=b1sb[:1, fo * D : (fo + 1) * D],
                            start=True,
                            stop=False,
                        )
                for fi in range(NFI1):
                    for fo in range(NFI1):
                        nc.tensor.matmul(
                            out=pz[fo][:],
                            lhsT=aT[fi][:],
                            rhs=w1sb[fi][:, fo * D : (fo + 1) * D],
                            start=(fi == 0 and not use_b1),
                            stop=(fi == NFI1 - 1),
                        )
                for fo in range(NFI1):
                    nc.scalar.activation(
                        out=h1u_t[:, fo * D : (fo + 1) * D], in_=pz[fo][:], func=relu
                    )
                nc.sync.dma_start(out=h1u[u * P : (u + 1) * P, :], in_=h1u_t[:])


            # W2/W3 resident loads: issued after L1's input stream so they
            # ride the AllGather shadow instead of delaying the first dtile.
            w2sb = []
            for fi in range(NFI2):
                w = wp.tile([P, H], bf16, tag="w2", name="w2sb", bufs=NFI2)
                nc.sync.dma_start(out=w[:], in_=w2_t[fi * P : (fi + 1) * P, :])
                w2sb.append(w)
            w3sb = []
            for fo in range(NFI2):
                w = wp.tile([P, B], bf16, tag="w3", name="w3sb", bufs=NFI2)
                nc.sync.dma_start(out=w[:], in_=w3i_t[fo * P : (fo + 1) * P, :])
                w3sb.append(w)

            # software-pipelined: transpose(u) overlaps GEMM(u-1); own-slab
            # chunks (local agg1p) run inside the AllGather shadow, and the
            # first few are issued before the AllGather so the gpsimd queue
            # has work while the collective's input wait resolves.
            prev = None
            for u in range(4):
                aT = rec_front(u, agg1p)
                if prev is not None:
                    rec_back(prev[0], prev[1])
                prev = (u, aT)

            nc.gpsimd.collective_compute(
                "AllGather",
                mybir.AluOpType.bypass,
                replica_groups=rg,
                ins=[agg1p[:]],
                outs=[agg1f[:]],
            )

            for u in range(4, NUCH):
                aT = rec_front(u, agg1p if u < NOWN else agg1f)
                rec_back(prev[0], prev[1])
                prev = (u, aT)
            rec_back(prev[0], prev[1])

            # ---------------- Layer 2 + Q (dtile pairs) --------------------
            NH = NFI2 // 2  # 8 feature chunks per half-row pass
            h1u_half = h1u[:].rearrange("n (h d) -> (n h) d", h=2)
            QS = SLAB // 4

            def q_quarter_ag(j):
                nc.gpsimd.collective_compute(
                    "AllGather",
                    mybir.AluOpType.bypass,
                    replica_groups=rg,
                    ins=[q_slab[j * QS : (j + 1) * QS, :]],
                    outs=[q_full[j * QS * NCORES : (j + 1) * QS * NCORES, :]],
                )

            for tp in range(NDT // 2):
                # issue quarter AllGathers two pairs after their rows complete:
                # the gpsimd queue runs ~a pair ahead of PE, and a collective's
                # SEQ wait stalls every later gather in the queue
                for j in range(2):
                    if tp == ((j + 1) * QS - 1) // (2 * DT) + 2:
                        q_quarter_ag(j)
                aggT = [
                    ap.tile([P, 2 * DT], bf16, tag="aggT2", name="aggTt2", bufs=NFI2)
                    for _ in range(NFI2)
                ]
                for t2 in range(2):
                    t = tp * 2 + t2
                    idx_t = mp.tile([P, 2 * ECH], i32, tag="idx")
                    nc.sync.dma_start(out=idx_t[:], in_=idxL2_t[t])
                    s_t = sp.tile([P, ECH * DT], bf16, tag="s")
                    nc.sync.dma_start(out=s_t[:], in_=s_tab[t])

                    for hf in range(2):
                        pa = [ps.tile([P, DT], f32, tag="ps", name="pa2") for _ in range(NH)]
                        for c in range(ECH):
                            g = gp.tile([P, H // 2], bf16, tag="g")
                            nc.gpsimd.indirect_dma_start(
                                out=g[:],
                                out_offset=None,
                                in_=h1u_half,
                                in_offset=bass.IndirectOffsetOnAxis(
                                    ap=idx_t[:, hf * ECH + c : hf * ECH + c + 1], axis=0
                                ),
                            )
                            for j in range(NH):
                                nc.tensor.matmul(
                                    out=pa[j][:],
                                    lhsT=g[:, j * P : (j + 1) * P],
                                    rhs=s_t[:, c * DT : (c + 1) * DT],
                                    start=(c == 0),
                                    stop=(c == ECH - 1),
                                )
                        for j in range(NH):
                            fi = hf * NH + j
                            if j % 2 == 0:
                                nc.vector.tensor_copy(
                                    out=aggT[fi][:, t2 * DT : (t2 + 1) * DT], in_=pa[j][:]
                                )
                            else:
                                nc.scalar.activation(
                                    out=aggT[fi][:, t2 * DT : (t2 + 1) * DT],
                                    in_=pa[j][:],
                                    func=mybir.ActivationFunctionType.Copy,
                                )

                pq = [ps.tile([P, B], f32, tag="ps", name="pq") for _ in range(4)]
                for fo in range(NFI2):
                    pz = ps.tile([P, 2 * DT], f32, tag="ps", name="pz2")
                    for fi in range(NFI2):
                        nc.tensor.matmul(
                            out=pz[:],
                            lhsT=w2sb[fi][:, fo * P : (fo + 1) * P],
                            rhs=aggT[fi][:],
                            start=(fi == 0),
                            stop=(fi == NFI2 - 1),
                        )
                    h2t = ap.tile([P, 2 * DT], bf16, tag="h2", name="h2t", bufs=NFI2)
                    if use_b2:
                        nc.scalar.activation(
                            out=h2t[:], in_=pz[:], func=relu,
                            bias=b2sb[:, fo : fo + 1],
                        )
                    else:
                        nc.scalar.activation(out=h2t[:], in_=pz[:], func=relu)

                    for dh in range(4):
                        nc.tensor.matmul(
                            out=pq[dh][:],
                            lhsT=h2t[:, dh * P : (dh + 1) * P],
                            rhs=w3sb[fo][:],
                            start=(fo == 0),
                            stop=(fo == NFI2 - 1),
                        )
                for dh in range(4):
                    qn = mp.tile([P, B], bf16, tag="qn")
                    nc.vector.tensor_copy(out=qn[:], in_=pq[dh][:])
                    nc.sync.dma_start(
                        out=q_slab[tp * 2 * DT + dh * P : tp * 2 * DT + (dh + 1) * P, :],
                        in_=qn[:],
                    )



            q_quarter_ag(2)
            q_quarter_ag(3)

            # ---------------- Layer 3 (= output) ---------------------------
            # SBUF fp32 accumulators per dtile; single-shot matmul per chunk +
            # DVE add. Chunk order is A-half-gated chunks (all dtiles) first,
            # so their gathers run under the second Q AllGather, then B chunks.
            idxall = cp.tile([P, NDT * ECH], i32, tag="idxall")
            nc.sync.dma_start(out=idxall[:], in_=idxQ2_t[:])
            acc = [
                ap.tile([B, DT], f32, tag="acc", name="acc3", bufs=NDT)
                for _ in range(NDT)
            ]
            first = [True] * NDT
            ordered = sorted(
                ((t, c) for t in range(NDT) for c in range(ECH)),
                key=lambda tc: gateQ[tc[0]][tc[1]],
            )
            for t, c in ordered:
                s3 = mp.tile([P, DT], bf16, tag="s3", bufs=8)
                nc.sync.dma_start(
                    out=s3[:], in_=s_tab[t][:, c * DT : (c + 1) * DT]
                )
                g = gp.tile([P, B], bf16, tag="g3", bufs=12)
                gq = gateQ[t][c]
                src_ap = q_full[0 : (gq + 1) * (SLAB // 4) * NCORES, :]
                nc.gpsimd.indirect_dma_start(
                    out=g[:],
                    out_offset=None,
                    in_=src_ap,
                    in_offset=bass.IndirectOffsetOnAxis(
                        ap=idxall[:, t * ECH + c : t * ECH + c + 1], axis=0
                    ),
                )
                pa = ps.tile([B, DT], f32, tag="ps", name="pa3")
                nc.tensor.matmul(
                    out=pa[:], lhsT=g[:], rhs=s3[:], start=True, stop=True
                )
                if first[t]:
                    nc.vector.tensor_copy(out=acc[t][:], in_=pa[:])
                    first[t] = False
                else:
                    nc.vector.tensor_tensor(
                        out=acc[t][:], in0=acc[t][:], in1=pa[:],
                        op=mybir.AluOpType.add,
                    )
            for t in range(NDT):
                nc.sync.dma_start(out=out_t[:, t * DT : (t + 1) * DT], in_=acc[t][:])

    nc.finalize()
    return nc


_CACHE: dict = {}


def kernel(**inputs: np.ndarray) -> np.ndarray:
    import ml_dtypes

    nodes = np.asarray(inputs["nodes"], dtype=np.float32)
    edge_index = np.asarray(inputs["edge_index"])
    img = np.asarray(inputs["img"], dtype=np.float32)
    W1 = np.asarray(inputs["W1"], dtype=np.float32)
    b1 = np.asarray(inputs["b1"], dtype=np.float32)
    W2 = np.asarray(inputs["W2"], dtype=np.float32)
    b2 = np.asarray(inputs["b2"], dtype=np.float32)
    W3 = np.asarray(inputs["W3"], dtype=np.float32)
    b3 = np.asarray(inputs["b3"], dtype=np.float32)

    ECH, idxA, idxB, S, gateQ = _preprocess(edge_index)
    S = S.astype(ml_dtypes.bfloat16)
    use_b1 = bool(np.any(b1))
    use_b2 = bool(np.any(b2))

    # per-core source rows: all 3584 own-slab rows first (recomputed locally,
    # hidden under the AllGather), then unique remote rows.
    uniq = []   # [NCORES] arrays of remote agg1f row ids, sorted
    idxL2 = []  # [NCORES][NDT, P, 2*ECH] int32 doubled half-row positions
    for k in range(NCORES):
        own_lo, own_hi = k * SLAB, (k + 1) * SLAB
        rem = np.unique(idxB[k])
        rem = rem[(rem < own_lo) | (rem >= own_hi)]
        uniq.append(rem.astype(np.int32))
        pos_map = np.zeros(NCORES * SLAB, dtype=np.int32)
        pos_map[own_lo:own_hi] = np.arange(SLAB)
        pos_map[rem] = SLAB + np.arange(len(rem))
        posk = pos_map[idxB[k]]  # [NDT, P, ECH]
        idxL2.append(np.concatenate([2 * posk, 2 * posk + 1], axis=2))
    NOWN = SLAB // P
    NUCH = NOWN + max(-(-len(u) // P) for u in uniq)
    # q_full is quarter-major: rows [j*7168:(j+1)*7168) hold quarter j
    # (cores' slab rows j*896..j*896+895, core-major within the quarter)
    q_core = idxB // SLAB
    q_r = idxB % SLAB
    QS = SLAB // 4
    idxQ = (
        (q_r // QS) * (NCORES * QS) + q_core * QS + (q_r % QS)
    ).astype(np.int32)

    key = (ECH, NUCH, gateQ, use_b1, use_b2)
    if key not in _CACHE:
        _CACHE[key] = _build(ECH, NUCH, gateQ, use_b1, use_b2)
    nc = _CACHE[key]

    w3img = (W3.astype(np.float32) @ img.astype(np.float32).T).astype(
        ml_dtypes.bfloat16
    )  # [H, B]
    outbias = img @ b3  # [B]

    nodes_r = nodes.astype(ml_dtypes.bfloat16)
    w1_r = W1.astype(ml_dtypes.bfloat16)
    w2_r = W2.astype(ml_dtypes.bfloat16)
    b1_r = b1.reshape(1, H).astype(ml_dtypes.bfloat16)
    b2_r = _round_fp32r(np.ascontiguousarray(b2.reshape(NFI2, P).T))
    ident = np.eye(P, dtype=ml_dtypes.bfloat16)

    in_maps = []
    for k in range(NCORES):
        g1 = nodes_r[idxA[k]].reshape(NDT, P, ECH * D)
        u_pad = np.zeros(NUCH * P, dtype=np.int32)
        u_pad[:SLAB] = np.arange(SLAB)  # own rows: local agg1p row ids
        u_pad[SLAB : SLAB + len(uniq[k])] = uniq[k]
        in_maps.append(
            {
                "G1": np.ascontiguousarray(g1),
                "W1": w1_r,
                "W2": w2_r,
                "W3img": w3img,
                "b1": b1_r,
                "b2": b2_r,
                "IDENT": ident,
                "idxUT": np.ascontiguousarray(u_pad.reshape(NUCH, P).T),
                "idxL2": np.ascontiguousarray(idxL2[k]),
                "idxQ2": np.ascontiguousarray(
                    idxQ[k].transpose(1, 0, 2).reshape(P, NDT * ECH)
                ),
                "S": np.ascontiguousarray(S[k]),
            }
        )

    res = run_bass_kernel_spmd(nc, in_maps, core_ids=list(range(NCORES)))

    full = np.concatenate([res.results[k]["out"] for k in range(NCORES)], axis=1)
    n_ids = np.arange(N_SKIP, N)
    cols = (n_ids // NODES_PER) * SLAB + (n_ids % NODES_PER)
    out = full[:, cols] + outbias[:, None]
    return out.astype(np.float32)


if __name__ == "__main__":
    rng = np.random.default_rng(0)
    ins = {
        "nodes": rng.standard_normal((N, D)).astype(np.float32),
        "edge_index": rng.integers(0, N, size=(2, E)).astype(np.int64),
        "img": rng.standard_normal((B, D)).astype(np.float32),
        "W1": (rng.standard_normal((D, H)) * 0.02).astype(np.float32),
        "b1": np.zeros(H, np.float32),
        "W2": (rng.standard_normal((H, H)) * 0.02).astype(np.float32),
        "b2": np.zeros(H, np.float32),
        "W3": (rng.standard_normal((H, D)) * 0.02).astype(np.float32),
        "b3": np.zeros(D, np.float32),
    }
    out = kernel(**ins)
    print("out", out.shape, out.dtype, np.abs(out).mean())


# revision 20
# speedup vs baseline: 1.6404x; 1.0030x over previous
"""3-layer GCN + img@pair_embed.T for Trainium2, distributed over 8 NeuronCores.

Strategy (destination-sharded graph parallelism, agg1-exchange variant):
  - Each core owns a contiguous slab of destination nodes (3567, padded 3584).
  - Edges (plus self-loops) are bucketed per 256-destination tile and padded to
    128-edge chunks. Host builds per chunk a dense [128 edges x 256 dests]
    one-hot norm matrix S, so segment-sum aggregation becomes TensorE matmuls.
  - Layer-1 source rows are PRE-GATHERED ON HOST (X is a static input), and the
    layer-1 aggregation computes agg1 = A@X directly in node-row orientation
    (lhsT = S chunk), so agg1 [SLAB, 512] is written without any transpose.
  - KEY: the cross-core exchange moves agg1 (512 wide) instead of h1 (2048
    wide): ONE AllGather of [SLAB,512] -> [8*SLAB,512] (29MB out) instead of
    117MB. Each core then recomputes h1 = relu(agg1 @ W1) for only the unique
    source rows its layer-2/3 edges touch (~13k rows): gather agg1 rows,
    PE-transpose them into contraction layout, GEMM against resident W1.
  - Layer 2 gathers 1024-wide half-rows of the local recomputed h1_u in two
    passes (PSUM has only 8 accumulation banks), GEMMs in dtile pairs
    (free dim 512), and folds img into layer 3: W3img = W3@img.T, Q = h2@W3img.
  - Layer 3 aggregates 64-wide Q after a small Q AllGather.
  - Everything exchanged/gathered travels bf16; W1 float32r; W2/W3img bf16;
    PSUM accumulation fp32.
"""

import numpy as np

from concourse import bacc, bass, mybir
from concourse import tile as tile_mod
from concourse.bass_utils import run_bass_kernel_spmd

# Problem shapes (hardcoded per spec nn_GraphModel_26268019982828)
N = 28535
E = 113000
D = 512
H = 2048
B = 64
N_SKIP = 115 + 245  # attrs + objs; pair nodes are N_SKIP..N-1

NCORES = 8
NODES_PER = -(-N // NCORES)  # 3567
P = 128
DT = 256  # destination tile width
NDT = 14  # dest tiles per core
SLAB = NDT * DT  # 3584 padded dests per core
NFI1 = D // P  # 4 feature chunks of layer-1 width
NFI2 = H // P  # 16 feature chunks of hidden width

f32 = mybir.dt.float32
f32r = mybir.dt.float32r
bf16 = mybir.dt.bfloat16
i32 = mybir.dt.int32


def _round_fp32r(x: np.ndarray) -> np.ndarray:
    """Round-to-nearest-even fp32 -> fp32r (11-bit mantissa), numpy."""
    u = np.ascontiguousarray(x, dtype=np.float32).view(np.uint32)
    r = u + (0x7FF + ((u >> 12) & np.uint32(1)))
    r &= np.uint32(0xFFFFF000)
    return r.view(np.float32)


def _preprocess(edge_index: np.ndarray):
    """Sort/bucket edges by destination; build gather indices + S blocks."""
    src = np.concatenate([edge_index[0], np.arange(N, dtype=np.int64)])
    dst = np.concatenate([edge_index[1], np.arange(N, dtype=np.int64)])
    deg = np.bincount(dst, minlength=N).astype(np.float32)  # includes loops
    dinv = (1.0 / np.sqrt(deg)).astype(np.float32)
    norm = (dinv[src] * dinv[dst]).astype(np.float32)

    core = (dst // NODES_PER).astype(np.int64)
    local = (dst - core * NODES_PER).astype(np.int64)
    t_idx = local // DT
    d_local = local % DT
    bucket = core * NDT + t_idx

    # secondary key: source's q-quarter so layer-3 chunks gate on the
    # earliest quarter AllGather that covers all their sources
    quart = (src % NODES_PER) // (SLAB // 4)
    order = np.argsort(bucket * 4 + quart, kind="stable")
    src_s = src[order]
    bucket_s = bucket[order]
    dl_s = d_local[order]
    norm_s = norm[order]

    counts = np.bincount(bucket_s, minlength=NCORES * NDT)
    ECH = int(-(-counts.max() // P))

    idxA = np.zeros((NCORES, NDT, P, ECH), dtype=np.int32)
    idxB = np.zeros((NCORES, NDT, P, ECH), dtype=np.int32)
    S = np.zeros((NCORES, NDT, P, ECH * DT), dtype=np.float32)

    starts = np.zeros(NCORES * NDT + 1, dtype=np.int64)
    np.cumsum(counts, out=starts[1:])
    pos = np.arange(len(bucket_s)) - starts[bucket_s]
    c_idx = pos // P
    e_idx = pos % P

    ci = bucket_s // NDT
    ti = bucket_s % NDT
    srcB = (src_s // NODES_PER) * SLAB + (src_s % NODES_PER)
    idxA[ci, ti, e_idx, c_idx] = src_s.astype(np.int32)
    idxB[ci, ti, e_idx, c_idx] = srcB.astype(np.int32)
    S[ci, ti, e_idx, c_idx * DT + dl_s] = norm_s
    # gate[t][c] = max source-quarter of chunk c across cores (pads -> 0)
    quart_s = quart[order]
    cnt = counts.reshape(NCORES, NDT)
    qmax = np.zeros((NCORES, NDT, ECH), dtype=np.int64)
    for k in range(NCORES):
        for t in range(NDT):
            b = k * NDT + t
            qs = quart_s[starts[b] : starts[b] + cnt[k, t]]
            for c in range(ECH):
                last = min((c + 1) * P, cnt[k, t]) - 1
                qmax[k, t, c] = qs[last] if last >= c * P else 0
    gate = tuple(
        tuple(int(qmax[:, t, c].max()) for c in range(ECH)) for t in range(NDT)
    )
    return ECH, idxA, idxB, S, gate


def _build(ECH: int, NUCH: int, gateQ, use_b1: bool, use_b2: bool):
    nc = bacc.Bacc("TRN2", target_bir_lowering=False, num_devices=NCORES)
    NU = NUCH * P  # padded unique-source rows per core

    g1_t = nc.dram_tensor("G1", [NDT, P, ECH * D], bf16, kind="ExternalInput")
    w1_t = nc.dram_tensor("W1", [D, H], bf16, kind="ExternalInput")
    w2_t = nc.dram_tensor("W2", [H, H], bf16, kind="ExternalInput")
    w3i_t = nc.dram_tensor("W3img", [H, B], bf16, kind="ExternalInput")
    b1_t = nc.dram_tensor("b1", [1, H], bf16, kind="ExternalInput")
    b2_t = nc.dram_tensor("b2", [P, NFI2], f32r, kind="ExternalInput")
    idxU_t = nc.dram_tensor("idxUT", [P, NUCH], i32, kind="ExternalInput")
    idxL2_t = nc.dram_tensor("idxL2", [NDT, P, 2 * ECH], i32, kind="ExternalInput")
    idxQ2_t = nc.dram_tensor("idxQ2", [P, NDT * ECH], i32, kind="ExternalInput")
    s_tab = nc.dram_tensor("S", [NDT, P, ECH * DT], bf16, kind="ExternalInput")
    ident_t = nc.dram_tensor("IDENT", [P, P], bf16, kind="ExternalInput")
    out_t = nc.dram_tensor("out", [B, SLAB], f32, kind="ExternalOutput")

    agg1p = nc.dram_tensor("agg1p", [SLAB, D], bf16)
    agg1f = nc.dram_tensor("agg1f", [SLAB * NCORES, D], bf16, addr_space="Shared")
    h1u = nc.dram_tensor("h1u", [NU, H], bf16)
    q_slab = nc.dram_tensor("q_slab", [SLAB, B], bf16)
    q_full = nc.dram_tensor("q_full", [SLAB * NCORES, B], bf16, addr_space="Shared")

    rg = [list(range(NCORES))]

    with tile_mod.TileContext(nc) as tc:
        with (
            tc.tile_pool(name="w", bufs=1) as wp,
            tc.tile_pool(name="gio", bufs=6) as gp,
            tc.tile_pool(name="g1io", bufs=2) as g1p,
            tc.tile_pool(name="stab", bufs=3) as sp,
            tc.tile_pool(name="agg", bufs=1) as ap,
            tc.tile_pool(name="small", bufs=3) as mp,
            tc.tile_pool(name="hout", bufs=2) as hp,
            tc.tile_pool(name="consts", bufs=1) as cp,
            tc.tile_pool(name="ps", bufs=8, space="PSUM") as ps,
        ):
            # --- resident constants (W1 loads deferred past layer 1) ---
            idxUall = cp.tile([P, NUCH], i32, tag="idxUall")
            nc.sync.dma_start(out=idxUall[:], in_=idxU_t[:])
            idt = cp.tile([P, P], bf16, tag="idt")
            nc.sync.dma_start(out=idt[:], in_=ident_t[:])
            if use_b1:
                b1sb = cp.tile([1, H], bf16, tag="b1")
                nc.sync.dma_start(out=b1sb[:], in_=b1_t[:])
                ones1 = cp.tile([1, P], bf16, tag="ones")
                nc.gpsimd.memset(ones1[:], 1.0)
            if use_b2:
                b2sb = cp.tile([P, NFI2], f32r, tag="b2")
                nc.sync.dma_start(out=b2sb[:], in_=b2_t[:])

            relu = mybir.ActivationFunctionType.Relu

            # ---------------- Layer 1: agg1 = A@X  (node-row orientation) ---
            for tp in range(NDT // 2):
                g1s, sts = [], []
                for t2 in range(2):
                    t = tp * 2 + t2
                    g1 = g1p.tile([P, ECH * D], bf16, tag="g1")
                    nc.sync.dma_start(out=g1[:], in_=g1_t[t])
                    s_t = sp.tile([P, ECH * DT], bf16, tag="s")
                    nc.sync.dma_start(out=s_t[:], in_=s_tab[t])
                    g1s.append(g1)
                    sts.append(s_t)
                pd = [ps.tile([P, D], f32, tag="ps", name="pd") for _ in range(4)]
                for t2 in range(2):
                    for c in range(ECH):
                        for dh in range(2):
                            nc.tensor.matmul(
                                out=pd[t2 * 2 + dh][:],
                                lhsT=sts[t2][:, c * DT + dh * P : c * DT + (dh + 1) * P],
                                rhs=g1s[t2][:, c * D : (c + 1) * D],
                                start=(c == 0),
                                stop=(c == ECH - 1),
                            )
                for t2 in range(2):
                    for dh in range(2):
                        a1 = mp.tile([P, D], bf16, tag="a1")
                        if dh % 2 == 0:
                            nc.vector.tensor_copy(out=a1[:], in_=pd[t2 * 2 + dh][:])
                        else:
                            nc.scalar.activation(
                                out=a1[:], in_=pd[t2 * 2 + dh][:],
                                func=mybir.ActivationFunctionType.Copy,
                            )
                        nc.sync.dma_start(
                            out=agg1p[
                                (tp * 2 + t2) * DT + dh * P : (tp * 2 + t2) * DT + (dh + 1) * P, :
                            ],
                            in_=a1[:],
                        )

            NOWN = SLAB // P  # 28 chunks of own-slab rows, recomputed locally

            def rec_front(u, src_tab):
                """Gather + transpose chunk u; returns aT tiles."""
                gu = gp.tile([P, D], bf16, tag="gu")
                if u < NOWN:
                    # own rows are contiguous in agg1p: plain DMA, prefetches
                    # as soon as layer 1 writes those rows
                    nc.sync.dma_start(
                        out=gu[:], in_=agg1p[u * P : (u + 1) * P, :]
                    )
                else:
                    nc.gpsimd.indirect_dma_start(
                        out=gu[:],
                        out_offset=None,
                        in_=src_tab[:],
                        in_offset=bass.IndirectOffsetOnAxis(
                            ap=idxUall[:, u : u + 1], axis=0
                        ),
                    )
                aT = []
                for fi in range(NFI1):
                    pt = ps.tile([P, P], bf16, tag="ps", name="pt")
                    nc.tensor.transpose(
                        out=pt[:], in_=gu[:, fi * P : (fi + 1) * P], identity=idt[:]
                    )
                    a = ap.tile([P, P], bf16, tag="aT", name="aTt", bufs=8)
                    if fi % 2 == 0:
                        nc.vector.tensor_copy(out=a[:], in_=pt[:])
                    else:
                        nc.scalar.activation(
                            out=a[:], in_=pt[:],
                            func=mybir.ActivationFunctionType.Copy,
                        )
                    aT.append(a)
                return aT

            def rec_back(u, aT):
                """GEMM + relu + writeback for chunk u."""
                h1u_t = hp.tile([P, H], bf16, tag="hout")
                pz = [ps.tile([P, D], f32, tag="ps", name="pz") for _ in range(NFI1)]
                if use_b1:
                    for fo in range(NFI1):
                        nc.tensor.matmul(
                            out=pz[fo][:],
                            lhsT=ones1[:1, :],
                            rhs=b1sb[:1, fo * D : (fo + 1) * D],
                            start=True,
                            stop=False,
                        )
                for fi in range(NFI1):
                    for fo in range(NFI1):
                        nc.tensor.matmul(
                            out=pz[fo][:],
                            lhsT=aT[fi][:],
                            rhs=w1sb[fi][:, fo * D : (fo + 1) * D],
                            start=(fi == 0 and not use_b1),
                            stop=(fi == NFI1 - 1),
                        )
                for fo in range(NFI1):
                    nc.scalar.activation(
                        out=h1u_t[:, fo * D : (fo + 1) * D], in_=pz[fo][:], func=relu
                    )
                nc.sync.dma_start(out=h1u[u * P : (u + 1) * P, :], in_=h1u_t[:])


            # W2/W3 resident loads: issued after L1's input stream so they
            # ride the AllGather shadow instead of delaying the first dtile.
            w2sb = []
            for fi in range(NFI2):
                w = wp.tile([P, H], bf16, tag="w2", name="w2sb", bufs=NFI2)
                nc.sync.dma_start(out=w[:], in_=w2_t[fi * P : (fi + 1) * P, :])
                w2sb.append(w)
            w3sb = []
            for fo in range(NFI2):
                w = wp.tile([P, B], bf16, tag="w3", name="w3sb", bufs=NFI2)
                nc.sync.dma_start(out=w[:], in_=w3i_t[fo * P : (fo + 1) * P, :])
                w3sb.append(w)

            w1sb = []
            for fi in range(NFI1):
                w = wp.tile([P, H], bf16, tag="w1", name="wsb", bufs=NFI1)
                nc.sync.dma_start(out=w[:], in_=w1_t[fi * P : (fi + 1) * P, :])
                w1sb.append(w)

            # software-pipelined: transpose(u) overlaps GEMM(u-1); own-slab
            # chunks (local agg1p) run inside the AllGather shadow, and the
            # first few are issued before the AllGather so the gpsimd queue
            # has work while the collective's input wait resolves.
            prev = None
            for u in range(4):
                aT = rec_front(u, agg1p)
                if prev is not None:
                    rec_back(prev[0], prev[1])
                prev = (u, aT)

            nc.gpsimd.collective_compute(
                "AllGather",
                mybir.AluOpType.bypass,
                replica_groups=rg,
                ins=[agg1p[:]],
                outs=[agg1f[:]],
            )

            for u in range(4, NUCH):
                aT = rec_front(u, agg1p if u < NOWN else agg1f)
                rec_back(prev[0], prev[1])
                prev = (u, aT)
            rec_back(prev[0], prev[1])

            # ---------------- Layer 2 + Q (dtile pairs) --------------------
            NH = NFI2 // 2  # 8 feature chunks per half-row pass
            h1u_half = h1u[:].rearrange("n (h d) -> (n h) d", h=2)
            QS = SLAB // 4

            def q_quarter_ag(j):
                nc.gpsimd.collective_compute(
                    "AllGather",
                    mybir.AluOpType.bypass,
                    replica_groups=rg,
                    ins=[q_slab[j * QS : (j + 1) * QS, :]],
                    outs=[q_full[j * QS * NCORES : (j + 1) * QS * NCORES, :]],
                )

            for tp in range(NDT // 2):
                # issue quarter AllGathers two pairs after their rows complete:
                # the gpsimd queue runs ~a pair ahead of PE, and a collective's
                # SEQ wait stalls every later gather in the queue
                for j in range(2):
                    if tp == ((j + 1) * QS - 1) // (2 * DT) + 3:
                        q_quarter_ag(j)
                aggT = [
                    ap.tile([P, 2 * DT], bf16, tag="aggT2", name="aggTt2", bufs=NFI2)
                    for _ in range(NFI2)
                ]
                for t2 in range(2):
                    t = tp * 2 + t2
                    idx_t = mp.tile([P, 2 * ECH], i32, tag="idx")
                    nc.sync.dma_start(out=idx_t[:], in_=idxL2_t[t])
                    s_t = sp.tile([P, ECH * DT], bf16, tag="s")
                    nc.sync.dma_start(out=s_t[:], in_=s_tab[t])

                    for hf in range(2):
                        pa = [ps.tile([P, DT], f32, tag="ps", name="pa2") for _ in range(NH)]
                        for c in range(ECH):
                            g = gp.tile([P, H // 2], bf16, tag="g")
                            nc.gpsimd.indirect_dma_start(
                                out=g[:],
                                out_offset=None,
                                in_=h1u_half,
                                in_offset=bass.IndirectOffsetOnAxis(
                                    ap=idx_t[:, hf * ECH + c : hf * ECH + c + 1], axis=0
                                ),
                            )
                            for j in range(NH):
                                nc.tensor.matmul(
                                    out=pa[j][:],
                                    lhsT=g[:, j * P : (j + 1) * P],
                                    rhs=s_t[:, c * DT : (c + 1) * DT],
                                    start=(c == 0),
                                    stop=(c == ECH - 1),
                                )
                        for j in range(NH):
                            fi = hf * NH + j
                            if j % 2 == 0:
                                nc.vector.tensor_copy(
                                    out=aggT[fi][:, t2 * DT : (t2 + 1) * DT], in_=pa[j][:]
                                )
                            else:
                                nc.scalar.activation(
                                    out=aggT[fi][:, t2 * DT : (t2 + 1) * DT],
                                    in_=pa[j][:],
                                    func=mybir.ActivationFunctionType.Copy,
                                )

                pq = [ps.tile([P, B], f32, tag="ps", name="pq") for _ in range(4)]
                for fo in range(NFI2):
                    pz = ps.tile([P, 2 * DT], f32, tag="ps", name="pz2")
                    for fi in range(NFI2):
                        nc.tensor.matmul(
                            out=pz[:],
                            lhsT=w2sb[fi][:, fo * P : (fo + 1) * P],
                            rhs=aggT[fi][:],
                            start=(fi == 0),
                            stop=(fi == NFI2 - 1),
                        )
                    h2t = ap.tile([P, 2 * DT], bf16, tag="h2", name="h2t", bufs=NFI2)
                    if use_b2:
                        nc.scalar.activation(
                            out=h2t[:], in_=pz[:], func=relu,
                            bias=b2sb[:, fo : fo + 1],
                        )
                    else:
                        nc.scalar.activation(out=h2t[:], in_=pz[:], func=relu)

                    for dh in range(4):
                        nc.tensor.matmul(
                            out=pq[dh][:],
                            lhsT=h2t[:, dh * P : (dh + 1) * P],
                            rhs=w3sb[fo][:],
                            start=(fo == 0),
                            stop=(fo == NFI2 - 1),
                        )
                for dh in range(4):
                    qn = mp.tile([P, B], bf16, tag="qn")
                    nc.vector.tensor_copy(out=qn[:], in_=pq[dh][:])
                    nc.sync.dma_start(
                        out=q_slab[tp * 2 * DT + dh * P : tp * 2 * DT + (dh + 1) * P, :],
                        in_=qn[:],
                    )



            q_quarter_ag(2)
            q_quarter_ag(3)

            # ---------------- Layer 3 (= output) ---------------------------
            # SBUF fp32 accumulators per dtile; single-shot matmul per chunk +
            # DVE add. Chunk order is A-half-gated chunks (all dtiles) first,
            # so their gathers run under the second Q AllGather, then B chunks.
            idxall = cp.tile([P, NDT * ECH], i32, tag="idxall")
            nc.sync.dma_start(out=idxall[:], in_=idxQ2_t[:])
            acc = [
                ap.tile([B, DT], f32, tag="acc", name="acc3", bufs=NDT)
                for _ in range(NDT)
            ]
            first = [True] * NDT
            ordered = sorted(
                ((t, c) for t in range(NDT) for c in range(ECH)),
                key=lambda tc: gateQ[tc[0]][tc[1]],
            )
            for t, c in ordered:
                s3 = mp.tile([P, DT], bf16, tag="s3", bufs=8)
                nc.sync.dma_start(
                    out=s3[:], in_=s_tab[t][:, c * DT : (c + 1) * DT]
                )
                g = gp.tile([P, B], bf16, tag="g3", bufs=12)
                gq = gateQ[t][c]
                src_ap = q_full[0 : (gq + 1) * (SLAB // 4) * NCORES, :]
                nc.gpsimd.indirect_dma_start(
                    out=g[:],
                    out_offset=None,
                    in_=src_ap,
                    in_offset=bass.IndirectOffsetOnAxis(
                        ap=idxall[:, t * ECH + c : t * ECH + c + 1], axis=0
                    ),
                )
                pa = ps.tile([B, DT], f32, tag="ps", name="pa3")
                nc.tensor.matmul(
                    out=pa[:], lhsT=g[:], rhs=s3[:], start=True, stop=True
                )
                if first[t]:
                    nc.vector.tensor_copy(out=acc[t][:], in_=pa[:])
                    first[t] = False
                else:
                    nc.vector.tensor_tensor(
                        out=acc[t][:], in0=acc[t][:], in1=pa[:],
                        op=mybir.AluOpType.add,
                    )
            for t in range(NDT):
                nc.sync.dma_start(out=out_t[:, t * DT : (t + 1) * DT], in_=acc[t][:])

    nc.finalize()
    return nc


_CACHE: dict = {}


def kernel(**inputs: np.ndarray) -> np.ndarray:
    import ml_dtypes

    nodes = np.asarray(inputs["nodes"], dtype=np.float32)
    edge_index = np.asarray(inputs["edge_index"])
    img = np.asarray(inputs["img"], dtype=np.float32)
    W1 = np.asarray(inputs["W1"], dtype=np.float32)
    b1 = np.asarray(inputs["b1"], dtype=np.float32)
    W2 = np.asarray(inputs["W2"], dtype=np.float32)
    b2 = np.asarray(inputs["b2"], dtype=np.float32)
    W3 = np.asarray(inputs["W3"], dtype=np.float32)
    b3 = np.asarray(inputs["b3"], dtype=np.float32)

    ECH, idxA, idxB, S, gateQ = _preprocess(edge_index)
    S = S.astype(ml_dtypes.bfloat16)
    use_b1 = bool(np.any(b1))
    use_b2 = bool(np.any(b2))

    # per-core source rows: all 3584 own-slab rows first (recomputed locally,
    # hidden under the AllGather), then unique remote rows.
    uniq = []   # [NCORES] arrays of remote agg1f row ids, sorted
    idxL2 = []  # [NCORES][NDT, P, 2*ECH] int32 doubled half-row positions
    for k in range(NCORES):
        own_lo, own_hi = k * SLAB, (k + 1) * SLAB
        rem = np.unique(idxB[k])
        rem = rem[(rem < own_lo) | (rem >= own_hi)]
        uniq.append(rem.astype(np.int32))
        pos_map = np.zeros(NCORES * SLAB, dtype=np.int32)
        pos_map[own_lo:own_hi] = np.arange(SLAB)
        pos_map[rem] = SLAB + np.arange(len(rem))
        posk = pos_map[idxB[k]]  # [NDT, P, ECH]
        idxL2.append(np.concatenate([2 * posk, 2 * posk + 1], axis=2))
    NOWN = SLAB // P
    NUCH = NOWN + max(-(-len(u) // P) for u in uniq)
    # q_full is quarter-major: rows [j*7168:(j+1)*7168) hold quarter j
    # (cores' slab rows j*896..j*896+895, core-major within the quarter)
    q_core = idxB // SLAB
    q_r = idxB % SLAB
    QS = SLAB // 4
    idxQ = (
        (q_r // QS) * (NCORES * QS) + q_core * QS + (q_r % QS)
    ).astype(np.int32)

    key = (ECH, NUCH, gateQ, use_b1, use_b2)
    if key not in _CACHE:
        _CACHE[key] = _build(ECH, NUCH, gateQ, use_b1, use_b2)
    nc = _CACHE[key]

    w3img = (W3.astype(np.float32) @ img.astype(np.float32).T).astype(
        ml_dtypes.bfloat16
    )  # [H, B]
    outbias = img @ b3  # [B]

    nodes_r = nodes.astype(ml_dtypes.bfloat16)
    w1_r = W1.astype(ml_dtypes.bfloat16)
    w2_r = W2.astype(ml_dtypes.bfloat16)
    b1_r = b1.reshape(1, H).astype(ml_dtypes.bfloat16)
    b2_r = _round_fp32r(np.ascontiguousarray(b2.reshape(NFI2, P).T))
    ident = np.eye(P, dtype=ml_dtypes.bfloat16)

    in_maps = []
    for k in range(NCORES):
        g1 = nodes_r[idxA[k]].reshape(NDT, P, ECH * D)
        u_pad = np.zeros(NUCH * P, dtype=np.int32)
        u_pad[:SLAB] = np.arange(SLAB)  # own rows: local agg1p row ids
        u_pad[SLAB : SLAB + len(uniq[k])] = uniq[k]
        in_maps.append(
            {
                "G1": np.ascontiguousarray(g1),
                "W1": w1_r,
                "W2": w2_r,
                "W3img": w3img,
                "b1": b1_r,
                "b2": b2_r,
                "IDENT": ident,
                "idxUT": np.ascontiguousarray(u_pad.reshape(NUCH, P).T),
                "idxL2": np.ascontiguousarray(idxL2[k]),
                "idxQ2": np.ascontiguousarray(
                    idxQ[k].transpose(1, 0, 2).reshape(P, NDT * ECH)
                ),
                "S": np.ascontiguousarray(S[k]),
            }
        )

    res = run_bass_kernel_spmd(nc, in_maps, core_ids=list(range(NCORES)))

    full = np.concatenate([res.results[k]["out"] for k in range(NCORES)], axis=1)
    n_ids = np.arange(N_SKIP, N)
    cols = (n_ids // NODES_PER) * SLAB + (n_ids % NODES_PER)
    out = full[:, cols] + outbias[:, None]
    return out.astype(np.float32)


if __name__ == "__main__":
    rng = np.random.default_rng(0)
    ins = {
        "nodes": rng.standard_normal((N, D)).astype(np.float32),
        "edge_index": rng.integers(0, N, size=(2, E)).astype(np.int64),
        "img": rng.standard_normal((B, D)).astype(np.float32),
        "W1": (rng.standard_normal((D, H)) * 0.02).astype(np.float32),
        "b1": np.zeros(H, np.float32),
        "W2": (rng.standard_normal((H, H)) * 0.02).astype(np.float32),
        "b2": np.zeros(H, np.float32),
        "W3": (rng.standard_normal((H, D)) * 0.02).astype(np.float32),
        "b3": np.zeros(D, np.float32),
    }
    out = kernel(**ins)
    print("out", out.shape, out.dtype, np.abs(out).mean())
